# revision 1
# baseline (speedup 1.0000x reference)
"""Two-layer GAT on 8 Trainium2 NeuronCores (Bass/Tile) — v2.

Changes vs v1 (889us):
  - Phase A sharded: each core computes T1 rows for its own 6250 nodes
    (padded to 6272), AllGather broadcasts the table (timing build: local
    copy of own contribution only).
  - b1 folded into T1 value columns (sum(alpha)=1), b2 and the elu "-1"
    folded into W2ext/b2eff at finalize.
  - selT (dst-onehot, d-on-partitions) shipped from host as fp8 and used
    directly as matmul lhsT (mixed fp8 x f16 matmul) — kills the dlT DMA,
    the DVE selT build, and the adf path stays cheap.
  - exp broadcast stays on ACT; leaky/et on DVE; PSUM->SBUF copies and
    t2r/adst2 finalization moved to Pool (gpsimd).
  - softmax denominator accumulated via a second per-chunk matmul with a
    strided rhs view of exx (no den-copy op).
  - asrc2/adst2 computed via the h2 matmul itself (W2ext has v_s|v_d cols).
"""

import os

import numpy as np

import concourse.bass as bass
import concourse.bacc as bacc
import concourse.tile as tile
import concourse.mybir as mybir
from concourse.bass_utils import run_bass_kernel_spmd
from concourse.masks import make_identity

F32 = mybir.dt.float32
F16 = mybir.dt.float16
F8 = mybir.dt.float8e4
I16 = mybir.dt.int16
I32 = mybir.dt.int32
A = mybir.AluOpType
AF = mybir.ActivationFunctionType
NP_F8 = mybir.dt.np(F8)

# -------- problem constants --------
N, E, IN, HID, OUT, H = 50000, 800000, 128, 32, 64, 8
C1 = H * HID  # 256
NCORES = 8
NPC = N // NCORES        # 6250 dst nodes per core
RPC = 6272               # T1 rows per core (6250 padded to 49*128)
NR = NCORES * RPC        # 50176 T1 rows
T1_LO = 4 * RPC          # 25088: rows of cores 0-3
T1_W = 384               # T1 row stride (768B) — gather granularity
CHL = 9                  # chunks per (tile, half)
CH = 2 * CHL             # chunks per gather call
CALLW = CH * 128         # 2304 edge slots per call
EPS = 1e-16
NEG = 0.2


def _row_of(n):
    """T1 row of node n (cores own contiguous 6250-node ranges, padded)."""
    c = n // NPC
    return c * RPC + (n - c * NPC)


# ---------------------------------------------------------------------------
# host-side preprocessing
# ---------------------------------------------------------------------------

def _prep_weights(W1, as1, ad1, b1, W2, as2, ad2, b2):
    As = np.zeros((C1, H), np.float32)
    Ad = np.zeros((C1, H), np.float32)
    for h in range(H):
        As[h * HID:(h + 1) * HID, h] = as1[h]
        Ad[h * HID:(h + 1) * HID, h] = ad1[h]
    W1ext = np.concatenate([W1, W1 @ As, W1 @ Ad], axis=1)  # [128, 272]
    iotarep = np.zeros((128, 128 * CH), np.float16)
    for d in range(128):
        iotarep[:, d * CH:(d + 1) * CH] = d
    b1ext = np.zeros((272,), np.float32)
    b1ext[:C1] = b1
    vs = W2 @ as2[0]   # [256]
    vd = W2 @ ad2[0]   # [256]
    W2ext = np.concatenate([W2, vs[:, None], vd[:, None]], axis=1)  # [256,66]
    b2eff = np.concatenate([b2 - W2.sum(0), [-vs.sum()], [-vd.sum()]])  # [66]
    return {
        "W1ext": W1ext.astype(np.float16),
        "W2ext": W2ext.astype(np.float16),
        "b1ext": np.tile(b1ext[None, :], (128, 1)).astype(np.float16),
        "b2e66": np.tile(b2eff[None, :], (128, 1)).astype(np.float16),
        "iotarep": iotarep,
    }


def _greedy_tiles(deg_lo1, deg_hi1, deg_lo2, deg_hi2):
    cap = CHL * 128
    tiles = []
    i, n = 0, len(deg_lo1)
    while i < n:
        l1 = h1 = l2 = h2 = 0
        j = i
        while j < n and j - i < 128:
            nl1, nh1 = l1 + deg_lo1[j], h1 + deg_hi1[j]
            nl2, nh2 = l2 + deg_lo2[j], h2 + deg_hi2[j]
            if nl1 > cap or nh1 > cap or nl2 > cap or nh2 > cap:
                break
            l1, h1, l2, h2 = nl1, nh1, nl2, nh2
            j += 1
        assert j > i, "single node exceeds chunk caps"
        tiles.append((i, j))
        i = j
    return tiles


def _pack_calls(rows_half, dloc_half, T, both=False):
    """Build gather idx [128, T*CH*8] i16, dloc [128, T*CH] f16, and the
    fp8 one-hot stream: selT only [128, T*CH*128], or selT|sel interleaved
    per call [128, T*2*CH*128] when both=True."""
    idx16 = np.zeros((128, T * CH * 8), np.int16)
    dloc = np.full((128, T * CH), -1, np.float16)
    dlocT_flat = np.full((T * CALLW,), -1, np.int32)
    for pr in range(T // 2):
        for h in (0, 1):
            g = 2 * pr + h
            rows = np.zeros((CALLW,), np.int64)
            dl = np.full((CALLW,), -1, np.int64)
            for k, t in enumerate((2 * pr, 2 * pr + 1)):
                r = rows_half.get((t, h))
                if r is None:
                    continue
                d = dloc_half[(t, h)]
                off = k * CHL * 128
                rows[off:off + len(r)] = r
                dl[off:off + len(r)] = d
            blk = rows.reshape(CH * 8, 16).T.astype(np.int16)
            idx16[:, g * CH * 8:(g + 1) * CH * 8] = np.tile(blk, (8, 1))
            dloc[:, g * CH:(g + 1) * CH] = dl.reshape(CH, 128).T.astype(np.float16)
            dlocT_flat[g * CALLW:(g + 1) * CALLW] = dl
    # selT[p, g*CH*128 + j*128 + e] = 1.0 iff dloc(edge (j,e) of g) == p
    selT = (dlocT_flat[None, :] == np.arange(128)[:, None])
    if not both:
        return idx16, dloc, selT.astype(NP_F8)
    # sel[p, g, j, d] = 1.0 iff dloc(edge (p,j) of g) == d
    sel = (dloc[:, :, None].astype(np.int32) ==
           np.arange(128)[None, None, :]).reshape(128, T, CH, 128)
    selB = np.stack([selT.reshape(128, T, CH, 128), sel], axis=2).reshape(
        128, T * 2 * CH * 128).astype(NP_F8)
    return idx16, dloc, selB


def _prep_core(c, src, dst):
    base = c * NPC
    own = (dst >= base) & (dst < base + NPC)
    s = src[own].astype(np.int64)
    d = (dst[own] - base).astype(np.int64)
    order = np.argsort(d, kind="stable")
    s, d = s[order], d[order]
    ptr = np.zeros(NPC + 1, np.int64)
    np.cumsum(np.bincount(d, minlength=NPC), out=ptr[1:])

    srow = _row_of(s)
    lo1m = srow < T1_LO
    lo2m = s < (N // 2)  # slot-major half split: cores 0-3 vs 4-7
    deg_lo1 = np.bincount(d, weights=lo1m, minlength=NPC).astype(np.int64)
    deg_hi1 = np.bincount(d, weights=~lo1m, minlength=NPC).astype(np.int64)
    deg_lo2 = np.bincount(d, weights=lo2m, minlength=NPC).astype(np.int64)
    deg_hi2 = np.bincount(d, weights=~lo2m, minlength=NPC).astype(np.int64)
    tiles = _greedy_tiles(deg_lo1, deg_hi1, deg_lo2, deg_hi2)

    rows1, dloc1 = {}, {}
    for t, (n0, n1) in enumerate(tiles):
        e0, e1 = ptr[n0], ptr[n1]
        es, ed = srow[e0:e1], d[e0:e1]
        dl = ed - n0
        m1 = es < T1_LO
        rows1[(t, 0)] = es[m1]
        dloc1[(t, 0)] = dl[m1]
        rows1[(t, 1)] = es[~m1] - T1_LO
        dloc1[(t, 1)] = dl[~m1]
    return {
        "Treal": len(tiles), "tiles": tiles, "s": s, "d": d, "ptr": ptr,
        "rows1": rows1, "dloc1": dloc1,
    }


def _finish_core(pc, c, T, slotrow):
    tiles = list(pc["tiles"]) + [(0, 0)] * (T - pc["Treal"])
    idx1, dloc1, selT1 = _pack_calls(pc["rows1"], pc["dloc1"], T, both=True)

    t2lo = 4 * T * 128
    s, d, ptr = pc["s"], pc["d"], pc["ptr"]
    srow = slotrow[s]
    rows2, dloc2 = {}, {}
    for t, (n0, n1) in enumerate(pc["tiles"]):
        e0, e1 = ptr[n0], ptr[n1]
        dl = d[e0:e1] - n0
        m2 = srow[e0:e1] < t2lo
        rows2[(t, 0)] = srow[e0:e1][m2]
        dloc2[(t, 0)] = dl[m2]
        rows2[(t, 1)] = srow[e0:e1][~m2] - t2lo
        dloc2[(t, 1)] = dl[~m2]
    idx2, dloc2a, selT2 = _pack_calls(rows2, dloc2, T)

    # adidx: local t1slice rows of each tile's nodes (slot-major, clamped),
    # in dma_gather int16 index layout, one call of T*128 rows
    p = np.arange(128)
    rows = np.zeros((T * 128,), np.int64)
    for t, (n0, n1) in enumerate(tiles):
        w = n1 - n0
        rows[t * 128:(t + 1) * 128] = n0 + np.minimum(p, max(w - 1, 0))
    blk = rows.reshape(T * 8, 16).T.astype(np.int16)
    adidx = np.tile(blk, (8, 1))  # [128, T*8]
    return {
        "g1idx": idx1, "dloc1": dloc1, "selT1": selT1,
        "g2idx": idx2, "dloc2": dloc2a, "selT2": selT2,
        "adidx": adidx,
    }


def host_prep(inputs):
    ei = np.asarray(inputs["edge_index"]).astype(np.int64)
    wd = _prep_weights(
        np.asarray(inputs["W1"], np.float32),
        np.asarray(inputs["att_src1"], np.float32),
        np.asarray(inputs["att_dst1"], np.float32),
        np.asarray(inputs["b1"], np.float32),
        np.asarray(inputs["W2"], np.float32),
        np.asarray(inputs["att_src2"], np.float32),
        np.asarray(inputs["att_dst2"], np.float32),
        np.asarray(inputs["b2"], np.float32),
    )
    loops = np.arange(N, dtype=np.int64)
    src = np.concatenate([ei[0], loops])
    dst = np.concatenate([ei[1], loops])

    x = np.asarray(inputs["x"], np.float32).astype(np.float16)

    cores = [_prep_core(c, src, dst) for c in range(NCORES)]
    T = max(pc["Treal"] for pc in cores)
    if T % 2:
        T += 1
    slotrow = np.zeros(N, np.int64)
    for c, pc in enumerate(cores):
        base_row = c * T * 128
        for t, (n0, n1) in enumerate(pc["tiles"]):
            nodes = c * NPC + np.arange(n0, n1)
            slotrow[nodes] = base_row + t * 128 + np.arange(n1 - n0)
    per_core = [_finish_core(pc, c, T, slotrow) for c, pc in enumerate(cores)]
    for c in range(NCORES):
        xc = np.zeros((IN, RPC), np.float16)
        xc[:, :NPC] = x[c * NPC:(c + 1) * NPC].T
        per_core[c]["xTc"] = xc

    common = dict(wd)
    common["slotrow"] = slotrow
    return T, common, per_core


# ---------------------------------------------------------------------------
# device program
# ---------------------------------------------------------------------------

def _gather_raw(eng, out_ap, in_ap, idxs_ap, num_idxs, elem_size, elem_step):
    """dma_gather with elem_size_bytes not a multiple of 256B (non-transpose
    path only; the 256B rule is a transpose-mode restriction — the Q7 kernel
    packets arbitrary elem sizes, only the row stride is encoded in 256B
    units).  Mirrors BassGpSimd.dma_gather's construction."""
    from concourse.ap_utils import ap_is_contiguous
    import concourse.mybir as mb
    assert idxs_ap.dtype == mybir.dt.int16
    assert in_ap.dtype == out_ap.dtype
    elem_size_bytes = elem_size * mybir.dt.size(in_ap.dtype)
    assert in_ap.ap[-1][1] == out_ap.ap[-1][1] == elem_size
    assert ap_is_contiguous(out_ap.ap[1:])
    assert ap_is_contiguous(idxs_ap.ap[1:])
    assert in_ap.ap[0][0] == elem_step
    stride_bytes = elem_step * mybir.dt.size(in_ap.dtype)
    assert stride_bytes % 256 == 0 and stride_bytes // 256 < 256
    _in_ap = eng.lower_ap_dma(in_ap, for_custom_bir_dma=True)
    _idxs_ap = eng.lower_ap(idxs_ap)
    _out_ap = eng.lower_ap(out_ap)
    return eng.add_instruction(
        mb.InstDMAGatherAnt(
            name=eng.bass.get_next_instruction_name(),
            ins=[*_in_ap, _idxs_ap,
                 eng.lower_val_access(eng.to_reg(num_idxs))],
            outs=[_out_ap],
            transpose=False,
            num_idxs=num_idxs,
            elem_size=elem_size,
            stride_bytes_256=stride_bytes // 256,
            gen_mode=0,
            single_packet=False,
            queue_num=0,
            sbuf_tokens_per_rank=0,
            sbuf_free_dim_per_rank=0,
            sbuf_free_dim_pad_per_rank=0,
            sbuf_byte_offset=0,
        )
    )


def build_nc(T, num_devices=NCORES, with_collective=True, phases="ABCD",
             dbg=False):
    nc = bacc.Bacc("TRN2", target_bir_lowering=False, debug=False,
                   num_devices=num_devices)
    dt = nc.dram_tensor
    xTc = dt("xTc", [IN, RPC], F16, kind="ExternalInput").ap()
    W1ext = dt("W1ext", [128, 272], F16, kind="ExternalInput").ap()
    W2ext = dt("W2ext", [256, 66], F16, kind="ExternalInput").ap()
    b1ext = dt("b1ext", [128, 272], F16, kind="ExternalInput").ap()
    b2e66 = dt("b2e66", [128, 66], F16, kind="ExternalInput").ap()
    iotarep = dt("iotarep", [128, 128 * CH], F16, kind="ExternalInput").ap()
    g1idx = dt("g1idx", [128, T * CH * 8], I16, kind="ExternalInput").ap()
    g2idx = dt("g2idx", [128, T * CH * 8], I16, kind="ExternalInput").ap()
    dloc1 = dt("dloc1", [128, T * CH], F16, kind="ExternalInput").ap()
    dloc2 = dt("dloc2", [128, T * CH], F16, kind="ExternalInput").ap()
    selT1 = dt("selT1", [128, T * 2 * CH * 128], F8, kind="ExternalInput").ap()
    selT2 = dt("selT2", [128, T * CH * 128], F8, kind="ExternalInput").ap()
    adidx = dt("adidx", [128, T * 8], I16, kind="ExternalInput").ap()
    t1slice = dt("t1slice", [RPC, T1_W], F16, kind="Internal").ap()
    T1 = dt("T1", [NR, T1_W], F16, kind="Internal",
            addr_space="Shared" if with_collective else "Local").ap()
    t2rows = T * 128
    t2slice = dt("t2slice", [t2rows, 128], F16, kind="Internal").ap()
    t2full = dt("t2full", [NCORES * t2rows, 128], F16, kind="Internal",
                addr_space="Shared" if with_collective else "Local").ap()
    outp = dt("out", [t2rows, 64], F32, kind="ExternalOutput").ap()
    if dbg:
        t1dbg = dt("t1dbg", [RPC, T1_W], F16, kind="ExternalOutput").ap()
        addbg = dt("addbg", [128, T * 8], F16, kind="ExternalOutput").ap()
        t2dbg = dt("t2dbg", [t2rows, 128], F16, kind="ExternalOutput").ap()
        gtdbg = dt("gtdbg", [128, CH * 264], F16, kind="ExternalOutput").ap()
        etdbg = dt("etdbg", [128, CH * 8], F16, kind="ExternalOutput").ap()
        wdbg = dt("wdbg", [128, CH * 256], F16, kind="ExternalOutput").ap()
        lkdbg = dt("lkdbg", [128, CH * 8], F16, kind="ExternalOutput").ap()
        exxdbg = dt("exxdbg", [128, CH * 256], F16,
                    kind="ExternalOutput").ap()
        h1dbg = dt("h1dbg", [128, 256 + 8], F32, kind="ExternalOutput").ap()
        nc._dbg = dict(gtdbg=gtdbg, etdbg=etdbg, wdbg=wdbg, h1dbg=h1dbg,
                       lkdbg=lkdbg, exxdbg=exxdbg)
    else:
        nc._dbg = None

    with tile.TileContext(nc) as tc:
        with tc.tile_pool(name="consts", bufs=1) as cp:
            W1e_sb = cp.tile([128, 272], F16)
            nc.sync.dma_start(out=W1e_sb[:], in_=W1ext[:])
            W2a_sb = cp.tile([128, 66], F16)
            nc.sync.dma_start(out=W2a_sb[:], in_=W2ext[0:128, :])
            W2b_sb = cp.tile([128, 66], F16)
            nc.sync.dma_start(out=W2b_sb[:], in_=W2ext[128:256, :])
            b1_sb = cp.tile([128, 272], F16)
            nc.sync.dma_start(out=b1_sb[:], in_=b1ext[:])
            b2_sb = cp.tile([128, 66], F16)
            nc.sync.dma_start(out=b2_sb[:], in_=b2e66[:])
            oneall = cp.tile([128, 128], F16)
            nc.vector.memset(oneall[:], 1.0 / 128.0)
            iot_sb = cp.tile([128, 128 * CH], F16)
            nc.sync.dma_start(out=iot_sb[:], in_=iotarep[:])
            dl1_sb = cp.tile([128, T * CH], F16)
            nc.sync.dma_start(out=dl1_sb[:], in_=dloc1[:])
            dl2_sb = cp.tile([128, T * CH], F16)
            nc.sync.dma_start(out=dl2_sb[:], in_=dloc2[:])
            idn = cp.tile([128, 128], F16)
            make_identity(nc, idn[:])
            g1i_sb = cp.tile([128, T * CH * 8], I16)
            nc.sync.dma_start(out=g1i_sb[:], in_=g1idx[:])
            dl1_sb = cp.tile([128, T * CH], F16)
            nc.sync.dma_start(out=dl1_sb[:], in_=dloc1[:])
            g2i_sb = cp.tile([128, T * CH * 8], I16)
            nc.sync.dma_start(out=g2i_sb[:], in_=g2idx[:])
            adidx_sb = cp.tile([128, T * 8], I16)
            nc.sync.dma_start(out=adidx_sb[:], in_=adidx[:])
            adtall_sb = cp.tile([128, T, 8], F16)  # bulk a_dst gather target
            adst2_sb = cp.tile([128, T], F16)  # written in B-fin, read in D

            # ---------------- Phase A: own T1 slice ----------------
            if "A" in phases:
                with tc.tile_pool(name="pa", bufs=2) as pa, \
                     tc.tile_pool(name="paps", bufs=4, space="PSUM") as paps:
                    XB = 2048
                    nblk = (RPC + XB - 1) // XB
                    for blk in range(nblk):
                        n0 = blk * XB
                        bw = min(XB, RPC - n0)
                        nt = bw // 128
                        xb = pa.tile([128, XB], F16, tag="xb", name="xb")
                        nc.sync.dma_start(out=xb[:, 0:bw],
                                          in_=xTc[:, n0:n0 + bw])
                        t1b = pa.tile([128, 16, 272], F16, tag="t1b",
                                      name="t1b")
                        for i in range(nt):
                            ps = paps.tile([128, 272], F32, tag="aps",
                                           name="aps")
                            nc.tensor.matmul(ps[:],
                                             lhsT=xb[:, i * 128:(i + 1) * 128],
                                             rhs=W1e_sb[:], start=True,
                                             stop=False)
                            nc.tensor.matmul(ps[:], lhsT=oneall[:],
                                             rhs=b1_sb[:], start=False,
                                             stop=True)
                            if i % 2 == 0:
                                nc.vector.tensor_copy(t1b[:, i, :], ps[:])
                            else:
                                nc.scalar.copy(t1b[:, i, :], ps[:])
                        nc.sync.dma_start(
                            out=t1slice[n0:n0 + bw, 0:272].rearrange(
                                "(i p) c -> p i c", p=128),
                            in_=t1b[:, 0:nt, :])
                        if not with_collective and "B" in phases:
                            nc.sync.dma_start(
                                out=T1[n0:n0 + bw, 0:272],
                                in_=t1slice[n0:n0 + bw, 0:272])

            # ---------------- AllGather T1 ----------------
            if "B" in phases:
                if with_collective:
                    nc.gpsimd.collective_compute(
                        "AllGather", A.bypass,
                        replica_groups=[list(range(NCORES))],
                        ins=[t1slice[:]], outs=[T1[:]],
                    )
                # bulk a_dst gather: one call for all T tiles' 128 slots
                _gather_raw(nc.gpsimd, adtall_sb[:],
                            t1slice[0:RPC, 264:272], adidx_sb[:],
                            T * 128, 8, T1_W)

                # ---------------- Phase B: layer-1 aggregation ----------------
                _agg_layer(nc, tc, T, layer=1,
                           tbl_lo=T1[0:T1_LO, 0:264],
                           tbl_hi=T1[T1_LO:NR, 0:264],
                           gidx_sb=g1i_sb, dloc_sb=dl1_sb, selT_in=selT1,
                           iot_sb=iot_sb, idn=idn, oneall=oneall,
                           adtall_sb=adtall_sb,
                           W2a_sb=W2a_sb, W2b_sb=W2b_sb, b2_sb=b2_sb,
                           adst2_sb=adst2_sb,
                           t2slice=t2slice, outp=None)

            if "D" in phases:
                nc.sync.dma_start(out=g2i_sb[:], in_=g2idx[:])
                nc.sync.dma_start(out=dl2_sb[:], in_=dloc2[:])

            if dbg:
                nc.sync.dma_start(out=t1dbg[:], in_=t1slice[:])
                nc.sync.dma_start(
                    out=addbg[:],
                    in_=adtall_sb[:].rearrange("p t c -> p (t c)"))
                nc.sync.dma_start(out=t2dbg[:], in_=t2slice[:])

            # ---------------- AllGather T2 ----------------
            if "C" in phases:
                if with_collective:
                    nc.gpsimd.collective_compute(
                        "AllGather", A.bypass,
                        replica_groups=[list(range(NCORES))],
                        ins=[t2slice[:]], outs=[t2full[:]],
                    )
                else:
                    nc.sync.dma_start(out=t2full[0:t2rows, :], in_=t2slice[:])

            # ---------------- Phase D: layer-2 aggregation ----------------
            if "D" in phases:
                _agg_layer(nc, tc, T, layer=2,
                           tbl_lo=t2full[0:4 * t2rows, 0:65],
                           tbl_hi=t2full[4 * t2rows:8 * t2rows, 0:65],
                           gidx_sb=g2i_sb, dloc_sb=dl2_sb, selT_in=selT2,
                           iot_sb=iot_sb, idn=idn, oneall=None,
                           adtall_sb=None,
                           W2a_sb=None, W2b_sb=None, b2_sb=None,
                           adst2_sb=adst2_sb,
                           t2slice=None, outp=outp)

    nc.compile()
    return nc


def _agg_layer(nc, tc, T, layer, tbl_lo, tbl_hi, gidx_sb, dloc_sb, selT_in,
               iot_sb, idn, oneall, adtall_sb, W2a_sb, W2b_sb, b2_sb,
               adst2_sb, t2slice, outp):
    L1 = layer == 1
    GW = 264 if L1 else 65   # gathered elements per row (payload)
    GS = T1_W if L1 else 128  # table row stride in elements
    NH = 8 if L1 else 1
    VC = 256 if L1 else 64
    ACC_W = 264 if L1 else 65
    name = f"l{layer}"
    PBB = int(os.environ.get("V2_PBB", "3"))
    ACCB = int(os.environ.get("V2_ACCB", "3"))
    with tc.tile_pool(name=f"pb_{name}", bufs=PBB) as pb, \
         tc.tile_pool(name=f"pf_{name}", bufs=2) as pf, \
         tc.tile_pool(name=f"ps_acc_{name}", bufs=ACCB, space="PSUM") as ps_acc, \
         tc.tile_pool(name=f"ps_ad_{name}", bufs=2, space="PSUM") as ps_ad, \
         tc.tile_pool(name=f"ps_fin_{name}", bufs=2, space="PSUM") as ps_fin:
        for pr in range(T // 2):
            accs = [ps_acc.tile([128, ACC_W], F32, tag="acc", name="acc_a"),
                    ps_acc.tile([128, ACC_W], F32, tag="acc", name="acc_b")]
            if L1:
                scp = pb.tile([128, 2, 2, CH, 128], F8, tag="scp",
                              name="scp", bufs=2)
                nc.sync.dma_start(
                    out=scp[:].rearrange("p f s j e -> p (f s j e)"),
                    in_=selT_in[:, (2 * pr) * 2 * CALLW:
                                (2 * pr + 2) * 2 * CALLW])
            for hf in (0, 1):
                g = 2 * pr + hf
                if L1:
                    s8 = scp[:, hf, 0]     # selT: [d-part, j, e]
                    sel8 = scp[:, hf, 1]   # sel:  [e-part, j, d]
                else:
                    s8t = pb.tile([128, CH, 128], F8, tag="s8", name="s8",
                                  bufs=3)
                    nc.sync.dma_start(
                        out=s8t[:].rearrange("p j e -> p (j e)"),
                        in_=selT_in[:, g * CALLW:(g + 1) * CALLW])
                    s8 = s8t[:]
                gt = pb.tile([128, CH, GW], F16, tag="gt", name="gt", bufs=3)
                if os.environ.get("V2_GSPLIT", "0") == "1":
                    for gh in (0, 1):
                        _gather_raw(
                            nc.gpsimd, gt[:, gh * CHL:(gh + 1) * CHL],
                            tbl_lo if hf == 0 else tbl_hi,
                            gidx_sb[:, g * CH * 8 + gh * CHL * 8:
                                    g * CH * 8 + (gh + 1) * CHL * 8],
                            CHL * 128, GW, GS)
                else:
                    _gather_raw(
                        nc.gpsimd, gt[:], tbl_lo if hf == 0 else tbl_hi,
                        gidx_sb[:, g * CH * 8:(g + 1) * CH * 8],
                        CALLW, GW, GS)
                if not L1:
                    sel = pb.tile([128, 128, CH], F16, tag="sel", name="sel")
                    nc.vector.tensor_tensor(
                        out=sel[:],
                        in0=dloc_sb[:, None,
                                    g * CH:(g + 1) * CH].to_broadcast(
                            [128, 128, CH]),
                        in1=iot_sb[:].rearrange("p (d j) -> p d j", j=CH),
                        op=A.is_equal)
                # per-edge a_dst via fp8 selT one-hot matmul
                adps = ps_ad.tile([128, CH, NH], F32, tag="adps", name="adps")
                for j in range(CH):
                    t = 2 * pr + (0 if j < CHL else 1)
                    rhs = adtall_sb[:, t, :] if L1 else adst2_sb[:, t:t + 1]
                    nc.tensor.matmul(adps[:, j, :], lhsT=s8[:, j, :], rhs=rhs,
                                     start=True, stop=True)
                et = pb.tile([128, CH, NH], F16, tag="et", name="et")
                asrc_ap = gt[:, :, 256:264] if L1 else gt[:, :, 64:65]
                if L1:
                    adf = pb.tile([128, CH, NH], F16, tag="adf", name="adf")
                    nc.vector.tensor_copy(adf[:], adps[:])
                    nc.vector.tensor_tensor(out=et[:], in0=asrc_ap,
                                            in1=adf[:], op=A.add)
                else:
                    nc.vector.scalar_tensor_tensor(
                        out=et[:], in0=adps[:], scalar=1.0, in1=asrc_ap,
                        op0=A.mult, op1=A.add)
                lk = pb.tile([128, CH, NH], F16, tag="lk", name="lk")
                nc.vector.scalar_tensor_tensor(out=lk[:], in0=et[:],
                                               scalar=NEG, in1=et[:],
                                               op0=A.mult, op1=A.max)
                # v1-style: exp broadcast on ACT, 264-wide w with den
                # columns, one matmul stream per chunk
                exx = pb.tile([128, CH, VC], F16, tag="exx", name="exx")
                if L1:
                    nc.scalar.activation(
                        exx[:].rearrange("p j (h c) -> p j h c", h=8),
                        lk[:, :, :, None].to_broadcast([128, CH, 8, 32]),
                        AF.Exp)
                else:
                    nc.scalar.activation(
                        exx[:], lk[:].to_broadcast([128, CH, 64]), AF.Exp)
                w = pb.tile([128, CH, ACC_W], F16, tag="w", name="w")
                if L1:
                    nc.vector.tensor_copy(
                        w[:, :, 256:264],
                        exx[:].rearrange("p j (h c) -> p j h c",
                                         h=8)[:, :, :, 0])
                else:
                    nc.vector.tensor_copy(w[:, :, 64:65], exx[:, :, 0:1])
                nc.vector.tensor_tensor(out=w[:, :, 0:VC],
                                        in0=gt[:, :, 0:VC],
                                        in1=exx[:], op=A.mult)
                for j in range(CH):
                    acc = accs[0 if j < CHL else 1]
                    st = (hf == 0) and (j % CHL == 0)
                    sp = (hf == 1) and (j % CHL == CHL - 1)
                    lhs = sel8[:, j, :] if L1 else sel[:, :, j]
                    nc.tensor.matmul(acc[:], lhsT=lhs,
                                     rhs=w[:, j, :], start=st, stop=sp)
            for k in (0, 1):
                t = 2 * pr + k
                if L1:
                    _fin_l1(nc, t, accs[k], pf, ps_fin, idn, oneall, W2a_sb,
                            W2b_sb, b2_sb, adst2_sb, t2slice)
                else:
                    _fin_l2(nc, t, accs[k], pf, outp)


def _fin_l1(nc, t, acc, pf, ps_fin, idn, oneall, W2a_sb, W2b_sb, b2_sb,
            adst2_sb, t2slice):
    deps = pf.tile([128, 8], F32, tag="deps", name="deps")
    nc.vector.tensor_scalar_add(deps[:], acc[:, 256:264], EPS)
    rec = pf.tile([128, 8], F32, tag="rec", name="rec")
    nc.vector.reciprocal(rec[:], deps[:])
    h1b = pf.tile([128, 256], F16, tag="h1b", name="h1b")
    nc.vector.tensor_tensor(
        out=h1b[:].rearrange("p (h c) -> p h c", h=8),
        in0=acc[:, 0:256].rearrange("p (h c) -> p h c", h=8),
        in1=rec[:, :, None].to_broadcast([128, 8, 32]),
        op=A.mult)
    if t == 0 and getattr(nc, "_dbg", None):
        accs_sb = pf.tile([128, 264], F32, tag="accdbg", name="accdbg")
        nc.vector.tensor_copy(accs_sb[:, 0:256], acc[:, 0:256])
        nc.vector.tensor_copy(accs_sb[:, 256:264], acc[:, 256:264])
        nc.sync.dma_start(out=nc._dbg["h1dbg"][:], in_=accs_sb[:])
    # ho = elu(h1b) + 1 = relu(h1b) + exp(-relu(-h1b)); the -1 is folded
    # into b2eff via W2ext (v1's ACT-based elu decomposition)
    r1 = pf.tile([128, 256], F16, tag="r1", name="r1")
    nc.scalar.activation(r1[:], h1b[:], AF.Relu, scale=-1.0)
    e1 = pf.tile([128, 256], F16, tag="e1", name="e1")
    nc.scalar.activation(e1[:], r1[:], AF.Exp, scale=-1.0)
    rl = pf.tile([128, 256], F16, tag="rl", name="rl")
    nc.scalar.activation(rl[:], h1b[:], AF.Relu)
    ho = pf.tile([128, 256], F16, tag="ho", name="ho")
    nc.vector.tensor_tensor(out=ho[:], in0=rl[:], in1=e1[:], op=A.add)
    h2ps = ps_fin.tile([128, 66], F32, tag="h2ps", name="h2ps")
    for half in (0, 1):
        tp = ps_fin.tile([128, 128], F16, tag="tp", name="tp", bufs=1)
        nc.tensor.transpose(out=tp[:], in_=ho[:, half * 128:(half + 1) * 128],
                            identity=idn[:])
        hoT = pf.tile([128, 128], F16, tag="hoT", name="hoT")
        if half == 0:
            nc.vector.tensor_copy(hoT[:], tp[:])
        else:
            nc.scalar.copy(hoT[:], tp[:])
        nc.tensor.matmul(h2ps[:], lhsT=hoT[:],
                         rhs=(W2a_sb if half == 0 else W2b_sb)[:],
                         start=half == 0, stop=False)
    nc.tensor.matmul(h2ps[:], lhsT=oneall[:], rhs=b2_sb[:], start=False,
                     stop=True)
    t2r = pf.tile([128, 65], F16, tag="t2r", name="t2r")
    nc.scalar.copy(t2r[:], h2ps[:, 0:65])
    nc.vector.tensor_copy(adst2_sb[:, t:t + 1], h2ps[:, 65:66])
    nc.sync.dma_start(out=t2slice[t * 128:(t + 1) * 128, 0:65], in_=t2r[:])


def _fin_l2(nc, t, acc, pf, outp):
    dep = pf.tile([128, 1], F32, tag="dep2", name="dep2")
    nc.vector.tensor_scalar_add(dep[:], acc[:, 64:65], EPS)
    rec = pf.tile([128, 1], F32, tag="rec2", name="rec2")
    nc.vector.reciprocal(rec[:], dep[:])
    ot = pf.tile([128, 64], F32, tag="ot", name="ot")
    nc.vector.tensor_scalar_mul(ot[:], acc[:, 0:64], rec[:, 0:1])
    nc.sync.dma_start(out=outp[t * 128:(t + 1) * 128, :], in_=ot[:])


# ---------------------------------------------------------------------------
# entry point
# ---------------------------------------------------------------------------

def make_in_maps(T, common, per_core):
    in_maps = []
    for c in range(NCORES):
        m = {
            "W1ext": common["W1ext"], "W2ext": common["W2ext"],
            "b1ext": common["b1ext"], "b2e66": common["b2e66"],
            "iotarep": common["iotarep"],
        }
        pc = per_core[c]
        m.update({k: pc[k] for k in ("xTc", "g1idx", "g2idx", "dloc1",
                                     "dloc2", "selT1", "selT2", "adidx")})
        in_maps.append(m)
    return in_maps


def kernel(**inputs):
    T, common, per_core = host_prep(inputs)
    nc = build_nc(T)
    in_maps = make_in_maps(T, common, per_core)
    res = run_bass_kernel_spmd(nc, in_maps, core_ids=list(range(NCORES)))
    allrows = np.concatenate([res.results[c]["out"] for c in range(NCORES)],
                             axis=0)
    return allrows[common["slotrow"]].astype(np.float32)



# revision 13
# speedup vs baseline: 1.0393x; 1.0393x over previous
"""Two-layer GAT on 8 Trainium2 NeuronCores (Bass/Tile) — v2.

Changes vs v1 (889us):
  - Phase A sharded: each core computes T1 rows for its own 6250 nodes
    (padded to 6272), AllGather broadcasts the table (timing build: local
    copy of own contribution only).
  - b1 folded into T1 value columns (sum(alpha)=1), b2 and the elu "-1"
    folded into W2ext/b2eff at finalize.
  - selT (dst-onehot, d-on-partitions) shipped from host as fp8 and used
    directly as matmul lhsT (mixed fp8 x f16 matmul) — kills the dlT DMA,
    the DVE selT build, and the adf path stays cheap.
  - exp broadcast stays on ACT; leaky/et on DVE; PSUM->SBUF copies and
    t2r/adst2 finalization moved to Pool (gpsimd).
  - softmax denominator accumulated via a second per-chunk matmul with a
    strided rhs view of exx (no den-copy op).
  - asrc2/adst2 computed via the h2 matmul itself (W2ext has v_s|v_d cols).
"""

import os

import numpy as np

import concourse.bass as bass
import concourse.bacc as bacc
import concourse.tile as tile
import concourse.mybir as mybir
from concourse.bass_utils import run_bass_kernel_spmd
from concourse.masks import make_identity

F32 = mybir.dt.float32
F16 = mybir.dt.float16
F8 = mybir.dt.float8e4
I16 = mybir.dt.int16
I32 = mybir.dt.int32
A = mybir.AluOpType
AF = mybir.ActivationFunctionType
NP_F8 = mybir.dt.np(F8)

# -------- problem constants --------
N, E, IN, HID, OUT, H = 50000, 800000, 128, 32, 64, 8
C1 = H * HID  # 256
NCORES = 8
NPC = N // NCORES        # 6250 dst nodes per core
RPC = 6272               # T1 rows per core (6250 padded to 49*128)
NR = NCORES * RPC        # 50176 T1 rows
T1_LO = 4 * RPC          # 25088: rows of cores 0-3
T1_W = 384               # T1 row stride (768B) — gather granularity
CHL = 9                  # chunks per (tile, half)
CH = 2 * CHL             # chunks per gather call
CALLW = CH * 128         # 2304 edge slots per call
EPS = 1e-16
NEG = 0.2


def _row_of(n):
    """T1 row of node n (cores own contiguous 6250-node ranges, padded)."""
    c = n // NPC
    return c * RPC + (n - c * NPC)


# ---------------------------------------------------------------------------
# host-side preprocessing
# ---------------------------------------------------------------------------

def _prep_weights(W1, as1, ad1, b1, W2, as2, ad2, b2):
    As = np.zeros((C1, H), np.float32)
    Ad = np.zeros((C1, H), np.float32)
    for h in range(H):
        As[h * HID:(h + 1) * HID, h] = as1[h]
        Ad[h * HID:(h + 1) * HID, h] = ad1[h]
    W1ext = np.concatenate([W1, W1 @ As, W1 @ Ad], axis=1)  # [128, 272]
    iotarep = np.zeros((128, 128 * CH), np.float16)
    for d in range(128):
        iotarep[:, d * CH:(d + 1) * CH] = d
    b1ext = np.zeros((272,), np.float32)
    b1ext[:C1] = b1
    vs = W2 @ as2[0]   # [256]
    vd = W2 @ ad2[0]   # [256]
    W2ext = np.concatenate([W2, vs[:, None], vd[:, None]], axis=1)  # [256,66]
    b2eff = np.concatenate([b2 - W2.sum(0), [-vs.sum()], [-vd.sum()]])  # [66]
    return {
        "W1ext": W1ext.astype(np.float16),
        "W2ext": W2ext.astype(np.float16),
        "b1ext": np.tile(b1ext[None, :], (128, 1)).astype(np.float16),
        "b2e66": np.tile(b2eff[None, :], (128, 1)).astype(np.float16),
        "iotarep": iotarep,
    }


def _greedy_tiles(deg_lo1, deg_hi1, deg_lo2, deg_hi2):
    cap = CHL * 128
    tiles = []
    i, n = 0, len(deg_lo1)
    while i < n:
        l1 = h1 = l2 = h2 = 0
        j = i
        while j < n and j - i < 128:
            nl1, nh1 = l1 + deg_lo1[j], h1 + deg_hi1[j]
            nl2, nh2 = l2 + deg_lo2[j], h2 + deg_hi2[j]
            if nl1 > cap or nh1 > cap or nl2 > cap or nh2 > cap:
                break
            l1, h1, l2, h2 = nl1, nh1, nl2, nh2
            j += 1
        assert j > i, "single node exceeds chunk caps"
        tiles.append((i, j))
        i = j
    return tiles


def _pack_calls(rows_half, dloc_half, T, both=False):
    """Build gather idx [128, T*CH*8] i16, dloc [128, T*CH] f16, and the
    fp8 one-hot stream: selT only [128, T*CH*128], or selT|sel interleaved
    per call [128, T*2*CH*128] when both=True."""
    idx16 = np.zeros((128, T * CH * 8), np.int16)
    dloc = np.full((128, T * CH), -1, np.float16)
    dlocT_flat = np.full((T * CALLW,), -1, np.int32)
    for pr in range(T // 2):
        for h in (0, 1):
            g = 2 * pr + h
            rows = np.zeros((CALLW,), np.int64)
            dl = np.full((CALLW,), -1, np.int64)
            for k, t in enumerate((2 * pr, 2 * pr + 1)):
                r = rows_half.get((t, h))
                if r is None:
                    continue
                d = dloc_half[(t, h)]
                off = k * CHL * 128
                rows[off:off + len(r)] = r
                dl[off:off + len(r)] = d
            blk = rows.reshape(CH * 8, 16).T.astype(np.int16)
            idx16[:, g * CH * 8:(g + 1) * CH * 8] = np.tile(blk, (8, 1))
            dloc[:, g * CH:(g + 1) * CH] = dl.reshape(CH, 128).T.astype(np.float16)
            dlocT_flat[g * CALLW:(g + 1) * CALLW] = dl
    # selT[p, g*CH*128 + j*128 + e] = 1.0 iff dloc(edge (j,e) of g) == p
    selT = (dlocT_flat[None, :] == np.arange(128)[:, None])
    if not both:
        return idx16, dloc, selT.astype(NP_F8)
    # sel[p, g, j, d] = 1.0 iff dloc(edge (p,j) of g) == d
    sel = (dloc[:, :, None].astype(np.int32) ==
           np.arange(128)[None, None, :]).reshape(128, T, CH, 128)
    selB = np.stack([selT.reshape(128, T, CH, 128), sel], axis=2).reshape(
        128, T * 2 * CH * 128).astype(NP_F8)
    return idx16, dloc, selB


def _prep_core(c, src, dst):
    base = c * NPC
    own = (dst >= base) & (dst < base + NPC)
    s = src[own].astype(np.int64)
    d = (dst[own] - base).astype(np.int64)
    order = np.argsort(d, kind="stable")
    s, d = s[order], d[order]
    ptr = np.zeros(NPC + 1, np.int64)
    np.cumsum(np.bincount(d, minlength=NPC), out=ptr[1:])

    srow = _row_of(s)
    lo1m = srow < T1_LO
    lo2m = s < (N // 2)  # slot-major half split: cores 0-3 vs 4-7
    deg_lo1 = np.bincount(d, weights=lo1m, minlength=NPC).astype(np.int64)
    deg_hi1 = np.bincount(d, weights=~lo1m, minlength=NPC).astype(np.int64)
    deg_lo2 = np.bincount(d, weights=lo2m, minlength=NPC).astype(np.int64)
    deg_hi2 = np.bincount(d, weights=~lo2m, minlength=NPC).astype(np.int64)
    tiles = _greedy_tiles(deg_lo1, deg_hi1, deg_lo2, deg_hi2)

    rows1, dloc1 = {}, {}
    for t, (n0, n1) in enumerate(tiles):
        e0, e1 = ptr[n0], ptr[n1]
        es, ed = srow[e0:e1], d[e0:e1]
        dl = ed - n0
        m1 = es < T1_LO
        rows1[(t, 0)] = es[m1]
        dloc1[(t, 0)] = dl[m1]
        rows1[(t, 1)] = es[~m1] - T1_LO
        dloc1[(t, 1)] = dl[~m1]
    return {
        "Treal": len(tiles), "tiles": tiles, "s": s, "d": d, "ptr": ptr,
        "rows1": rows1, "dloc1": dloc1,
    }


def _finish_core(pc, c, T, slotrow):
    tiles = list(pc["tiles"]) + [(0, 0)] * (T - pc["Treal"])
    idx1, dloc1, selT1 = _pack_calls(pc["rows1"], pc["dloc1"], T, both=True)

    t2lo = 4 * T * 128
    s, d, ptr = pc["s"], pc["d"], pc["ptr"]
    srow = slotrow[s]
    rows2, dloc2 = {}, {}
    for t, (n0, n1) in enumerate(pc["tiles"]):
        e0, e1 = ptr[n0], ptr[n1]
        dl = d[e0:e1] - n0
        m2 = srow[e0:e1] < t2lo
        rows2[(t, 0)] = srow[e0:e1][m2]
        dloc2[(t, 0)] = dl[m2]
        rows2[(t, 1)] = srow[e0:e1][~m2] - t2lo
        dloc2[(t, 1)] = dl[~m2]
    idx2, dloc2a, selT2 = _pack_calls(rows2, dloc2, T)

    # adidx: local t1slice rows of each tile's nodes (slot-major, clamped),
    # in dma_gather int16 index layout, one call of T*128 rows
    p = np.arange(128)
    rows = np.zeros((T * 128,), np.int64)
    for t, (n0, n1) in enumerate(tiles):
        w = n1 - n0
        rows[t * 128:(t + 1) * 128] = n0 + np.minimum(p, max(w - 1, 0))
    blk = rows.reshape(T * 8, 16).T.astype(np.int16)
    adidx = np.tile(blk, (8, 1))  # [128, T*8]
    return {
        "g1idx": idx1, "dloc1": dloc1, "selT1": selT1,
        "g2idx": idx2, "dloc2": dloc2a, "selT2": selT2,
        "adidx": adidx,
    }


def host_prep(inputs):
    ei = np.asarray(inputs["edge_index"]).astype(np.int64)
    wd = _prep_weights(
        np.asarray(inputs["W1"], np.float32),
        np.asarray(inputs["att_src1"], np.float32),
        np.asarray(inputs["att_dst1"], np.float32),
        np.asarray(inputs["b1"], np.float32),
        np.asarray(inputs["W2"], np.float32),
        np.asarray(inputs["att_src2"], np.float32),
        np.asarray(inputs["att_dst2"], np.float32),
        np.asarray(inputs["b2"], np.float32),
    )
    loops = np.arange(N, dtype=np.int64)
    src = np.concatenate([ei[0], loops])
    dst = np.concatenate([ei[1], loops])

    x = np.asarray(inputs["x"], np.float32).astype(np.float16)

    cores = [_prep_core(c, src, dst) for c in range(NCORES)]
    T = max(pc["Treal"] for pc in cores)
    if T % 2:
        T += 1
    slotrow = np.zeros(N, np.int64)
    for c, pc in enumerate(cores):
        base_row = c * T * 128
        for t, (n0, n1) in enumerate(pc["tiles"]):
            nodes = c * NPC + np.arange(n0, n1)
            slotrow[nodes] = base_row + t * 128 + np.arange(n1 - n0)
    per_core = [_finish_core(pc, c, T, slotrow) for c, pc in enumerate(cores)]
    for c in range(NCORES):
        xc = np.zeros((IN, RPC), np.float16)
        xc[:, :NPC] = x[c * NPC:(c + 1) * NPC].T
        per_core[c]["xTc"] = xc

    common = dict(wd)
    common["slotrow"] = slotrow
    return T, common, per_core


# ---------------------------------------------------------------------------
# device program
# ---------------------------------------------------------------------------

def _gather_raw(eng, out_ap, in_ap, idxs_ap, num_idxs, elem_size, elem_step):
    """dma_gather with elem_size_bytes not a multiple of 256B (non-transpose
    path only; the 256B rule is a transpose-mode restriction — the Q7 kernel
    packets arbitrary elem sizes, only the row stride is encoded in 256B
    units).  Mirrors BassGpSimd.dma_gather's construction."""
    from concourse.ap_utils import ap_is_contiguous
    import concourse.mybir as mb
    assert idxs_ap.dtype == mybir.dt.int16
    assert in_ap.dtype == out_ap.dtype
    elem_size_bytes = elem_size * mybir.dt.size(in_ap.dtype)
    assert in_ap.ap[-1][1] == out_ap.ap[-1][1] == elem_size
    assert ap_is_contiguous(out_ap.ap[1:])
    assert ap_is_contiguous(idxs_ap.ap[1:])
    assert in_ap.ap[0][0] == elem_step
    stride_bytes = elem_step * mybir.dt.size(in_ap.dtype)
    assert stride_bytes % 256 == 0 and stride_bytes // 256 < 256
    _in_ap = eng.lower_ap_dma(in_ap, for_custom_bir_dma=True)
    _idxs_ap = eng.lower_ap(idxs_ap)
    _out_ap = eng.lower_ap(out_ap)
    return eng.add_instruction(
        mb.InstDMAGatherAnt(
            name=eng.bass.get_next_instruction_name(),
            ins=[*_in_ap, _idxs_ap,
                 eng.lower_val_access(eng.to_reg(num_idxs))],
            outs=[_out_ap],
            transpose=False,
            num_idxs=num_idxs,
            elem_size=elem_size,
            stride_bytes_256=stride_bytes // 256,
            gen_mode=0,
            single_packet=False,
            queue_num=0,
            sbuf_tokens_per_rank=0,
            sbuf_free_dim_per_rank=0,
            sbuf_free_dim_pad_per_rank=0,
            sbuf_byte_offset=0,
        )
    )


def build_nc(T, num_devices=NCORES, with_collective=True, phases="ABCD",
             dbg=False):
    nc = bacc.Bacc("TRN2", target_bir_lowering=False, debug=False,
                   num_devices=num_devices)
    dt = nc.dram_tensor
    xTc = dt("xTc", [IN, RPC], F16, kind="ExternalInput").ap()
    W1ext = dt("W1ext", [128, 272], F16, kind="ExternalInput").ap()
    W2ext = dt("W2ext", [256, 66], F16, kind="ExternalInput").ap()
    b1ext = dt("b1ext", [128, 272], F16, kind="ExternalInput").ap()
    b2e66 = dt("b2e66", [128, 66], F16, kind="ExternalInput").ap()
    iotarep = dt("iotarep", [128, 128 * CH], F16, kind="ExternalInput").ap()
    g1idx = dt("g1idx", [128, T * CH * 8], I16, kind="ExternalInput").ap()
    g2idx = dt("g2idx", [128, T * CH * 8], I16, kind="ExternalInput").ap()
    dloc1 = dt("dloc1", [128, T * CH], F16, kind="ExternalInput").ap()
    dloc2 = dt("dloc2", [128, T * CH], F16, kind="ExternalInput").ap()
    selT1 = dt("selT1", [128, T * 2 * CH * 128], F8, kind="ExternalInput").ap()
    selT2 = dt("selT2", [128, T * CH * 128], F8, kind="ExternalInput").ap()
    adidx = dt("adidx", [128, T * 8], I16, kind="ExternalInput").ap()
    t1slice = dt("t1slice", [RPC, T1_W], F16, kind="Internal").ap()
    T1 = dt("T1", [NR, T1_W], F16, kind="Internal",
            addr_space="Shared" if with_collective else "Local").ap()
    t2rows = T * 128
    t2slice = dt("t2slice", [t2rows, 128], F16, kind="Internal").ap()
    t2full = dt("t2full", [NCORES * t2rows, 128], F16, kind="Internal",
                addr_space="Shared" if with_collective else "Local").ap()
    outp = dt("out", [t2rows, 64], F32, kind="ExternalOutput").ap()
    if dbg:
        t1dbg = dt("t1dbg", [RPC, T1_W], F16, kind="ExternalOutput").ap()
        addbg = dt("addbg", [128, T * 8], F16, kind="ExternalOutput").ap()
        t2dbg = dt("t2dbg", [t2rows, 128], F16, kind="ExternalOutput").ap()
        gtdbg = dt("gtdbg", [128, CH * 264], F16, kind="ExternalOutput").ap()
        etdbg = dt("etdbg", [128, CH * 8], F16, kind="ExternalOutput").ap()
        wdbg = dt("wdbg", [128, CH * 256], F16, kind="ExternalOutput").ap()
        lkdbg = dt("lkdbg", [128, CH * 8], F16, kind="ExternalOutput").ap()
        exxdbg = dt("exxdbg", [128, CH * 256], F16,
                    kind="ExternalOutput").ap()
        h1dbg = dt("h1dbg", [128, 256 + 8], F32, kind="ExternalOutput").ap()
        nc._dbg = dict(gtdbg=gtdbg, etdbg=etdbg, wdbg=wdbg, h1dbg=h1dbg,
                       lkdbg=lkdbg, exxdbg=exxdbg)
    else:
        nc._dbg = None

    with tile.TileContext(nc) as tc:
        with tc.tile_pool(name="consts", bufs=1) as cp:
            W1e_sb = cp.tile([128, 272], F16)
            nc.sync.dma_start(out=W1e_sb[:], in_=W1ext[:])
            W2a_sb = cp.tile([128, 66], F16)
            nc.sync.dma_start(out=W2a_sb[:], in_=W2ext[0:128, :])
            W2b_sb = cp.tile([128, 66], F16)
            nc.sync.dma_start(out=W2b_sb[:], in_=W2ext[128:256, :])
            b1_sb = cp.tile([128, 272], F16)
            nc.sync.dma_start(out=b1_sb[:], in_=b1ext[:])
            b2_sb = cp.tile([128, 66], F16)
            nc.sync.dma_start(out=b2_sb[:], in_=b2e66[:])
            oneall = cp.tile([128, 128], F16)
            nc.vector.memset(oneall[:], 1.0 / 128.0)
            iot_sb = cp.tile([128, 128 * CH], F16)
            nc.sync.dma_start(out=iot_sb[:], in_=iotarep[:])
            dl1_sb = cp.tile([128, T * CH], F16)
            nc.sync.dma_start(out=dl1_sb[:], in_=dloc1[:])
            dl2_sb = cp.tile([128, T * CH], F16)
            nc.sync.dma_start(out=dl2_sb[:], in_=dloc2[:])
            idn = cp.tile([128, 128], F16)
            make_identity(nc, idn[:])
            g1i_sb = cp.tile([128, T * CH * 8], I16)
            nc.sync.dma_start(out=g1i_sb[:], in_=g1idx[:])
            dl1_sb = cp.tile([128, T * CH], F16)
            nc.sync.dma_start(out=dl1_sb[:], in_=dloc1[:])
            g2i_sb = cp.tile([128, T * CH * 8], I16)
            nc.sync.dma_start(out=g2i_sb[:], in_=g2idx[:])
            adidx_sb = cp.tile([128, T * 8], I16)
            nc.sync.dma_start(out=adidx_sb[:], in_=adidx[:])
            adtall_sb = cp.tile([128, T, 8], F16)  # bulk a_dst gather target
            adst2_sb = cp.tile([128, T], F16)  # written in B-fin, read in D

            # ---------------- Phase A: own T1 slice ----------------
            if "A" in phases:
                with tc.tile_pool(name="pa", bufs=2) as pa, \
                     tc.tile_pool(name="paps", bufs=4, space="PSUM") as paps:
                    XB = 2048
                    nblk = (RPC + XB - 1) // XB
                    for blk in range(nblk):
                        n0 = blk * XB
                        bw = min(XB, RPC - n0)
                        nt = bw // 128
                        xb = pa.tile([128, XB], F16, tag="xb", name="xb")
                        nc.sync.dma_start(out=xb[:, 0:bw],
                                          in_=xTc[:, n0:n0 + bw])
                        t1b = pa.tile([128, 16, 272], F16, tag="t1b",
                                      name="t1b")
                        for i in range(nt):
                            ps = paps.tile([128, 272], F32, tag="aps",
                                           name="aps")
                            nc.tensor.matmul(ps[:],
                                             lhsT=xb[:, i * 128:(i + 1) * 128],
                                             rhs=W1e_sb[:], start=True,
                                             stop=False)
                            nc.tensor.matmul(ps[:], lhsT=oneall[:],
                                             rhs=b1_sb[:], start=False,
                                             stop=True)
                            if i % 2 == 0:
                                nc.vector.tensor_copy(t1b[:, i, :], ps[:])
                            else:
                                nc.scalar.copy(t1b[:, i, :], ps[:])
                        nc.sync.dma_start(
                            out=t1slice[n0:n0 + bw, 0:272].rearrange(
                                "(i p) c -> p i c", p=128),
                            in_=t1b[:, 0:nt, :])
                        if not with_collective and "B" in phases:
                            nc.sync.dma_start(
                                out=T1[n0:n0 + bw, 0:272],
                                in_=t1slice[n0:n0 + bw, 0:272])

            # ---------------- AllGather T1 ----------------
            if "B" in phases:
                if with_collective:
                    nc.gpsimd.collective_compute(
                        "AllGather", A.bypass,
                        replica_groups=[list(range(NCORES))],
                        ins=[t1slice[:]], outs=[T1[:]],
                    )
                # bulk a_dst gather: one call for all T tiles' 128 slots
                _gather_raw(nc.gpsimd, adtall_sb[:],
                            t1slice[0:RPC, 264:272], adidx_sb[:],
                            T * 128, 8, T1_W)

                # ---------------- Phase B: layer-1 aggregation ----------------
                _agg_layer(nc, tc, T, layer=1,
                           tbl_lo=T1[0:T1_LO, 0:264],
                           tbl_hi=T1[T1_LO:NR, 0:264],
                           gidx_sb=g1i_sb, dloc_sb=dl1_sb, selT_in=selT1,
                           iot_sb=iot_sb, idn=idn, oneall=oneall,
                           adtall_sb=adtall_sb,
                           W2a_sb=W2a_sb, W2b_sb=W2b_sb, b2_sb=b2_sb,
                           adst2_sb=adst2_sb,
                           t2slice=t2slice, outp=None)

            if "D" in phases:
                nc.sync.dma_start(out=g2i_sb[:], in_=g2idx[:])
                nc.sync.dma_start(out=dl2_sb[:], in_=dloc2[:])

            if dbg:
                nc.sync.dma_start(out=t1dbg[:], in_=t1slice[:])
                nc.sync.dma_start(
                    out=addbg[:],
                    in_=adtall_sb[:].rearrange("p t c -> p (t c)"))
                nc.sync.dma_start(out=t2dbg[:], in_=t2slice[:])

            # ---------------- AllGather T2 ----------------
            if "C" in phases:
                if with_collective:
                    nc.gpsimd.collective_compute(
                        "AllGather", A.bypass,
                        replica_groups=[list(range(NCORES))],
                        ins=[t2slice[:]], outs=[t2full[:]],
                    )
                else:
                    nc.sync.dma_start(out=t2full[0:t2rows, :], in_=t2slice[:])

            # ---------------- Phase D: layer-2 aggregation ----------------
            if "D" in phases:
                _agg_layer(nc, tc, T, layer=2,
                           tbl_lo=t2full[0:4 * t2rows, 0:65],
                           tbl_hi=t2full[4 * t2rows:8 * t2rows, 0:65],
                           gidx_sb=g2i_sb, dloc_sb=dl2_sb, selT_in=selT2,
                           iot_sb=iot_sb, idn=idn, oneall=None,
                           adtall_sb=None,
                           W2a_sb=None, W2b_sb=None, b2_sb=None,
                           adst2_sb=adst2_sb,
                           t2slice=None, outp=outp)

    nc.compile()
    return nc


def _agg_layer(nc, tc, T, layer, tbl_lo, tbl_hi, gidx_sb, dloc_sb, selT_in,
               iot_sb, idn, oneall, adtall_sb, W2a_sb, W2b_sb, b2_sb,
               adst2_sb, t2slice, outp):
    L1 = layer == 1
    GW = 264 if L1 else 65   # gathered elements per row (payload)
    GS = T1_W if L1 else 128  # table row stride in elements
    NH = 8 if L1 else 1
    VC = 256 if L1 else 64
    ACC_W = 264 if L1 else 65
    name = f"l{layer}"
    PBB = int(os.environ.get("V2_PBB", "3"))
    ACCB = int(os.environ.get("V2_ACCB", "3"))
    with tc.tile_pool(name=f"pb_{name}", bufs=PBB) as pb, \
         tc.tile_pool(name=f"pf_{name}", bufs=2) as pf, \
         tc.tile_pool(name=f"ps_acc_{name}", bufs=ACCB, space="PSUM") as ps_acc, \
         tc.tile_pool(name=f"ps_ad_{name}", bufs=2, space="PSUM") as ps_ad, \
         tc.tile_pool(name=f"ps_fin_{name}", bufs=2, space="PSUM") as ps_fin:
        for pr in range(T // 2):
            accs = [ps_acc.tile([128, ACC_W], F32, tag="acc", name="acc_a"),
                    ps_acc.tile([128, ACC_W], F32, tag="acc", name="acc_b")]
            if L1:
                scp = pb.tile([128, 2, 2, CH, 128], F8, tag="scp",
                              name="scp", bufs=2)
                nc.sync.dma_start(
                    out=scp[:].rearrange("p f s j e -> p (f s j e)"),
                    in_=selT_in[:, (2 * pr) * 2 * CALLW:
                                (2 * pr + 2) * 2 * CALLW])
            for hf in (0, 1):
                g = 2 * pr + hf
                if L1:
                    s8 = scp[:, hf, 0]     # selT: [d-part, j, e]
                    sel8 = scp[:, hf, 1]   # sel:  [e-part, j, d]
                else:
                    s8t = pb.tile([128, CH, 128], F8, tag="s8", name="s8",
                                  bufs=3)
                    nc.sync.dma_start(
                        out=s8t[:].rearrange("p j e -> p (j e)"),
                        in_=selT_in[:, g * CALLW:(g + 1) * CALLW])
                    s8 = s8t[:]
                gt = pb.tile([128, CH, GW], F16, tag="gt", name="gt", bufs=3)
                if os.environ.get("V2_GSPLIT", "0") == "1":
                    for gh in (0, 1):
                        _gather_raw(
                            nc.gpsimd, gt[:, gh * CHL:(gh + 1) * CHL],
                            tbl_lo if hf == 0 else tbl_hi,
                            gidx_sb[:, g * CH * 8 + gh * CHL * 8:
                                    g * CH * 8 + (gh + 1) * CHL * 8],
                            CHL * 128, GW, GS)
                else:
                    _gather_raw(
                        nc.gpsimd, gt[:], tbl_lo if hf == 0 else tbl_hi,
                        gidx_sb[:, g * CH * 8:(g + 1) * CH * 8],
                        CALLW, GW, GS)
                if not L1:
                    sel = pb.tile([128, 128, CH], F16, tag="sel", name="sel")
                    nc.vector.tensor_tensor(
                        out=sel[:],
                        in0=dloc_sb[:, None,
                                    g * CH:(g + 1) * CH].to_broadcast(
                            [128, 128, CH]),
                        in1=iot_sb[:].rearrange("p (d j) -> p d j", j=CH),
                        op=A.is_equal)
                # per-edge et = a_dst + a_src entirely in PSUM: the fp8 selT
                # one-hot matmul broadcasts a_dst, then an identity matmul
                # accumulates the gathered a_src columns on top.
                adps = ps_ad.tile([128, CH, NH], F32, tag="adps", name="adps")
                asrc_ap = gt[:, :, 256:264] if L1 else gt[:, :, 64:65]
                for j in range(CH):
                    t = 2 * pr + (0 if j < CHL else 1)
                    rhs = adtall_sb[:, t, :] if L1 else adst2_sb[:, t:t + 1]
                    nc.tensor.matmul(adps[:, j, :], lhsT=s8[:, j, :], rhs=rhs,
                                     start=True, stop=False)
                    nc.tensor.matmul(adps[:, j, :], lhsT=idn[:],
                                     rhs=asrc_ap[:, j, :], start=False,
                                     stop=True)
                lk = pb.tile([128, CH, NH], F16, tag="lk", name="lk")
                nc.scalar.activation(lk[:], adps[:], AF.Prelu, alpha=NEG)
                # exp at pair width on ACT (cheap), broadcast to the value
                # width inside the DVE multiply via a stride-0 middle dim —
                # the last dim stays packed so the mult keeps 2x DVE mode.
                exf = pb.tile([128, CH, NH, 2], F16, tag="exf", name="exf")
                nc.scalar.activation(
                    exf[:], lk[:, :, :, None].to_broadcast([128, CH, NH, 2]),
                    AF.Exp)
                w = pb.tile([128, CH, ACC_W], F16, tag="w", name="w")
                nc.vector.tensor_copy(w[:, :, VC:ACC_W], exf[:, :, :, 0])
                cph = VC // NH // 2  # 16 (L1) / 32 (L2) value pairs per head
                nc.vector.tensor_tensor(
                    out=w[:, :, 0:VC].rearrange("p j (h k two) -> p j h k two",
                                                h=NH, two=2),
                    in0=gt[:, :, 0:VC].rearrange("p j (h k two) -> p j h k two",
                                                 h=NH, two=2),
                    in1=exf[:, :, :, None, :].to_broadcast(
                        [128, CH, NH, cph, 2]),
                    op=A.mult)
                for j in range(CH):
                    acc = accs[0 if j < CHL else 1]
                    st = (hf == 0) and (j % CHL == 0)
                    sp = (hf == 1) and (j % CHL == CHL - 1)
                    lhs = sel8[:, j, :] if L1 else sel[:, :, j]
                    nc.tensor.matmul(acc[:], lhsT=lhs,
                                     rhs=w[:, j, :], start=st, stop=sp)
            for k in (0, 1):
                t = 2 * pr + k
                if L1:
                    _fin_l1(nc, t, accs[k], pf, ps_fin, idn, oneall, W2a_sb,
                            W2b_sb, b2_sb, adst2_sb, t2slice)
                else:
                    _fin_l2(nc, t, accs[k], pf, outp)


def _fin_l1(nc, t, acc, pf, ps_fin, idn, oneall, W2a_sb, W2b_sb, b2_sb,
            adst2_sb, t2slice):
    # EPS keeps padded dst rows (den=0) finite — their garbage h1 values are
    # never read, but adst2 must stay finite (0*inf = NaN leaks via selT).
    deps = pf.tile([128, 8], F32, tag="deps", name="deps")
    nc.vector.tensor_scalar_add(deps[:], acc[:, 256:264], EPS)
    rec = pf.tile([128, 8], F32, tag="rec", name="rec")
    nc.vector.reciprocal(rec[:], deps[:])
    h1b = pf.tile([128, 256], F16, tag="h1b", name="h1b")
    nc.vector.tensor_tensor(
        out=h1b[:].rearrange("p (h c) -> p h c", h=8),
        in0=acc[:, 0:256].rearrange("p (h c) -> p h c", h=8),
        in1=rec[:, :, None].to_broadcast([128, 8, 32]),
        op=A.mult)
    if t == 0 and getattr(nc, "_dbg", None):
        accs_sb = pf.tile([128, 264], F32, tag="accdbg", name="accdbg")
        nc.vector.tensor_copy(accs_sb[:, 0:256], acc[:, 0:256])
        nc.vector.tensor_copy(accs_sb[:, 256:264], acc[:, 256:264])
        nc.sync.dma_start(out=nc._dbg["h1dbg"][:], in_=accs_sb[:])
    # ho = elu(h1b) + 1 = relu(h1b) + exp(-relu(-h1b)); the -1 is folded
    # into b2eff via W2ext (v1's ACT-based elu decomposition)
    r1 = pf.tile([128, 256], F16, tag="r1", name="r1")
    nc.scalar.activation(r1[:], h1b[:], AF.Relu, scale=-1.0)
    e1 = pf.tile([128, 256], F16, tag="e1", name="e1")
    nc.scalar.activation(e1[:], r1[:], AF.Exp, scale=-1.0)
    rl = pf.tile([128, 256], F16, tag="rl", name="rl")
    nc.scalar.activation(rl[:], h1b[:], AF.Relu)
    ho = pf.tile([128, 256], F16, tag="ho", name="ho")
    nc.gpsimd.tensor_tensor(out=ho[:], in0=rl[:], in1=e1[:], op=A.add)
    h2ps = ps_fin.tile([128, 66], F32, tag="h2ps", name="h2ps")
    for half in (0, 1):
        tp = ps_fin.tile([128, 128], F16, tag="tp", name="tp", bufs=1)
        nc.tensor.transpose(out=tp[:], in_=ho[:, half * 128:(half + 1) * 128],
                            identity=idn[:])
        hoT = pf.tile([128, 128], F16, tag="hoT", name="hoT")
        if half == 0:
            nc.vector.tensor_copy(hoT[:], tp[:])
        else:
            nc.scalar.copy(hoT[:], tp[:])
        nc.tensor.matmul(h2ps[:], lhsT=hoT[:],
                         rhs=(W2a_sb if half == 0 else W2b_sb)[:],
                         start=half == 0, stop=False)
    nc.tensor.matmul(h2ps[:], lhsT=oneall[:], rhs=b2_sb[:], start=False,
                     stop=True)
    t2r = pf.tile([128, 65], F16, tag="t2r", name="t2r")
    nc.scalar.copy(t2r[:], h2ps[:, 0:65])
    nc.vector.tensor_copy(adst2_sb[:, t:t + 1], h2ps[:, 65:66])
    nc.sync.dma_start(out=t2slice[t * 128:(t + 1) * 128, 0:65], in_=t2r[:])


def _fin_l2(nc, t, acc, pf, outp):
    rec = pf.tile([128, 1], F32, tag="rec2", name="rec2")
    nc.vector.reciprocal(rec[:], acc[:, 64:65])
    ot = pf.tile([128, 64], F32, tag="ot", name="ot")
    nc.vector.tensor_scalar_mul(ot[:], acc[:, 0:64], rec[:, 0:1])
    nc.sync.dma_start(out=outp[t * 128:(t + 1) * 128, :], in_=ot[:])


# ---------------------------------------------------------------------------
# entry point
# ---------------------------------------------------------------------------

def make_in_maps(T, common, per_core):
    in_maps = []
    for c in range(NCORES):
        m = {
            "W1ext": common["W1ext"], "W2ext": common["W2ext"],
            "b1ext": common["b1ext"], "b2e66": common["b2e66"],
            "iotarep": common["iotarep"],
        }
        pc = per_core[c]
        m.update({k: pc[k] for k in ("xTc", "g1idx", "g2idx", "dloc1",
                                     "dloc2", "selT1", "selT2", "adidx")})
        in_maps.append(m)
    return in_maps


def kernel(**inputs):
    T, common, per_core = host_prep(inputs)
    nc = build_nc(T)
    in_maps = make_in_maps(T, common, per_core)
    res = run_bass_kernel_spmd(nc, in_maps, core_ids=list(range(NCORES)))
    allrows = np.concatenate([res.results[c]["out"] for c in range(NCORES)],
                             axis=0)
    return allrows[common["slotrow"]].astype(np.float32)



# revision 25
# speedup vs baseline: 1.0459x; 1.0064x over previous
"""Two-layer GAT on 8 Trainium2 NeuronCores (Bass/Tile) — v2.

Changes vs v1 (889us):
  - Phase A sharded: each core computes T1 rows for its own 6250 nodes
    (padded to 6272), AllGather broadcasts the table (timing build: local
    copy of own contribution only).
  - b1 folded into T1 value columns (sum(alpha)=1), b2 and the elu "-1"
    folded into W2ext/b2eff at finalize.
  - selT (dst-onehot, d-on-partitions) shipped from host as fp8 and used
    directly as matmul lhsT (mixed fp8 x f16 matmul) — kills the dlT DMA,
    the DVE selT build, and the adf path stays cheap.
  - exp broadcast stays on ACT; leaky/et on DVE; PSUM->SBUF copies and
    t2r/adst2 finalization moved to Pool (gpsimd).
  - softmax denominator accumulated via a second per-chunk matmul with a
    strided rhs view of exx (no den-copy op).
  - asrc2/adst2 computed via the h2 matmul itself (W2ext has v_s|v_d cols).
"""

import os

import numpy as np

import concourse.bass as bass
import concourse.bacc as bacc
import concourse.tile as tile
import concourse.mybir as mybir
from concourse.bass_utils import run_bass_kernel_spmd
from concourse.masks import make_identity

F32 = mybir.dt.float32
F16 = mybir.dt.float16
F8 = mybir.dt.float8e4
I16 = mybir.dt.int16
I32 = mybir.dt.int32
A = mybir.AluOpType
AF = mybir.ActivationFunctionType
NP_F8 = mybir.dt.np(F8)

# -------- problem constants --------
N, E, IN, HID, OUT, H = 50000, 800000, 128, 32, 64, 8
C1 = H * HID  # 256
NCORES = 8
NPC = N // NCORES        # 6250 dst nodes per core
RPC = 6272               # T1 rows per core (6250 padded to 49*128)
NR = NCORES * RPC        # 50176 T1 rows
T1_LO = 4 * RPC          # 25088: rows of cores 0-3
T1_W = 384               # T1 row stride (768B) — gather granularity
CHL = 9                  # chunks per (tile, half)
CH = 2 * CHL             # chunks per gather call
CALLW = CH * 128         # 2304 edge slots per call
EPS = 1e-16
NEG = 0.2


def _row_of(n):
    """T1 row of node n (cores own contiguous 6250-node ranges, padded)."""
    c = n // NPC
    return c * RPC + (n - c * NPC)


# ---------------------------------------------------------------------------
# host-side preprocessing
# ---------------------------------------------------------------------------

def _prep_weights(W1, as1, ad1, b1, W2, as2, ad2, b2):
    As = np.zeros((C1, H), np.float32)
    Ad = np.zeros((C1, H), np.float32)
    for h in range(H):
        As[h * HID:(h + 1) * HID, h] = as1[h]
        Ad[h * HID:(h + 1) * HID, h] = ad1[h]
    W1ext = np.concatenate([W1, W1 @ As, W1 @ Ad], axis=1)  # [128, 272]
    iotarep = np.zeros((128, 128 * CH), np.float16)
    for d in range(128):
        iotarep[:, d * CH:(d + 1) * CH] = d
    b1ext = np.zeros((272,), np.float32)
    b1ext[:C1] = b1
    vs = W2 @ as2[0]   # [256]
    vd = W2 @ ad2[0]   # [256]
    W2ext = np.concatenate([W2, vs[:, None], vd[:, None]], axis=1)  # [256,66]
    b2eff = np.concatenate([b2 - W2.sum(0), [-vs.sum()], [-vd.sum()]])  # [66]
    return {
        "W1ext": W1ext.astype(np.float16),
        "W2ext": W2ext.astype(np.float16),
        "b1ext": np.tile(b1ext[None, :], (128, 1)).astype(np.float16),
        "b2e66": np.tile(b2eff[None, :], (128, 1)).astype(np.float16),
        "iotarep": iotarep,
    }


def _greedy_tiles(deg_lo1, deg_hi1, deg_lo2, deg_hi2):
    cap = CHL * 128
    tiles = []
    i, n = 0, len(deg_lo1)
    while i < n:
        l1 = h1 = l2 = h2 = 0
        j = i
        while j < n and j - i < 128:
            nl1, nh1 = l1 + deg_lo1[j], h1 + deg_hi1[j]
            nl2, nh2 = l2 + deg_lo2[j], h2 + deg_hi2[j]
            if nl1 > cap or nh1 > cap or nl2 > cap or nh2 > cap:
                break
            l1, h1, l2, h2 = nl1, nh1, nl2, nh2
            j += 1
        assert j > i, "single node exceeds chunk caps"
        tiles.append((i, j))
        i = j
    return tiles


def _pack_calls(rows_half, dloc_half, T, both=False):
    """Build gather idx [128, T*CH*8] i16, dloc [128, T*CH] f16, and the
    fp8 one-hot stream: selT only [128, T*CH*128], or selT|sel interleaved
    per call [128, T*2*CH*128] when both=True."""
    idx16 = np.zeros((128, T * CH * 8), np.int16)
    dloc = np.full((128, T * CH), -1, np.float16)
    dlocT_flat = np.full((T * CALLW,), -1, np.int32)
    for pr in range(T // 2):
        for h in (0, 1):
            g = 2 * pr + h
            rows = np.zeros((CALLW,), np.int64)
            dl = np.full((CALLW,), -1, np.int64)
            for k, t in enumerate((2 * pr, 2 * pr + 1)):
                r = rows_half.get((t, h))
                if r is None:
                    continue
                d = dloc_half[(t, h)]
                off = k * CHL * 128
                rows[off:off + len(r)] = r
                dl[off:off + len(r)] = d
            blk = rows.reshape(CH * 8, 16).T.astype(np.int16)
            idx16[:, g * CH * 8:(g + 1) * CH * 8] = np.tile(blk, (8, 1))
            dloc[:, g * CH:(g + 1) * CH] = dl.reshape(CH, 128).T.astype(np.float16)
            dlocT_flat[g * CALLW:(g + 1) * CALLW] = dl
    # selT[p, g*CH*128 + j*128 + e] = 1.0 iff dloc(edge (j,e) of g) == p
    selT = (dlocT_flat[None, :] == np.arange(128)[:, None])
    if not both:
        return idx16, dloc, selT.astype(NP_F8)
    # sel[p, g, j, d] = 1.0 iff dloc(edge (p,j) of g) == d
    sel = (dloc[:, :, None].astype(np.int32) ==
           np.arange(128)[None, None, :]).reshape(128, T, CH, 128)
    selB = np.stack([selT.reshape(128, T, CH, 128), sel], axis=2).reshape(
        128, T * 2 * CH * 128).astype(NP_F8)
    return idx16, dloc, selB


def _prep_core(c, src, dst):
    base = c * NPC
    own = (dst >= base) & (dst < base + NPC)
    s = src[own].astype(np.int64)
    d = (dst[own] - base).astype(np.int64)
    order = np.argsort(d, kind="stable")
    s, d = s[order], d[order]
    ptr = np.zeros(NPC + 1, np.int64)
    np.cumsum(np.bincount(d, minlength=NPC), out=ptr[1:])

    srow = _row_of(s)
    lo1m = srow < T1_LO
    lo2m = s < (N // 2)  # slot-major half split: cores 0-3 vs 4-7
    deg_lo1 = np.bincount(d, weights=lo1m, minlength=NPC).astype(np.int64)
    deg_hi1 = np.bincount(d, weights=~lo1m, minlength=NPC).astype(np.int64)
    deg_lo2 = np.bincount(d, weights=lo2m, minlength=NPC).astype(np.int64)
    deg_hi2 = np.bincount(d, weights=~lo2m, minlength=NPC).astype(np.int64)
    tiles = _greedy_tiles(deg_lo1, deg_hi1, deg_lo2, deg_hi2)

    rows1, dloc1 = {}, {}
    for t, (n0, n1) in enumerate(tiles):
        e0, e1 = ptr[n0], ptr[n1]
        es, ed = srow[e0:e1], d[e0:e1]
        dl = ed - n0
        m1 = es < T1_LO
        rows1[(t, 0)] = es[m1]
        dloc1[(t, 0)] = dl[m1]
        rows1[(t, 1)] = es[~m1] - T1_LO
        dloc1[(t, 1)] = dl[~m1]
    return {
        "Treal": len(tiles), "tiles": tiles, "s": s, "d": d, "ptr": ptr,
        "rows1": rows1, "dloc1": dloc1,
    }


def _finish_core(pc, c, T, slotrow):
    tiles = list(pc["tiles"]) + [(0, 0)] * (T - pc["Treal"])
    idx1, dloc1, selT1 = _pack_calls(pc["rows1"], pc["dloc1"], T, both=True)

    t2lo = 4 * T * 128
    s, d, ptr = pc["s"], pc["d"], pc["ptr"]
    srow = slotrow[s]
    rows2, dloc2 = {}, {}
    for t, (n0, n1) in enumerate(pc["tiles"]):
        e0, e1 = ptr[n0], ptr[n1]
        dl = d[e0:e1] - n0
        m2 = srow[e0:e1] < t2lo
        rows2[(t, 0)] = srow[e0:e1][m2]
        dloc2[(t, 0)] = dl[m2]
        rows2[(t, 1)] = srow[e0:e1][~m2] - t2lo
        dloc2[(t, 1)] = dl[~m2]
    idx2, dloc2a, selT2 = _pack_calls(rows2, dloc2, T)

    # adidx: local t1slice rows of each tile's nodes (slot-major, clamped),
    # in dma_gather int16 index layout, one call of T*128 rows
    p = np.arange(128)
    rows = np.zeros((T * 128,), np.int64)
    for t, (n0, n1) in enumerate(tiles):
        w = n1 - n0
        rows[t * 128:(t + 1) * 128] = n0 + np.minimum(p, max(w - 1, 0))
    blk = rows.reshape(T * 8, 16).T.astype(np.int16)
    adidx = np.tile(blk, (8, 1))  # [128, T*8]
    return {
        "g1idx": idx1, "dloc1": dloc1, "selT1": selT1,
        "g2idx": idx2, "dloc2": dloc2a, "selT2": selT2,
        "adidx": adidx,
    }


def host_prep(inputs):
    ei = np.asarray(inputs["edge_index"]).astype(np.int64)
    wd = _prep_weights(
        np.asarray(inputs["W1"], np.float32),
        np.asarray(inputs["att_src1"], np.float32),
        np.asarray(inputs["att_dst1"], np.float32),
        np.asarray(inputs["b1"], np.float32),
        np.asarray(inputs["W2"], np.float32),
        np.asarray(inputs["att_src2"], np.float32),
        np.asarray(inputs["att_dst2"], np.float32),
        np.asarray(inputs["b2"], np.float32),
    )
    loops = np.arange(N, dtype=np.int64)
    src = np.concatenate([ei[0], loops])
    dst = np.concatenate([ei[1], loops])

    x = np.asarray(inputs["x"], np.float32).astype(np.float16)

    cores = [_prep_core(c, src, dst) for c in range(NCORES)]
    T = max(pc["Treal"] for pc in cores)
    if T % 2:
        T += 1
    slotrow = np.zeros(N, np.int64)
    for c, pc in enumerate(cores):
        base_row = c * T * 128
        for t, (n0, n1) in enumerate(pc["tiles"]):
            nodes = c * NPC + np.arange(n0, n1)
            slotrow[nodes] = base_row + t * 128 + np.arange(n1 - n0)
    per_core = [_finish_core(pc, c, T, slotrow) for c, pc in enumerate(cores)]
    for c in range(NCORES):
        xc = np.zeros((IN, RPC), np.float16)
        xc[:, :NPC] = x[c * NPC:(c + 1) * NPC].T
        per_core[c]["xTc"] = xc

    common = dict(wd)
    common["slotrow"] = slotrow
    return T, common, per_core


# ---------------------------------------------------------------------------
# device program
# ---------------------------------------------------------------------------

def _gather_raw(eng, out_ap, in_ap, idxs_ap, num_idxs, elem_size, elem_step):
    """dma_gather with elem_size_bytes not a multiple of 256B (non-transpose
    path only; the 256B rule is a transpose-mode restriction — the Q7 kernel
    packets arbitrary elem sizes, only the row stride is encoded in 256B
    units).  Mirrors BassGpSimd.dma_gather's construction."""
    from concourse.ap_utils import ap_is_contiguous
    import concourse.mybir as mb
    assert idxs_ap.dtype == mybir.dt.int16
    assert in_ap.dtype == out_ap.dtype
    elem_size_bytes = elem_size * mybir.dt.size(in_ap.dtype)
    assert in_ap.ap[-1][1] == out_ap.ap[-1][1] == elem_size
    assert ap_is_contiguous(out_ap.ap[1:])
    assert ap_is_contiguous(idxs_ap.ap[1:])
    assert in_ap.ap[0][0] == elem_step
    stride_bytes = elem_step * mybir.dt.size(in_ap.dtype)
    assert stride_bytes % 256 == 0 and stride_bytes // 256 < 256
    _in_ap = eng.lower_ap_dma(in_ap, for_custom_bir_dma=True)
    _idxs_ap = eng.lower_ap(idxs_ap)
    _out_ap = eng.lower_ap(out_ap)
    return eng.add_instruction(
        mb.InstDMAGatherAnt(
            name=eng.bass.get_next_instruction_name(),
            ins=[*_in_ap, _idxs_ap,
                 eng.lower_val_access(eng.to_reg(num_idxs))],
            outs=[_out_ap],
            transpose=False,
            num_idxs=num_idxs,
            elem_size=elem_size,
            stride_bytes_256=stride_bytes // 256,
            gen_mode=0,
            single_packet=False,
            queue_num=0,
            sbuf_tokens_per_rank=0,
            sbuf_free_dim_per_rank=0,
            sbuf_free_dim_pad_per_rank=0,
            sbuf_byte_offset=0,
        )
    )


def build_nc(T, num_devices=NCORES, with_collective=True, phases="ABCD",
             dbg=False):
    nc = bacc.Bacc("TRN2", target_bir_lowering=False, debug=False,
                   num_devices=num_devices)
    dt = nc.dram_tensor
    xTc = dt("xTc", [IN, RPC], F16, kind="ExternalInput").ap()
    W1ext = dt("W1ext", [128, 272], F16, kind="ExternalInput").ap()
    W2ext = dt("W2ext", [256, 66], F16, kind="ExternalInput").ap()
    b1ext = dt("b1ext", [128, 272], F16, kind="ExternalInput").ap()
    b2e66 = dt("b2e66", [128, 66], F16, kind="ExternalInput").ap()
    iotarep = dt("iotarep", [128, 128 * CH], F16, kind="ExternalInput").ap()
    g1idx = dt("g1idx", [128, T * CH * 8], I16, kind="ExternalInput").ap()
    g2idx = dt("g2idx", [128, T * CH * 8], I16, kind="ExternalInput").ap()
    dloc1 = dt("dloc1", [128, T * CH], F16, kind="ExternalInput").ap()
    dloc2 = dt("dloc2", [128, T * CH], F16, kind="ExternalInput").ap()
    selT1 = dt("selT1", [128, T * 2 * CH * 128], F8, kind="ExternalInput").ap()
    selT2 = dt("selT2", [128, T * CH * 128], F8, kind="ExternalInput").ap()
    adidx = dt("adidx", [128, T * 8], I16, kind="ExternalInput").ap()
    t1slice = dt("t1slice", [RPC, T1_W], F16, kind="Internal").ap()
    T1 = dt("T1", [NR, T1_W], F16, kind="Internal",
            addr_space="Shared" if with_collective else "Local").ap()
    t2rows = T * 128
    # t2 rows are fp8: 64 fp8 h2 values + asrc2 as raw f16 in bytes 64:66,
    # padded to a 256B stride (gather stride must be a 256B multiple). The
    # 66B gather elem rides the 7ns/desc floor instead of f16's 11.6ns.
    t2slice = dt("t2slice", [t2rows, 256], F8, kind="Internal").ap()
    t2full = dt("t2full", [NCORES * t2rows, 256], F8, kind="Internal",
                addr_space="Shared" if with_collective else "Local").ap()
    outp = dt("out", [t2rows, 64], F32, kind="ExternalOutput").ap()
    if dbg:
        t1dbg = dt("t1dbg", [RPC, T1_W], F16, kind="ExternalOutput").ap()
        addbg = dt("addbg", [128, T * 8], F16, kind="ExternalOutput").ap()
        t2dbg = dt("t2dbg", [t2rows, 128], F16, kind="ExternalOutput").ap()
        gtdbg = dt("gtdbg", [128, CH * 264], F16, kind="ExternalOutput").ap()
        etdbg = dt("etdbg", [128, CH * 8], F16, kind="ExternalOutput").ap()
        wdbg = dt("wdbg", [128, CH * 256], F16, kind="ExternalOutput").ap()
        lkdbg = dt("lkdbg", [128, CH * 8], F16, kind="ExternalOutput").ap()
        exxdbg = dt("exxdbg", [128, CH * 256], F16,
                    kind="ExternalOutput").ap()
        h1dbg = dt("h1dbg", [128, 256 + 8], F32, kind="ExternalOutput").ap()
        nc._dbg = dict(gtdbg=gtdbg, etdbg=etdbg, wdbg=wdbg, h1dbg=h1dbg,
                       lkdbg=lkdbg, exxdbg=exxdbg)
    else:
        nc._dbg = None

    with tile.TileContext(nc) as tc:
        with tc.tile_pool(name="consts", bufs=1) as cp:
            W1e_sb = cp.tile([128, 272], F16)
            nc.sync.dma_start(out=W1e_sb[:], in_=W1ext[:])
            W2a_sb = cp.tile([128, 66], F16)
            nc.sync.dma_start(out=W2a_sb[:], in_=W2ext[0:128, :])
            W2b_sb = cp.tile([128, 66], F16)
            nc.sync.dma_start(out=W2b_sb[:], in_=W2ext[128:256, :])
            b1_sb = cp.tile([128, 272], F16)
            nc.sync.dma_start(out=b1_sb[:], in_=b1ext[:])
            b2_sb = cp.tile([128, 66], F16)
            nc.sync.dma_start(out=b2_sb[:], in_=b2e66[:])
            oneall = cp.tile([128, 128], F16)
            nc.vector.memset(oneall[:], 1.0 / 128.0)
            iot_sb = cp.tile([128, 128 * CH], F16)
            nc.sync.dma_start(out=iot_sb[:], in_=iotarep[:])
            dl1_sb = cp.tile([128, T * CH], F16)
            nc.sync.dma_start(out=dl1_sb[:], in_=dloc1[:])
            dl2_sb = cp.tile([128, T * CH], F16)
            nc.sync.dma_start(out=dl2_sb[:], in_=dloc2[:])
            idn = cp.tile([128, 128], F16)
            make_identity(nc, idn[:])
            g1i_sb = cp.tile([128, T * CH * 8], I16)
            nc.sync.dma_start(out=g1i_sb[:], in_=g1idx[:])
            dl1_sb = cp.tile([128, T * CH], F16)
            nc.sync.dma_start(out=dl1_sb[:], in_=dloc1[:])
            g2i_sb = cp.tile([128, T * CH * 8], I16)
            nc.sync.dma_start(out=g2i_sb[:], in_=g2idx[:])
            adidx_sb = cp.tile([128, T * 8], I16)
            nc.sync.dma_start(out=adidx_sb[:], in_=adidx[:])
            adtall_sb = cp.tile([128, T, 8], F16)  # bulk a_dst gather target
            adst2_sb = cp.tile([128, T], F16)  # written in B-fin, read in D

            # ---------------- Phase A: own T1 slice ----------------
            if "A" in phases:
                with tc.tile_pool(name="pa", bufs=2) as pa, \
                     tc.tile_pool(name="paps", bufs=4, space="PSUM") as paps:
                    XB = 2048
                    nblk = (RPC + XB - 1) // XB
                    for blk in range(nblk):
                        n0 = blk * XB
                        bw = min(XB, RPC - n0)
                        nt = bw // 128
                        xb = pa.tile([128, XB], F16, tag="xb", name="xb")
                        nc.sync.dma_start(out=xb[:, 0:bw],
                                          in_=xTc[:, n0:n0 + bw])
                        t1b = pa.tile([128, 16, 272], F16, tag="t1b",
                                      name="t1b")
                        for i in range(nt):
                            ps = paps.tile([128, 272], F32, tag="aps",
                                           name="aps")
                            nc.tensor.matmul(ps[:],
                                             lhsT=xb[:, i * 128:(i + 1) * 128],
                                             rhs=W1e_sb[:], start=True,
                                             stop=False)
                            nc.tensor.matmul(ps[:], lhsT=oneall[:],
                                             rhs=b1_sb[:], start=False,
                                             stop=True)
                            if i % 2 == 0:
                                nc.vector.tensor_copy(t1b[:, i, :], ps[:])
                            else:
                                nc.scalar.copy(t1b[:, i, :], ps[:])
                        nc.sync.dma_start(
                            out=t1slice[n0:n0 + bw, 0:272].rearrange(
                                "(i p) c -> p i c", p=128),
                            in_=t1b[:, 0:nt, :])
                        if not with_collective and "B" in phases:
                            nc.sync.dma_start(
                                out=T1[n0:n0 + bw, 0:272],
                                in_=t1slice[n0:n0 + bw, 0:272])

            # ---------------- AllGather T1 ----------------
            if "B" in phases:
                if with_collective:
                    nc.gpsimd.collective_compute(
                        "AllGather", A.bypass,
                        replica_groups=[list(range(NCORES))],
                        ins=[t1slice[:]], outs=[T1[:]],
                    )
                # bulk a_dst gather: one call for all T tiles' 128 slots
                _gather_raw(nc.gpsimd, adtall_sb[:],
                            t1slice[0:RPC, 264:272], adidx_sb[:],
                            T * 128, 8, T1_W)

                # ---------------- Phase B: layer-1 aggregation ----------------
                _agg_layer(nc, tc, T, layer=1,
                           tbl_lo=T1[0:T1_LO, 0:264],
                           tbl_hi=T1[T1_LO:NR, 0:264],
                           gidx_sb=g1i_sb, dloc_sb=dl1_sb, selT_in=selT1,
                           iot_sb=iot_sb, idn=idn, oneall=oneall,
                           adtall_sb=adtall_sb,
                           W2a_sb=W2a_sb, W2b_sb=W2b_sb, b2_sb=b2_sb,
                           adst2_sb=adst2_sb,
                           t2slice=t2slice, outp=None)

            if "D" in phases:
                nc.sync.dma_start(out=g2i_sb[:], in_=g2idx[:])
                nc.sync.dma_start(out=dl2_sb[:], in_=dloc2[:])

            if dbg:
                nc.sync.dma_start(out=t1dbg[:], in_=t1slice[:])
                nc.sync.dma_start(
                    out=addbg[:],
                    in_=adtall_sb[:].rearrange("p t c -> p (t c)"))
                nc.sync.dma_start(out=t2dbg[:], in_=t2slice[:])

            # ---------------- AllGather T2 ----------------
            if "C" in phases:
                if with_collective:
                    nc.gpsimd.collective_compute(
                        "AllGather", A.bypass,
                        replica_groups=[list(range(NCORES))],
                        ins=[t2slice[:]], outs=[t2full[:]],
                    )
                else:
                    nc.sync.dma_start(out=t2full[0:t2rows, :], in_=t2slice[:])

            # ---------------- Phase D: layer-2 aggregation ----------------
            if "D" in phases:
                _agg_layer(nc, tc, T, layer=2,
                           tbl_lo=t2full[0:4 * t2rows, 0:68],
                           tbl_hi=t2full[4 * t2rows:8 * t2rows, 0:68],
                           gidx_sb=g2i_sb, dloc_sb=dl2_sb, selT_in=selT2,
                           iot_sb=iot_sb, idn=idn, oneall=None,
                           adtall_sb=None,
                           W2a_sb=None, W2b_sb=None, b2_sb=None,
                           adst2_sb=adst2_sb,
                           t2slice=None, outp=outp)

    nc.compile()
    return nc


def _agg_layer(nc, tc, T, layer, tbl_lo, tbl_hi, gidx_sb, dloc_sb, selT_in,
               iot_sb, idn, oneall, adtall_sb, W2a_sb, W2b_sb, b2_sb,
               adst2_sb, t2slice, outp):
    L1 = layer == 1
    GW = 264 if L1 else 68   # gathered elements per row (payload)
    GS = T1_W if L1 else 256  # table row stride in elements
    NH = 8 if L1 else 1
    VC = 256 if L1 else 64
    ACC_W = 264 if L1 else 65
    name = f"l{layer}"
    PBB = int(os.environ.get("V2_PBB", "3"))
    ACCB = int(os.environ.get("V2_ACCB", "3"))
    with tc.tile_pool(name=f"pb_{name}", bufs=PBB) as pb, \
         tc.tile_pool(name=f"pf_{name}", bufs=2) as pf, \
         tc.tile_pool(name=f"ps_acc_{name}", bufs=ACCB, space="PSUM") as ps_acc, \
         tc.tile_pool(name=f"ps_ad_{name}", bufs=2, space="PSUM") as ps_ad, \
         tc.tile_pool(name=f"ps_fin_{name}", bufs=2, space="PSUM") as ps_fin:
        for pr in range(T // 2):
            accs = [ps_acc.tile([128, ACC_W], F32, tag="acc", name="acc_a"),
                    ps_acc.tile([128, ACC_W], F32, tag="acc", name="acc_b")]
            if L1:
                scp = pb.tile([128, 2, 2, CH, 128], F8, tag="scp",
                              name="scp", bufs=2)
                nc.sync.dma_start(
                    out=scp[:].rearrange("p f s j e -> p (f s j e)"),
                    in_=selT_in[:, (2 * pr) * 2 * CALLW:
                                (2 * pr + 2) * 2 * CALLW])
            for hf in (0, 1):
                g = 2 * pr + hf
                if L1:
                    s8 = scp[:, hf, 0]     # selT: [d-part, j, e]
                    sel8 = scp[:, hf, 1]   # sel:  [e-part, j, d]
                else:
                    s8t = pb.tile([128, CH, 128], F8, tag="s8", name="s8",
                                  bufs=3)
                    nc.sync.dma_start(
                        out=s8t[:].rearrange("p j e -> p (j e)"),
                        in_=selT_in[:, g * CALLW:(g + 1) * CALLW])
                    s8 = s8t[:]
                gt = pb.tile([128, CH, GW], F16 if L1 else F8, tag="gt",
                             name="gt", bufs=3)
                if os.environ.get("V2_GSPLIT", "0") == "1":
                    for gh in (0, 1):
                        _gather_raw(
                            nc.gpsimd, gt[:, gh * CHL:(gh + 1) * CHL],
                            tbl_lo if hf == 0 else tbl_hi,
                            gidx_sb[:, g * CH * 8 + gh * CHL * 8:
                                    g * CH * 8 + (gh + 1) * CHL * 8],
                            CHL * 128, GW, GS)
                else:
                    _gather_raw(
                        nc.gpsimd, gt[:], tbl_lo if hf == 0 else tbl_hi,
                        gidx_sb[:, g * CH * 8:(g + 1) * CH * 8],
                        CALLW, GW, GS)
                if not L1:
                    sel = pb.tile([128, 128, CH], F16, tag="sel", name="sel")
                    nc.vector.tensor_tensor(
                        out=sel[:],
                        in0=dloc_sb[:, None,
                                    g * CH:(g + 1) * CH].to_broadcast(
                            [128, 128, CH]),
                        in1=iot_sb[:].rearrange("p (d j) -> p d j", j=CH),
                        op=A.is_equal)
                # per-edge et = a_dst + a_src entirely in PSUM: the fp8 selT
                # one-hot matmul broadcasts a_dst, then an identity matmul
                # accumulates the gathered a_src columns on top.
                adps = ps_ad.tile([128, CH, NH], F32, tag="adps", name="adps")
                asrc_ap = (gt[:, :, 256:264] if L1
                           else gt[:, :, 64:66].bitcast(F16))
                for j in range(CH):
                    t = 2 * pr + (0 if j < CHL else 1)
                    rhs = adtall_sb[:, t, :] if L1 else adst2_sb[:, t:t + 1]
                    nc.tensor.matmul(adps[:, j, :], lhsT=s8[:, j, :], rhs=rhs,
                                     start=True, stop=False)
                    nc.tensor.matmul(adps[:, j, :], lhsT=idn[:],
                                     rhs=asrc_ap[:, j, :], start=False,
                                     stop=True)
                lk = pb.tile([128, CH, NH], F16, tag="lk", name="lk")
                nc.scalar.activation(lk[:], adps[:], AF.Prelu, alpha=NEG)
                # exp at pair width on ACT (cheap), broadcast to the value
                # width inside the DVE multiply via a stride-0 middle dim —
                # the last dim stays packed so the mult keeps 2x DVE mode.
                exf = pb.tile([128, CH, NH, 2], F16, tag="exf", name="exf")
                nc.scalar.activation(
                    exf[:], lk[:, :, :, None].to_broadcast([128, CH, NH, 2]),
                    AF.Exp)
                w = pb.tile([128, CH, ACC_W], F16, tag="w", name="w")
                nc.vector.tensor_copy(w[:, :, VC:ACC_W], exf[:, :, :, 0])
                cph = VC // NH // 2  # 16 (L1) / 32 (L2) value pairs per head
                nc.vector.tensor_tensor(
                    out=w[:, :, 0:VC].rearrange("p j (h k two) -> p j h k two",
                                                h=NH, two=2),
                    in0=gt[:, :, 0:VC].rearrange("p j (h k two) -> p j h k two",
                                                 h=NH, two=2),
                    in1=exf[:, :, :, None, :].to_broadcast(
                        [128, CH, NH, cph, 2]),
                    op=A.mult)
                for j in range(CH):
                    acc = accs[0 if j < CHL else 1]
                    st = (hf == 0) and (j % CHL == 0)
                    sp = (hf == 1) and (j % CHL == CHL - 1)
                    lhs = sel8[:, j, :] if L1 else sel[:, :, j]
                    nc.tensor.matmul(acc[:], lhsT=lhs,
                                     rhs=w[:, j, :], start=st, stop=sp)
            for k in (0, 1):
                t = 2 * pr + k
                if L1:
                    _fin_l1(nc, t, accs[k], pf, ps_fin, idn, oneall, W2a_sb,
                            W2b_sb, b2_sb, adst2_sb, t2slice)
                else:
                    _fin_l2(nc, t, accs[k], pf, outp)


def _fin_l1(nc, t, acc, pf, ps_fin, idn, oneall, W2a_sb, W2b_sb, b2_sb,
            adst2_sb, t2slice):
    # EPS keeps padded dst rows (den=0) finite — their garbage h1 values are
    # never read, but adst2 must stay finite (0*inf = NaN leaks via selT).
    deps = pf.tile([128, 8], F32, tag="deps", name="deps")
    nc.vector.tensor_scalar_add(deps[:], acc[:, 256:264], EPS)
    rec = pf.tile([128, 8], F32, tag="rec", name="rec")
    nc.vector.reciprocal(rec[:], deps[:])
    h1b = pf.tile([128, 256], F16, tag="h1b", name="h1b")
    nc.vector.tensor_tensor(
        out=h1b[:].rearrange("p (h c) -> p h c", h=8),
        in0=acc[:, 0:256].rearrange("p (h c) -> p h c", h=8),
        in1=rec[:, :, None].to_broadcast([128, 8, 32]),
        op=A.mult)
    if t == 0 and getattr(nc, "_dbg", None):
        accs_sb = pf.tile([128, 264], F32, tag="accdbg", name="accdbg")
        nc.vector.tensor_copy(accs_sb[:, 0:256], acc[:, 0:256])
        nc.vector.tensor_copy(accs_sb[:, 256:264], acc[:, 256:264])
        nc.sync.dma_start(out=nc._dbg["h1dbg"][:], in_=accs_sb[:])
    # ho = elu(h1b) + 1 = relu(h1b) + exp(-relu(-h1b)); the -1 is folded
    # into b2eff via W2ext (v1's ACT-based elu decomposition)
    r1 = pf.tile([128, 256], F16, tag="r1", name="r1")
    nc.scalar.activation(r1[:], h1b[:], AF.Relu, scale=-1.0)
    e1 = pf.tile([128, 256], F16, tag="e1", name="e1")
    nc.scalar.activation(e1[:], r1[:], AF.Exp, scale=-1.0)
    rl = pf.tile([128, 256], F16, tag="rl", name="rl")
    nc.scalar.activation(rl[:], h1b[:], AF.Relu)
    ho = pf.tile([128, 256], F16, tag="ho", name="ho")
    nc.gpsimd.tensor_tensor(out=ho[:], in0=rl[:], in1=e1[:], op=A.add)
    h2ps = ps_fin.tile([128, 66], F32, tag="h2ps", name="h2ps")
    for half in (0, 1):
        tp = ps_fin.tile([128, 128], F16, tag="tp", name="tp", bufs=1)
        nc.tensor.transpose(out=tp[:], in_=ho[:, half * 128:(half + 1) * 128],
                            identity=idn[:])
        hoT = pf.tile([128, 128], F16, tag="hoT", name="hoT")
        if half == 0:
            nc.vector.tensor_copy(hoT[:], tp[:])
        else:
            nc.scalar.copy(hoT[:], tp[:])
        nc.tensor.matmul(h2ps[:], lhsT=hoT[:],
                         rhs=(W2a_sb if half == 0 else W2b_sb)[:],
                         start=half == 0, stop=False)
    nc.tensor.matmul(h2ps[:], lhsT=oneall[:], rhs=b2_sb[:], start=False,
                     stop=True)
    t2r = pf.tile([128, 66], F8, tag="t2r", name="t2r")
    nc.scalar.copy(t2r[:, 0:64], h2ps[:, 0:64])
    nc.scalar.copy(t2r[:, 64:66].bitcast(F16), h2ps[:, 64:65])
    nc.vector.tensor_copy(adst2_sb[:, t:t + 1], h2ps[:, 65:66])
    nc.sync.dma_start(out=t2slice[t * 128:(t + 1) * 128, 0:66], in_=t2r[:])


def _fin_l2(nc, t, acc, pf, outp):
    rec = pf.tile([128, 1], F32, tag="rec2", name="rec2")
    nc.vector.reciprocal(rec[:], acc[:, 64:65])
    ot = pf.tile([128, 64], F32, tag="ot", name="ot")
    nc.vector.tensor_scalar_mul(ot[:], acc[:, 0:64], rec[:, 0:1])
    nc.sync.dma_start(out=outp[t * 128:(t + 1) * 128, :], in_=ot[:])


# ---------------------------------------------------------------------------
# entry point
# ---------------------------------------------------------------------------

def make_in_maps(T, common, per_core):
    in_maps = []
    for c in range(NCORES):
        m = {
            "W1ext": common["W1ext"], "W2ext": common["W2ext"],
            "b1ext": common["b1ext"], "b2e66": common["b2e66"],
            "iotarep": common["iotarep"],
        }
        pc = per_core[c]
        m.update({k: pc[k] for k in ("xTc", "g1idx", "g2idx", "dloc1",
                                     "dloc2", "selT1", "selT2", "adidx")})
        in_maps.append(m)
    return in_maps


def kernel(**inputs):
    T, common, per_core = host_prep(inputs)
    nc = build_nc(T)
    in_maps = make_in_maps(T, common, per_core)
    res = run_bass_kernel_spmd(nc, in_maps, core_ids=list(range(NCORES)))
    allrows = np.concatenate([res.results[c]["out"] for c in range(NCORES)],
                             axis=0)
    return allrows[common["slotrow"]].astype(np.float32)



# revision 36
# speedup vs baseline: 1.0735x; 1.0264x over previous
"""Two-layer GAT on 8 Trainium2 NeuronCores (Bass/Tile) — v2.

Changes vs v1 (889us):
  - Phase A sharded: each core computes T1 rows for its own 6250 nodes
    (padded to 6272), AllGather broadcasts the table (timing build: local
    copy of own contribution only).
  - b1 folded into T1 value columns (sum(alpha)=1), b2 and the elu "-1"
    folded into W2ext/b2eff at finalize.
  - selT (dst-onehot, d-on-partitions) shipped from host as fp8 and used
    directly as matmul lhsT (mixed fp8 x f16 matmul) — kills the dlT DMA,
    the DVE selT build, and the adf path stays cheap.
  - exp broadcast stays on ACT; leaky/et on DVE; PSUM->SBUF copies and
    t2r/adst2 finalization moved to Pool (gpsimd).
  - softmax denominator accumulated via a second per-chunk matmul with a
    strided rhs view of exx (no den-copy op).
  - asrc2/adst2 computed via the h2 matmul itself (W2ext has v_s|v_d cols).
"""

import os

import numpy as np

import concourse.bass as bass
import concourse.bacc as bacc
import concourse.tile as tile
import concourse.mybir as mybir
from concourse.bass_utils import run_bass_kernel_spmd
from concourse.masks import make_identity

F32 = mybir.dt.float32
F16 = mybir.dt.float16
F8 = mybir.dt.float8e4
I16 = mybir.dt.int16
I32 = mybir.dt.int32
A = mybir.AluOpType
AF = mybir.ActivationFunctionType
NP_F8 = mybir.dt.np(F8)

# -------- problem constants --------
N, E, IN, HID, OUT, H = 50000, 800000, 128, 32, 64, 8
C1 = H * HID  # 256
NCORES = 8
NPC = N // NCORES        # 6250 dst nodes per core
RPC = 6272               # T1 rows per core (6250 padded to 49*128)
NR = NCORES * RPC        # 50176 T1 rows
T1_LO = 4 * RPC          # 25088: rows of cores 0-3
T1_W = 384               # T1 row stride (768B) — gather granularity
CHL = 9                  # chunks per (tile, half)
CH = 2 * CHL             # chunks per gather call
CALLW = CH * 128         # 2304 edge slots per call
EPS = 1e-16
NEG = 0.2


def _row_of(n):
    """T1 row of node n (cores own contiguous 6250-node ranges, padded)."""
    c = n // NPC
    return c * RPC + (n - c * NPC)


# ---------------------------------------------------------------------------
# host-side preprocessing
# ---------------------------------------------------------------------------

def _prep_weights(W1, as1, ad1, b1, W2, as2, ad2, b2):
    As = np.zeros((C1, H), np.float32)
    Ad = np.zeros((C1, H), np.float32)
    for h in range(H):
        As[h * HID:(h + 1) * HID, h] = as1[h]
        Ad[h * HID:(h + 1) * HID, h] = ad1[h]
    W1ext = np.concatenate([W1, W1 @ As, W1 @ Ad], axis=1)  # [128, 272]
    iotarep = np.zeros((128, 128 * CH), np.float16)
    for d in range(128):
        iotarep[:, d * CH:(d + 1) * CH] = d
    b1ext = np.zeros((272,), np.float32)
    b1ext[:C1] = b1
    vs = W2 @ as2[0]   # [256]
    vd = W2 @ ad2[0]   # [256]
    W2ext = np.concatenate([W2, vs[:, None], vd[:, None]], axis=1)  # [256,66]
    b2eff = np.concatenate([b2 - W2.sum(0), [-vs.sum()], [-vd.sum()]])  # [66]
    return {
        "W1ext": W1ext.astype(np.float16),
        "W2ext": W2ext.astype(np.float16),
        "b1ext": np.tile(b1ext[None, :], (128, 1)).astype(np.float16),
        "b2e66": np.tile(b2eff[None, :], (128, 1)).astype(np.float16),
        "iotarep": iotarep,
    }


def _greedy_tiles(deg_lo1, deg_hi1, deg_lo2, deg_hi2):
    cap = CHL * 128
    tiles = []
    i, n = 0, len(deg_lo1)
    while i < n:
        l1 = h1 = l2 = h2 = 0
        j = i
        while j < n and j - i < 128:
            nl1, nh1 = l1 + deg_lo1[j], h1 + deg_hi1[j]
            nl2, nh2 = l2 + deg_lo2[j], h2 + deg_hi2[j]
            if nl1 > cap or nh1 > cap or nl2 > cap or nh2 > cap:
                break
            l1, h1, l2, h2 = nl1, nh1, nl2, nh2
            j += 1
        assert j > i, "single node exceeds chunk caps"
        tiles.append((i, j))
        i = j
    return tiles


def _pack_calls(rows_half, dloc_half, T, both=False):
    """Build gather idx [128, T*CH*8] i16, dloc [128, T*CH] f16, and the
    fp8 one-hot stream: selT only [128, T*CH*128], or selT|sel interleaved
    per call [128, T*2*CH*128] when both=True."""
    idx16 = np.zeros((128, T * CH * 8), np.int16)
    dloc = np.full((128, T * CH), -1, np.float16)
    dlocT_flat = np.full((T * CALLW,), -1, np.int32)
    for pr in range(T // 2):
        for h in (0, 1):
            g = 2 * pr + h
            rows = np.zeros((CALLW,), np.int64)
            dl = np.full((CALLW,), -1, np.int64)
            for k, t in enumerate((2 * pr, 2 * pr + 1)):
                r = rows_half.get((t, h))
                if r is None:
                    continue
                d = dloc_half[(t, h)]
                off = k * CHL * 128
                rows[off:off + len(r)] = r
                dl[off:off + len(r)] = d
            blk = rows.reshape(CH * 8, 16).T.astype(np.int16)
            idx16[:, g * CH * 8:(g + 1) * CH * 8] = np.tile(blk, (8, 1))
            dloc[:, g * CH:(g + 1) * CH] = dl.reshape(CH, 128).T.astype(np.float16)
            dlocT_flat[g * CALLW:(g + 1) * CALLW] = dl
    # selT[p, g*CH*128 + j*128 + e] = 1.0 iff dloc(edge (j,e) of g) == p
    selT = (dlocT_flat[None, :] == np.arange(128)[:, None])
    if not both:
        return idx16, dloc, selT.astype(NP_F8)
    # sel[p, g, j, d] = 1.0 iff dloc(edge (p,j) of g) == d
    sel = (dloc[:, :, None].astype(np.int32) ==
           np.arange(128)[None, None, :]).reshape(128, T, CH, 128)
    selB = np.stack([selT.reshape(128, T, CH, 128), sel], axis=2).reshape(
        128, T * 2 * CH * 128).astype(NP_F8)
    return idx16, dloc, selB


def _prep_core(c, src, dst):
    base = c * NPC
    own = (dst >= base) & (dst < base + NPC)
    s = src[own].astype(np.int64)
    d = (dst[own] - base).astype(np.int64)
    order = np.argsort(d, kind="stable")
    s, d = s[order], d[order]
    ptr = np.zeros(NPC + 1, np.int64)
    np.cumsum(np.bincount(d, minlength=NPC), out=ptr[1:])

    srow = _row_of(s)
    lo1m = srow < T1_LO
    lo2m = s < (N // 2)  # slot-major half split: cores 0-3 vs 4-7
    deg_lo1 = np.bincount(d, weights=lo1m, minlength=NPC).astype(np.int64)
    deg_hi1 = np.bincount(d, weights=~lo1m, minlength=NPC).astype(np.int64)
    deg_lo2 = np.bincount(d, weights=lo2m, minlength=NPC).astype(np.int64)
    deg_hi2 = np.bincount(d, weights=~lo2m, minlength=NPC).astype(np.int64)
    tiles = _greedy_tiles(deg_lo1, deg_hi1, deg_lo2, deg_hi2)

    rows1, dloc1 = {}, {}
    for t, (n0, n1) in enumerate(tiles):
        e0, e1 = ptr[n0], ptr[n1]
        es, ed = srow[e0:e1], d[e0:e1]
        dl = ed - n0
        m1 = es < T1_LO
        rows1[(t, 0)] = es[m1]
        dloc1[(t, 0)] = dl[m1]
        rows1[(t, 1)] = es[~m1] - T1_LO
        dloc1[(t, 1)] = dl[~m1]
    return {
        "Treal": len(tiles), "tiles": tiles, "s": s, "d": d, "ptr": ptr,
        "rows1": rows1, "dloc1": dloc1,
    }


def _finish_core(pc, c, T, slotrow):
    tiles = list(pc["tiles"]) + [(0, 0)] * (T - pc["Treal"])
    idx1, dloc1, selT1 = _pack_calls(pc["rows1"], pc["dloc1"], T, both=True)

    t2lo = 4 * T * 128
    s, d, ptr = pc["s"], pc["d"], pc["ptr"]
    srow = slotrow[s]
    rows2, dloc2 = {}, {}
    for t, (n0, n1) in enumerate(pc["tiles"]):
        e0, e1 = ptr[n0], ptr[n1]
        dl = d[e0:e1] - n0
        m2 = srow[e0:e1] < t2lo
        rows2[(t, 0)] = srow[e0:e1][m2]
        dloc2[(t, 0)] = dl[m2]
        rows2[(t, 1)] = srow[e0:e1][~m2] - t2lo
        dloc2[(t, 1)] = dl[~m2]
    # idx2 only: the layer-2 slot layout (dloc/sel/selT) is identical to
    # layer-1's by construction — both split halves by core(src) < 4 and
    # keep the same dst-sorted order — so selT1 is reused in phase D.
    idx2, dloc2a, _ = _pack_calls(rows2, dloc2, T)
    assert all(np.array_equal(dloc2[k], pc["dloc1"][k]) for k in dloc2)

    # adidx: local t1slice rows of each tile's nodes (slot-major, clamped),
    # in dma_gather int16 index layout, one call of T*128 rows
    p = np.arange(128)
    rows = np.zeros((T * 128,), np.int64)
    for t, (n0, n1) in enumerate(tiles):
        w = n1 - n0
        rows[t * 128:(t + 1) * 128] = n0 + np.minimum(p, max(w - 1, 0))
    blk = rows.reshape(T * 8, 16).T.astype(np.int16)
    adidx = np.tile(blk, (8, 1))  # [128, T*8]
    return {
        "g1idx": idx1, "selT1": selT1,
        "g2idx": idx2,
        "adidx": adidx,
    }


def host_prep(inputs):
    ei = np.asarray(inputs["edge_index"]).astype(np.int64)
    wd = _prep_weights(
        np.asarray(inputs["W1"], np.float32),
        np.asarray(inputs["att_src1"], np.float32),
        np.asarray(inputs["att_dst1"], np.float32),
        np.asarray(inputs["b1"], np.float32),
        np.asarray(inputs["W2"], np.float32),
        np.asarray(inputs["att_src2"], np.float32),
        np.asarray(inputs["att_dst2"], np.float32),
        np.asarray(inputs["b2"], np.float32),
    )
    loops = np.arange(N, dtype=np.int64)
    src = np.concatenate([ei[0], loops])
    dst = np.concatenate([ei[1], loops])

    x = np.asarray(inputs["x"], np.float32).astype(np.float16)

    cores = [_prep_core(c, src, dst) for c in range(NCORES)]
    T = max(pc["Treal"] for pc in cores)
    if T % 2:
        T += 1
    slotrow = np.zeros(N, np.int64)
    for c, pc in enumerate(cores):
        base_row = c * T * 128
        for t, (n0, n1) in enumerate(pc["tiles"]):
            nodes = c * NPC + np.arange(n0, n1)
            slotrow[nodes] = base_row + t * 128 + np.arange(n1 - n0)
    per_core = [_finish_core(pc, c, T, slotrow) for c, pc in enumerate(cores)]
    for c in range(NCORES):
        xc = np.zeros((IN, RPC), np.float16)
        xc[:, :NPC] = x[c * NPC:(c + 1) * NPC].T
        per_core[c]["xTc"] = xc

    common = dict(wd)
    common["slotrow"] = slotrow
    return T, common, per_core


# ---------------------------------------------------------------------------
# device program
# ---------------------------------------------------------------------------

def _gather_raw(eng, out_ap, in_ap, idxs_ap, num_idxs, elem_size, elem_step):
    """dma_gather with elem_size_bytes not a multiple of 256B (non-transpose
    path only; the 256B rule is a transpose-mode restriction — the Q7 kernel
    packets arbitrary elem sizes, only the row stride is encoded in 256B
    units).  Mirrors BassGpSimd.dma_gather's construction."""
    from concourse.ap_utils import ap_is_contiguous
    import concourse.mybir as mb
    assert idxs_ap.dtype == mybir.dt.int16
    assert in_ap.dtype == out_ap.dtype
    elem_size_bytes = elem_size * mybir.dt.size(in_ap.dtype)
    assert in_ap.ap[-1][1] == out_ap.ap[-1][1] == elem_size
    assert ap_is_contiguous(out_ap.ap[1:])
    assert ap_is_contiguous(idxs_ap.ap[1:])
    assert in_ap.ap[0][0] == elem_step
    stride_bytes = elem_step * mybir.dt.size(in_ap.dtype)
    assert stride_bytes % 256 == 0 and stride_bytes // 256 < 256
    _in_ap = eng.lower_ap_dma(in_ap, for_custom_bir_dma=True)
    _idxs_ap = eng.lower_ap(idxs_ap)
    _out_ap = eng.lower_ap(out_ap)
    return eng.add_instruction(
        mb.InstDMAGatherAnt(
            name=eng.bass.get_next_instruction_name(),
            ins=[*_in_ap, _idxs_ap,
                 eng.lower_val_access(eng.to_reg(num_idxs))],
            outs=[_out_ap],
            transpose=False,
            num_idxs=num_idxs,
            elem_size=elem_size,
            stride_bytes_256=stride_bytes // 256,
            gen_mode=0,
            single_packet=False,
            queue_num=0,
            sbuf_tokens_per_rank=0,
            sbuf_free_dim_per_rank=0,
            sbuf_free_dim_pad_per_rank=0,
            sbuf_byte_offset=0,
        )
    )


def build_nc(T, num_devices=NCORES, with_collective=True, phases="ABCD",
             dbg=False):
    nc = bacc.Bacc("TRN2", target_bir_lowering=False, debug=False,
                   num_devices=num_devices)
    dt = nc.dram_tensor
    xTc = dt("xTc", [IN, RPC], F16, kind="ExternalInput").ap()
    W1ext = dt("W1ext", [128, 272], F16, kind="ExternalInput").ap()
    W2ext = dt("W2ext", [256, 66], F16, kind="ExternalInput").ap()
    b1ext = dt("b1ext", [128, 272], F16, kind="ExternalInput").ap()
    b2e66 = dt("b2e66", [128, 66], F16, kind="ExternalInput").ap()
    g1idx = dt("g1idx", [128, T * CH * 8], I16, kind="ExternalInput").ap()
    g2idx = dt("g2idx", [128, T * CH * 8], I16, kind="ExternalInput").ap()
    selT1 = dt("selT1", [128, T * 2 * CH * 128], F8, kind="ExternalInput").ap()
    adidx = dt("adidx", [128, T * 8], I16, kind="ExternalInput").ap()
    t1slice = dt("t1slice", [RPC, T1_W], F16, kind="Internal").ap()
    T1 = dt("T1", [NR, T1_W], F16, kind="Internal",
            addr_space="Shared" if with_collective else "Local").ap()
    t2rows = T * 128
    # t2 rows are fp8: 64 fp8 h2 values + asrc2 as raw f16 in bytes 64:66,
    # padded to a 256B stride (gather stride must be a 256B multiple). The
    # 66B gather elem rides the 7ns/desc floor instead of f16's 11.6ns.
    t2slice = dt("t2slice", [t2rows, 256], F8, kind="Internal").ap()
    t2full = dt("t2full", [NCORES * t2rows, 256], F8, kind="Internal",
                addr_space="Shared" if with_collective else "Local").ap()
    outp = dt("out", [t2rows, 64], F32, kind="ExternalOutput").ap()
    if dbg:
        t1dbg = dt("t1dbg", [RPC, T1_W], F16, kind="ExternalOutput").ap()
        addbg = dt("addbg", [128, T * 8], F16, kind="ExternalOutput").ap()
        t2dbg = dt("t2dbg", [t2rows, 128], F16, kind="ExternalOutput").ap()
        gtdbg = dt("gtdbg", [128, CH * 264], F16, kind="ExternalOutput").ap()
        etdbg = dt("etdbg", [128, CH * 8], F16, kind="ExternalOutput").ap()
        wdbg = dt("wdbg", [128, CH * 256], F16, kind="ExternalOutput").ap()
        lkdbg = dt("lkdbg", [128, CH * 8], F16, kind="ExternalOutput").ap()
        exxdbg = dt("exxdbg", [128, CH * 256], F16,
                    kind="ExternalOutput").ap()
        h1dbg = dt("h1dbg", [128, 256 + 8], F32, kind="ExternalOutput").ap()
        nc._dbg = dict(gtdbg=gtdbg, etdbg=etdbg, wdbg=wdbg, h1dbg=h1dbg,
                       lkdbg=lkdbg, exxdbg=exxdbg)
    else:
        nc._dbg = None

    with tile.TileContext(nc) as tc:
        with tc.tile_pool(name="consts", bufs=1) as cp:
            W1e_sb = cp.tile([128, 272], F16)
            nc.sync.dma_start(out=W1e_sb[:], in_=W1ext[:])
            W2a_sb = cp.tile([128, 66], F16)
            nc.sync.dma_start(out=W2a_sb[:], in_=W2ext[0:128, :])
            W2b_sb = cp.tile([128, 66], F16)
            nc.sync.dma_start(out=W2b_sb[:], in_=W2ext[128:256, :])
            b1_sb = cp.tile([128, 272], F16)
            nc.sync.dma_start(out=b1_sb[:], in_=b1ext[:])
            b2_sb = cp.tile([128, 66], F16)
            nc.sync.dma_start(out=b2_sb[:], in_=b2e66[:])
            oneall = cp.tile([128, 128], F16)
            nc.vector.memset(oneall[:], 1.0 / 128.0)
            idn = cp.tile([128, 128], F16)
            make_identity(nc, idn[:])
            g1i_sb = cp.tile([128, T * CH * 8], I16)
            nc.sync.dma_start(out=g1i_sb[:], in_=g1idx[:])
            g2i_sb = cp.tile([128, T * CH * 8], I16)
            nc.sync.dma_start(out=g2i_sb[:], in_=g2idx[:])
            adidx_sb = cp.tile([128, T * 8], I16)
            nc.sync.dma_start(out=adidx_sb[:], in_=adidx[:])
            adtall_sb = cp.tile([128, T, 8], F16)  # bulk a_dst gather target
            adst2_sb = cp.tile([128, T], F16)  # written in B-fin, read in D

            # ---------------- Phase A: own T1 slice ----------------
            if "A" in phases:
                with tc.tile_pool(name="pa", bufs=2) as pa, \
                     tc.tile_pool(name="paps", bufs=4, space="PSUM") as paps:
                    XB = 2048
                    nblk = (RPC + XB - 1) // XB
                    for blk in range(nblk):
                        n0 = blk * XB
                        bw = min(XB, RPC - n0)
                        nt = bw // 128
                        xb = pa.tile([128, XB], F16, tag="xb", name="xb")
                        nc.sync.dma_start(out=xb[:, 0:bw],
                                          in_=xTc[:, n0:n0 + bw])
                        t1b = pa.tile([128, 16, 272], F16, tag="t1b",
                                      name="t1b")
                        for i in range(nt):
                            ps = paps.tile([128, 272], F32, tag="aps",
                                           name="aps")
                            nc.tensor.matmul(ps[:],
                                             lhsT=xb[:, i * 128:(i + 1) * 128],
                                             rhs=W1e_sb[:], start=True,
                                             stop=False)
                            nc.tensor.matmul(ps[:], lhsT=oneall[:],
                                             rhs=b1_sb[:], start=False,
                                             stop=True)
                            if i % 2 == 0:
                                nc.vector.tensor_copy(t1b[:, i, :], ps[:])
                            else:
                                nc.scalar.copy(t1b[:, i, :], ps[:])
                        nc.sync.dma_start(
                            out=t1slice[n0:n0 + bw, 0:272].rearrange(
                                "(i p) c -> p i c", p=128),
                            in_=t1b[:, 0:nt, :])
                        if not with_collective and "B" in phases:
                            nc.sync.dma_start(
                                out=T1[n0:n0 + bw, 0:272],
                                in_=t1slice[n0:n0 + bw, 0:272])

            # ---------------- AllGather T1 ----------------
            if "B" in phases:
                if with_collective:
                    nc.gpsimd.collective_compute(
                        "AllGather", A.bypass,
                        replica_groups=[list(range(NCORES))],
                        ins=[t1slice[:]], outs=[T1[:]],
                    )
                # bulk a_dst gather: one call for all T tiles' 128 slots
                _gather_raw(nc.gpsimd, adtall_sb[:],
                            t1slice[0:RPC, 264:272], adidx_sb[:],
                            T * 128, 8, T1_W)

                # ---------------- Phase B: layer-1 aggregation ----------------
                _agg_layer(nc, tc, T, layer=1,
                           tbl_lo=T1[0:T1_LO, 0:264],
                           tbl_hi=T1[T1_LO:NR, 0:264],
                           gidx_sb=g1i_sb, selT_in=selT1,
                           idn=idn, oneall=oneall,
                           adtall_sb=adtall_sb,
                           W2a_sb=W2a_sb, W2b_sb=W2b_sb, b2_sb=b2_sb,
                           adst2_sb=adst2_sb,
                           t2slice=t2slice, outp=None)

            if "D" in phases:
                nc.sync.dma_start(out=g2i_sb[:], in_=g2idx[:])

            if dbg:
                nc.sync.dma_start(out=t1dbg[:], in_=t1slice[:])
                nc.sync.dma_start(
                    out=addbg[:],
                    in_=adtall_sb[:].rearrange("p t c -> p (t c)"))
                nc.sync.dma_start(out=t2dbg[:], in_=t2slice[:])

            # ---------------- AllGather T2 ----------------
            if "C" in phases:
                if with_collective:
                    nc.gpsimd.collective_compute(
                        "AllGather", A.bypass,
                        replica_groups=[list(range(NCORES))],
                        ins=[t2slice[:]], outs=[t2full[:]],
                    )
                else:
                    nc.sync.dma_start(out=t2full[0:t2rows, :], in_=t2slice[:])

            # ---------------- Phase D: layer-2 aggregation ----------------
            if "D" in phases:
                _agg_layer(nc, tc, T, layer=2,
                           tbl_lo=t2full[0:4 * t2rows, 0:68],
                           tbl_hi=t2full[4 * t2rows:8 * t2rows, 0:68],
                           gidx_sb=g2i_sb, selT_in=selT1,
                           idn=idn, oneall=None,
                           adtall_sb=None,
                           W2a_sb=None, W2b_sb=None, b2_sb=None,
                           adst2_sb=adst2_sb,
                           t2slice=None, outp=outp)

    nc.compile()
    return nc


def _agg_layer(nc, tc, T, layer, tbl_lo, tbl_hi, gidx_sb, selT_in,
               idn, oneall, adtall_sb, W2a_sb, W2b_sb, b2_sb,
               adst2_sb, t2slice, outp):
    L1 = layer == 1
    GW = 264 if L1 else 68   # gathered elements per row (payload)
    GS = T1_W if L1 else 256  # table row stride in elements
    NH = 8 if L1 else 1
    VC = 256 if L1 else 64
    ACC_W = 264 if L1 else 65
    name = f"l{layer}"
    PBB = int(os.environ.get("V2_PBB", "3"))
    ACCB = int(os.environ.get("V2_ACCB", "3"))
    with tc.tile_pool(name=f"pb_{name}", bufs=PBB) as pb, \
         tc.tile_pool(name=f"pf_{name}", bufs=2) as pf, \
         tc.tile_pool(name=f"ps_acc_{name}", bufs=ACCB, space="PSUM") as ps_acc, \
         tc.tile_pool(name=f"ps_ad_{name}", bufs=2, space="PSUM") as ps_ad, \
         tc.tile_pool(name=f"ps_fin_{name}", bufs=2, space="PSUM") as ps_fin:
        for pr in range(T // 2):
            accs = [ps_acc.tile([128, ACC_W], F32, tag="acc", name="acc_a"),
                    ps_acc.tile([128, ACC_W], F32, tag="acc", name="acc_b")]
            # both layers share the same tiles and edge-slot layout, so the
            # interleaved selT|sel fp8 stream is shipped once per layer from
            # the SAME host tensor (sel2 == sel1, selT2 == selT1).
            scp = pb.tile([128, 2, 2, CH, 128], F8, tag="scp",
                          name="scp", bufs=2)
            nc.sync.dma_start(
                out=scp[:].rearrange("p f s j e -> p (f s j e)"),
                in_=selT_in[:, (2 * pr) * 2 * CALLW:
                            (2 * pr + 2) * 2 * CALLW])
            for hf in (0, 1):
                g = 2 * pr + hf
                s8 = scp[:, hf, 0]     # selT: [d-part, j, e]
                sel8 = scp[:, hf, 1]   # sel:  [e-part, j, d]
                gt = pb.tile([128, CH, GW], F16 if L1 else F8, tag="gt",
                             name="gt", bufs=3)
                if os.environ.get("V2_GSPLIT", "0") == "1":
                    for gh in (0, 1):
                        _gather_raw(
                            nc.gpsimd, gt[:, gh * CHL:(gh + 1) * CHL],
                            tbl_lo if hf == 0 else tbl_hi,
                            gidx_sb[:, g * CH * 8 + gh * CHL * 8:
                                    g * CH * 8 + (gh + 1) * CHL * 8],
                            CHL * 128, GW, GS)
                else:
                    _gather_raw(
                        nc.gpsimd, gt[:], tbl_lo if hf == 0 else tbl_hi,
                        gidx_sb[:, g * CH * 8:(g + 1) * CH * 8],
                        CALLW, GW, GS)
                # per-edge et = a_dst + a_src entirely in PSUM: the fp8 selT
                # one-hot matmul broadcasts a_dst, then an identity matmul
                # accumulates the gathered a_src columns on top.
                adps = ps_ad.tile([128, CH, NH], F32, tag="adps", name="adps")
                asrc_ap = (gt[:, :, 256:264] if L1
                           else gt[:, :, 64:66].bitcast(F16))
                for j in range(CH):
                    t = 2 * pr + (0 if j < CHL else 1)
                    rhs = adtall_sb[:, t, :] if L1 else adst2_sb[:, t:t + 1]
                    nc.tensor.matmul(adps[:, j, :], lhsT=s8[:, j, :], rhs=rhs,
                                     start=True, stop=False)
                    nc.tensor.matmul(adps[:, j, :], lhsT=idn[:],
                                     rhs=asrc_ap[:, j, :], start=False,
                                     stop=True)
                lk = pb.tile([128, CH, NH], F16, tag="lk", name="lk")
                nc.scalar.activation(lk[:], adps[:], AF.Prelu, alpha=NEG)
                # exp at pair width on ACT (cheap), broadcast to the value
                # width inside the DVE multiply via a stride-0 middle dim —
                # the last dim stays packed so the mult keeps 2x DVE mode.
                exf = pb.tile([128, CH, NH, 2], F16, tag="exf", name="exf")
                nc.scalar.activation(
                    exf[:], lk[:, :, :, None].to_broadcast([128, CH, NH, 2]),
                    AF.Exp)
                w = pb.tile([128, CH, ACC_W], F16, tag="w", name="w")
                nc.vector.tensor_copy(w[:, :, VC:ACC_W], exf[:, :, :, 0])
                cph = VC // NH // 2  # 16 (L1) / 32 (L2) value pairs per head
                nc.vector.tensor_tensor(
                    out=w[:, :, 0:VC].rearrange("p j (h k two) -> p j h k two",
                                                h=NH, two=2),
                    in0=gt[:, :, 0:VC].rearrange("p j (h k two) -> p j h k two",
                                                 h=NH, two=2),
                    in1=exf[:, :, :, None, :].to_broadcast(
                        [128, CH, NH, cph, 2]),
                    op=A.mult)
                for j in range(CH):
                    acc = accs[0 if j < CHL else 1]
                    st = (hf == 0) and (j % CHL == 0)
                    sp = (hf == 1) and (j % CHL == CHL - 1)
                    nc.tensor.matmul(acc[:], lhsT=sel8[:, j, :],
                                     rhs=w[:, j, :], start=st, stop=sp)
            for k in (0, 1):
                t = 2 * pr + k
                if L1:
                    _fin_l1(nc, t, accs[k], pf, ps_fin, idn, oneall, W2a_sb,
                            W2b_sb, b2_sb, adst2_sb, t2slice)
                else:
                    _fin_l2(nc, t, accs[k], pf, outp)


def _fin_l1(nc, t, acc, pf, ps_fin, idn, oneall, W2a_sb, W2b_sb, b2_sb,
            adst2_sb, t2slice):
    # EPS keeps padded dst rows (den=0) finite — their garbage h1 values are
    # never read, but adst2 must stay finite (0*inf = NaN leaks via selT).
    deps = pf.tile([128, 8], F32, tag="deps", name="deps")
    nc.vector.tensor_scalar_add(deps[:], acc[:, 256:264], EPS)
    rec = pf.tile([128, 8], F32, tag="rec", name="rec")
    nc.vector.reciprocal(rec[:], deps[:])
    h1b = pf.tile([128, 256], F16, tag="h1b", name="h1b")
    nc.vector.tensor_tensor(
        out=h1b[:].rearrange("p (h c) -> p h c", h=8),
        in0=acc[:, 0:256].rearrange("p (h c) -> p h c", h=8),
        in1=rec[:, :, None].to_broadcast([128, 8, 32]),
        op=A.mult)
    if t == 0 and getattr(nc, "_dbg", None):
        accs_sb = pf.tile([128, 264], F32, tag="accdbg", name="accdbg")
        nc.vector.tensor_copy(accs_sb[:, 0:256], acc[:, 0:256])
        nc.vector.tensor_copy(accs_sb[:, 256:264], acc[:, 256:264])
        nc.sync.dma_start(out=nc._dbg["h1dbg"][:], in_=accs_sb[:])
    # ho = elu(h1b) + 1 = relu(h1b) + exp(-relu(-h1b)); the -1 is folded
    # into b2eff via W2ext (v1's ACT-based elu decomposition)
    r1 = pf.tile([128, 256], F16, tag="r1", name="r1")
    nc.scalar.activation(r1[:], h1b[:], AF.Relu, scale=-1.0)
    e1 = pf.tile([128, 256], F16, tag="e1", name="e1")
    nc.scalar.activation(e1[:], r1[:], AF.Exp, scale=-1.0)
    rl = pf.tile([128, 256], F16, tag="rl", name="rl")
    nc.scalar.activation(rl[:], h1b[:], AF.Relu)
    ho = pf.tile([128, 256], F16, tag="ho", name="ho")
    nc.gpsimd.tensor_tensor(out=ho[:], in0=rl[:], in1=e1[:], op=A.add)
    h2ps = ps_fin.tile([128, 66], F32, tag="h2ps", name="h2ps")
    for half in (0, 1):
        tp = ps_fin.tile([128, 128], F16, tag="tp", name="tp", bufs=1)
        nc.tensor.transpose(out=tp[:], in_=ho[:, half * 128:(half + 1) * 128],
                            identity=idn[:])
        hoT = pf.tile([128, 128], F16, tag="hoT", name="hoT")
        if half == 0:
            nc.vector.tensor_copy(hoT[:], tp[:])
        else:
            nc.scalar.copy(hoT[:], tp[:])
        nc.tensor.matmul(h2ps[:], lhsT=hoT[:],
                         rhs=(W2a_sb if half == 0 else W2b_sb)[:],
                         start=half == 0, stop=False)
    nc.tensor.matmul(h2ps[:], lhsT=oneall[:], rhs=b2_sb[:], start=False,
                     stop=True)
    t2r = pf.tile([128, 66], F8, tag="t2r", name="t2r")
    nc.scalar.copy(t2r[:, 0:64], h2ps[:, 0:64])
    nc.scalar.copy(t2r[:, 64:66].bitcast(F16), h2ps[:, 64:65])
    nc.vector.tensor_copy(adst2_sb[:, t:t + 1], h2ps[:, 65:66])
    nc.sync.dma_start(out=t2slice[t * 128:(t + 1) * 128, 0:66], in_=t2r[:])


def _fin_l2(nc, t, acc, pf, outp):
    rec = pf.tile([128, 1], F32, tag="rec2", name="rec2")
    nc.vector.reciprocal(rec[:], acc[:, 64:65])
    ot = pf.tile([128, 64], F32, tag="ot", name="ot")
    nc.vector.tensor_scalar_mul(ot[:], acc[:, 0:64], rec[:, 0:1])
    nc.sync.dma_start(out=outp[t * 128:(t + 1) * 128, :], in_=ot[:])


# ---------------------------------------------------------------------------
# entry point
# ---------------------------------------------------------------------------

def make_in_maps(T, common, per_core):
    in_maps = []
    for c in range(NCORES):
        m = {
            "W1ext": common["W1ext"], "W2ext": common["W2ext"],
            "b1ext": common["b1ext"], "b2e66": common["b2e66"],
        }
        pc = per_core[c]
        m.update({k: pc[k] for k in ("xTc", "g1idx", "g2idx", "selT1",
                                     "adidx")})
        in_maps.append(m)
    return in_maps


def kernel(**inputs):
    T, common, per_core = host_prep(inputs)
    nc = build_nc(T)
    in_maps = make_in_maps(T, common, per_core)
    res = run_bass_kernel_spmd(nc, in_maps, core_ids=list(range(NCORES)))
    allrows = np.concatenate([res.results[c]["out"] for c in range(NCORES)],
                             axis=0)
    return allrows[common["slotrow"]].astype(np.float32)



# revision 51
# speedup vs baseline: 1.2217x; 1.1381x over previous
"""Two-layer GAT on 8 Trainium2 NeuronCores (Bass/Tile) — v2.

Changes vs v1 (889us):
  - Phase A sharded: each core computes T1 rows for its own 6250 nodes
    (padded to 6272), AllGather broadcasts the table (timing build: local
    copy of own contribution only).
  - b1 folded into T1 value columns (sum(alpha)=1), b2 and the elu "-1"
    folded into W2ext/b2eff at finalize.
  - selT (dst-onehot, d-on-partitions) shipped from host as fp8 and used
    directly as matmul lhsT (mixed fp8 x f16 matmul) — kills the dlT DMA,
    the DVE selT build, and the adf path stays cheap.
  - exp broadcast stays on ACT; leaky/et on DVE; PSUM->SBUF copies and
    t2r/adst2 finalization moved to Pool (gpsimd).
  - softmax denominator accumulated via a second per-chunk matmul with a
    strided rhs view of exx (no den-copy op).
  - asrc2/adst2 computed via the h2 matmul itself (W2ext has v_s|v_d cols).
"""

import os

import numpy as np

import concourse.bass as bass
import concourse.bacc as bacc
import concourse.tile as tile
import concourse.mybir as mybir
from concourse.bass_utils import run_bass_kernel_spmd
from concourse.masks import make_identity

F32 = mybir.dt.float32
F16 = mybir.dt.float16
F8 = mybir.dt.float8e4
I16 = mybir.dt.int16
I32 = mybir.dt.int32
A = mybir.AluOpType
AF = mybir.ActivationFunctionType
NP_F8 = mybir.dt.np(F8)

# -------- problem constants --------
N, E, IN, HID, OUT, H = 50000, 800000, 128, 32, 64, 8
C1 = H * HID  # 256
NCORES = 8
NPC = N // NCORES        # 6250 dst nodes per core
RPC = 6272               # T1 rows per core (6250 padded to 49*128)
NR = NCORES * RPC        # 50176 T1 rows
T1_LO = 4 * RPC          # 25088: rows of cores 0-3
T1_W = 384               # T1 row stride (768B) — gather granularity
CHL = 9                  # chunks per (tile, half)
CH = 2 * CHL             # chunks per gather call
CALLW = CH * 128         # 2304 edge slots per call
EPS = 1e-16
NEG = 0.2


def _row_of(n):
    """T1 row of node n (cores own contiguous 6250-node ranges, padded)."""
    c = n // NPC
    return c * RPC + (n - c * NPC)


# ---------------------------------------------------------------------------
# host-side preprocessing
# ---------------------------------------------------------------------------

def _prep_weights(W1, as1, ad1, b1, W2, as2, ad2, b2):
    As = np.zeros((C1, H), np.float32)
    Ad = np.zeros((C1, H), np.float32)
    for h in range(H):
        As[h * HID:(h + 1) * HID, h] = as1[h]
        Ad[h * HID:(h + 1) * HID, h] = ad1[h]
    W1ext = np.concatenate([W1, W1 @ As, W1 @ Ad], axis=1)  # [128, 272]
    iotarep = np.zeros((128, 128 * CH), np.float16)
    for d in range(128):
        iotarep[:, d * CH:(d + 1) * CH] = d
    b1ext = np.zeros((272,), np.float32)
    b1ext[:C1] = b1
    vs = W2 @ as2[0]   # [256]
    vd = W2 @ ad2[0]   # [256]
    W2ext = np.concatenate([W2, vs[:, None], vd[:, None]], axis=1)  # [256,66]
    b2eff = np.concatenate([b2 - W2.sum(0), [-vs.sum()], [-vd.sum()]])  # [66]
    return {
        "W1ext": W1ext.astype(np.float16),
        "W2ext": W2ext.astype(np.float16),
        "b1ext": np.tile(b1ext[None, :], (128, 1)).astype(np.float16),
        "b2e66": np.tile(b2eff[None, :], (128, 1)).astype(np.float16),
        "iotarep": iotarep,
    }


def _greedy_tiles(deg_lo1, deg_hi1, deg_lo2, deg_hi2):
    cap = CHL * 128
    tiles = []
    i, n = 0, len(deg_lo1)
    while i < n:
        l1 = h1 = l2 = h2 = 0
        j = i
        while j < n and j - i < 128:
            nl1, nh1 = l1 + deg_lo1[j], h1 + deg_hi1[j]
            nl2, nh2 = l2 + deg_lo2[j], h2 + deg_hi2[j]
            if nl1 > cap or nh1 > cap or nl2 > cap or nh2 > cap:
                break
            l1, h1, l2, h2 = nl1, nh1, nl2, nh2
            j += 1
        assert j > i, "single node exceeds chunk caps"
        tiles.append((i, j))
        i = j
    return tiles


def _pack_calls(rows_half, dloc_half, T, both=False):
    """Build gather idx [128, T*CH*8] i16, dloc [128, T*CH] f16, and the
    fp8 one-hot stream: selT only [128, T*CH*128], or selT|sel interleaved
    per call [128, T*2*CH*128] when both=True."""
    idx16 = np.zeros((128, T * CH * 8), np.int16)
    dloc = np.full((128, T * CH), -1, np.float16)
    dlocT_flat = np.full((T * CALLW,), -1, np.int32)
    for pr in range(T // 2):
        for h in (0, 1):
            g = 2 * pr + h
            rows = np.zeros((CALLW,), np.int64)
            dl = np.full((CALLW,), -1, np.int64)
            for k, t in enumerate((2 * pr, 2 * pr + 1)):
                r = rows_half.get((t, h))
                if r is None:
                    continue
                d = dloc_half[(t, h)]
                off = k * CHL * 128
                rows[off:off + len(r)] = r
                dl[off:off + len(r)] = d
            blk = rows.reshape(CH * 8, 16).T.astype(np.int16)
            idx16[:, g * CH * 8:(g + 1) * CH * 8] = np.tile(blk, (8, 1))
            dloc[:, g * CH:(g + 1) * CH] = dl.reshape(CH, 128).T.astype(np.float16)
            dlocT_flat[g * CALLW:(g + 1) * CALLW] = dl
    # selT[p, g*CH*128 + j*128 + e] = 1.0 iff dloc(edge (j,e) of g) == p
    selT = (dlocT_flat[None, :] == np.arange(128)[:, None])
    if not both:
        return idx16, dloc, selT.astype(NP_F8)
    # sel[p, g, j, d] = 1.0 iff dloc(edge (p,j) of g) == d
    sel = (dloc[:, :, None].astype(np.int32) ==
           np.arange(128)[None, None, :]).reshape(128, T, CH, 128)
    selB = np.stack([selT.reshape(128, T, CH, 128), sel], axis=2).reshape(
        128, T * 2 * CH * 128).astype(NP_F8)
    return idx16, dloc, selB


def _prep_core(c, src, dst):
    base = c * NPC
    own = (dst >= base) & (dst < base + NPC)
    s = src[own].astype(np.int64)
    d = (dst[own] - base).astype(np.int64)
    order = np.argsort(d, kind="stable")
    s, d = s[order], d[order]
    ptr = np.zeros(NPC + 1, np.int64)
    np.cumsum(np.bincount(d, minlength=NPC), out=ptr[1:])

    srow = _row_of(s)
    lo1m = srow < T1_LO
    lo2m = s < (N // 2)  # slot-major half split: cores 0-3 vs 4-7
    deg_lo1 = np.bincount(d, weights=lo1m, minlength=NPC).astype(np.int64)
    deg_hi1 = np.bincount(d, weights=~lo1m, minlength=NPC).astype(np.int64)
    deg_lo2 = np.bincount(d, weights=lo2m, minlength=NPC).astype(np.int64)
    deg_hi2 = np.bincount(d, weights=~lo2m, minlength=NPC).astype(np.int64)
    tiles = _greedy_tiles(deg_lo1, deg_hi1, deg_lo2, deg_hi2)

    rows1, dloc1 = {}, {}
    for t, (n0, n1) in enumerate(tiles):
        e0, e1 = ptr[n0], ptr[n1]
        es, ed = srow[e0:e1], d[e0:e1]
        dl = ed - n0
        m1 = es < T1_LO
        rows1[(t, 0)] = es[m1]
        dloc1[(t, 0)] = dl[m1]
        rows1[(t, 1)] = es[~m1] - T1_LO
        dloc1[(t, 1)] = dl[~m1]
    return {
        "Treal": len(tiles), "tiles": tiles, "s": s, "d": d, "ptr": ptr,
        "rows1": rows1, "dloc1": dloc1,
    }


def _finish_core(pc, c, T, slotrow):
    tiles = list(pc["tiles"]) + [(0, 0)] * (T - pc["Treal"])
    idx1, dloc1, selT1 = _pack_calls(pc["rows1"], pc["dloc1"], T, both=True)

    t2lo = 4 * T * 128
    s, d, ptr = pc["s"], pc["d"], pc["ptr"]
    srow = slotrow[s]
    rows2, dloc2 = {}, {}
    for t, (n0, n1) in enumerate(pc["tiles"]):
        e0, e1 = ptr[n0], ptr[n1]
        dl = d[e0:e1] - n0
        m2 = srow[e0:e1] < t2lo
        rows2[(t, 0)] = srow[e0:e1][m2]
        dloc2[(t, 0)] = dl[m2]
        rows2[(t, 1)] = srow[e0:e1][~m2] - t2lo
        dloc2[(t, 1)] = dl[~m2]
    # idx2 only: the layer-2 slot layout (dloc/sel/selT) is identical to
    # layer-1's by construction — both split halves by core(src) < 4 and
    # keep the same dst-sorted order — so selT1 is reused in phase D.
    idx2, dloc2a, _ = _pack_calls(rows2, dloc2, T)
    assert all(np.array_equal(dloc2[k], pc["dloc1"][k]) for k in dloc2)

    # adidx: local t1slice rows of each tile's nodes (slot-major, clamped),
    # in dma_gather int16 index layout, one call of T*128 rows
    p = np.arange(128)
    rows = np.zeros((T * 128,), np.int64)
    for t, (n0, n1) in enumerate(tiles):
        w = n1 - n0
        rows[t * 128:(t + 1) * 128] = n0 + np.minimum(p, max(w - 1, 0))
    blk = rows.reshape(T * 8, 16).T.astype(np.int16)
    adidx = np.tile(blk, (8, 1))  # [128, T*8]
    return {
        "g1idx": idx1, "dloc1": dloc1, "selT1": selT1,
        "g2idx": idx2,
        "adidx": adidx,
    }


def host_prep(inputs):
    ei = np.asarray(inputs["edge_index"]).astype(np.int64)
    wd = _prep_weights(
        np.asarray(inputs["W1"], np.float32),
        np.asarray(inputs["att_src1"], np.float32),
        np.asarray(inputs["att_dst1"], np.float32),
        np.asarray(inputs["b1"], np.float32),
        np.asarray(inputs["W2"], np.float32),
        np.asarray(inputs["att_src2"], np.float32),
        np.asarray(inputs["att_dst2"], np.float32),
        np.asarray(inputs["b2"], np.float32),
    )
    loops = np.arange(N, dtype=np.int64)
    src = np.concatenate([ei[0], loops])
    dst = np.concatenate([ei[1], loops])

    x = np.asarray(inputs["x"], np.float32).astype(np.float16)

    cores = [_prep_core(c, src, dst) for c in range(NCORES)]
    T = max(pc["Treal"] for pc in cores)
    if T % 2:
        T += 1
    slotrow = np.zeros(N, np.int64)
    for c, pc in enumerate(cores):
        base_row = c * T * 128
        for t, (n0, n1) in enumerate(pc["tiles"]):
            nodes = c * NPC + np.arange(n0, n1)
            slotrow[nodes] = base_row + t * 128 + np.arange(n1 - n0)
    per_core = [_finish_core(pc, c, T, slotrow) for c, pc in enumerate(cores)]
    for c in range(NCORES):
        xc = np.zeros((IN, RPC), np.float16)
        xc[:, :NPC] = x[c * NPC:(c + 1) * NPC].T
        per_core[c]["xTc"] = xc

    common = dict(wd)
    common["slotrow"] = slotrow
    return T, common, per_core


# ---------------------------------------------------------------------------
# device program
# ---------------------------------------------------------------------------

def _gather_raw(eng, out_ap, in_ap, idxs_ap, num_idxs, elem_size, elem_step):
    """dma_gather with elem_size_bytes not a multiple of 256B (non-transpose
    path only; the 256B rule is a transpose-mode restriction — the Q7 kernel
    packets arbitrary elem sizes, only the row stride is encoded in 256B
    units).  Mirrors BassGpSimd.dma_gather's construction."""
    from concourse.ap_utils import ap_is_contiguous
    import concourse.mybir as mb
    assert idxs_ap.dtype == mybir.dt.int16
    assert in_ap.dtype == out_ap.dtype
    elem_size_bytes = elem_size * mybir.dt.size(in_ap.dtype)
    assert in_ap.ap[-1][1] == out_ap.ap[-1][1] == elem_size
    assert ap_is_contiguous(out_ap.ap[1:])
    assert ap_is_contiguous(idxs_ap.ap[1:])
    assert in_ap.ap[0][0] == elem_step
    stride_bytes = elem_step * mybir.dt.size(in_ap.dtype)
    assert stride_bytes % 256 == 0 and stride_bytes // 256 < 256
    _in_ap = eng.lower_ap_dma(in_ap, for_custom_bir_dma=True)
    _idxs_ap = eng.lower_ap(idxs_ap)
    _out_ap = eng.lower_ap(out_ap)
    return eng.add_instruction(
        mb.InstDMAGatherAnt(
            name=eng.bass.get_next_instruction_name(),
            ins=[*_in_ap, _idxs_ap,
                 eng.lower_val_access(eng.to_reg(num_idxs))],
            outs=[_out_ap],
            transpose=False,
            num_idxs=num_idxs,
            elem_size=elem_size,
            stride_bytes_256=stride_bytes // 256,
            gen_mode=0,
            single_packet=False,
            queue_num=0,
            sbuf_tokens_per_rank=0,
            sbuf_free_dim_per_rank=0,
            sbuf_free_dim_pad_per_rank=0,
            sbuf_byte_offset=0,
        )
    )


def build_nc(T, num_devices=NCORES, with_collective=True, phases="ABCD",
             dbg=False):
    nc = bacc.Bacc("TRN2", target_bir_lowering=False, debug=False,
                   num_devices=num_devices)
    dt = nc.dram_tensor
    xTc = dt("xTc", [IN, RPC], F16, kind="ExternalInput").ap()
    W1ext = dt("W1ext", [128, 272], F16, kind="ExternalInput").ap()
    W2ext = dt("W2ext", [256, 66], F16, kind="ExternalInput").ap()
    b1ext = dt("b1ext", [128, 272], F16, kind="ExternalInput").ap()
    b2e66 = dt("b2e66", [128, 66], F16, kind="ExternalInput").ap()
    g1idx = dt("g1idx", [128, T * CH * 8], I16, kind="ExternalInput").ap()
    g2idx = dt("g2idx", [128, T * CH * 8], I16, kind="ExternalInput").ap()
    selT1 = dt("selT1", [128, T * 2 * CH * 128], F8, kind="ExternalInput").ap()
    adidx = dt("adidx", [128, T * 8], I16, kind="ExternalInput").ap()
    t1slice = dt("t1slice", [RPC, T1_W], F16, kind="Internal").ap()
    T1 = dt("T1", [NR, T1_W], F16, kind="Internal",
            addr_space="Shared" if with_collective else "Local").ap()
    t2rows = T * 128
    # t2 rows are fp8: 64 fp8 h2 values + asrc2 as raw f16 in bytes 64:66,
    # padded to a 256B stride (gather stride must be a 256B multiple). The
    # 66B gather elem rides the 7ns/desc floor instead of f16's 11.6ns.
    t2slice = dt("t2slice", [t2rows, 256], F8, kind="Internal").ap()
    t2full = dt("t2full", [NCORES * t2rows, 256], F8, kind="Internal",
                addr_space="Shared" if with_collective else "Local").ap()
    outp = dt("out", [t2rows, 64], F32, kind="ExternalOutput").ap()
    if dbg:
        t1dbg = dt("t1dbg", [RPC, T1_W], F16, kind="ExternalOutput").ap()
        addbg = dt("addbg", [128, T * 8], F16, kind="ExternalOutput").ap()
        t2dbg = dt("t2dbg", [t2rows, 128], F16, kind="ExternalOutput").ap()
        gtdbg = dt("gtdbg", [128, CH * 264], F16, kind="ExternalOutput").ap()
        etdbg = dt("etdbg", [128, CH * 8], F16, kind="ExternalOutput").ap()
        wdbg = dt("wdbg", [128, CH * 256], F16, kind="ExternalOutput").ap()
        lkdbg = dt("lkdbg", [128, CH * 8], F16, kind="ExternalOutput").ap()
        exxdbg = dt("exxdbg", [128, CH * 256], F16,
                    kind="ExternalOutput").ap()
        h1dbg = dt("h1dbg", [128, 256 + 8], F32, kind="ExternalOutput").ap()
        nc._dbg = dict(gtdbg=gtdbg, etdbg=etdbg, wdbg=wdbg, h1dbg=h1dbg,
                       lkdbg=lkdbg, exxdbg=exxdbg)
    else:
        nc._dbg = None

    with tile.TileContext(nc) as tc:
        with tc.tile_pool(name="consts", bufs=1) as cp:
            W1e_sb = cp.tile([128, 272], F16)
            nc.sync.dma_start(out=W1e_sb[:], in_=W1ext[:])
            W2a_sb = cp.tile([128, 66], F16)
            nc.sync.dma_start(out=W2a_sb[:], in_=W2ext[0:128, :])
            W2b_sb = cp.tile([128, 66], F16)
            nc.sync.dma_start(out=W2b_sb[:], in_=W2ext[128:256, :])
            b1_sb = cp.tile([128, 272], F16)
            nc.sync.dma_start(out=b1_sb[:], in_=b1ext[:])
            b2_sb = cp.tile([128, 66], F16)
            nc.sync.dma_start(out=b2_sb[:], in_=b2e66[:])
            oneall = cp.tile([128, 128], F16)
            nc.vector.memset(oneall[:], 1.0 / 128.0)
            idn = cp.tile([128, 128], F16)
            make_identity(nc, idn[:])
            g1i_sb = cp.tile([128, T * CH * 8], I16)
            nc.sync.dma_start(out=g1i_sb[:], in_=g1idx[:])
            g2i_sb = cp.tile([128, T * CH * 8], I16)
            nc.sync.dma_start(out=g2i_sb[:], in_=g2idx[:])
            adidx_sb = cp.tile([128, T * 8], I16)
            nc.sync.dma_start(out=adidx_sb[:], in_=adidx[:])
            adtall_sb = cp.tile([128, T, 8], F16)  # bulk a_dst gather target
            adst2_sb = cp.tile([128, T], F16)  # written in B-fin, read in D

            # ---------------- Phase A: own T1 slice ----------------
            if "A" in phases:
                with tc.tile_pool(name="pa", bufs=2) as pa, \
                     tc.tile_pool(name="paps", bufs=4, space="PSUM") as paps:
                    XB = 2048
                    nblk = (RPC + XB - 1) // XB
                    for blk in range(nblk):
                        n0 = blk * XB
                        bw = min(XB, RPC - n0)
                        nt = bw // 128
                        xb = pa.tile([128, XB], F16, tag="xb", name="xb")
                        nc.sync.dma_start(out=xb[:, 0:bw],
                                          in_=xTc[:, n0:n0 + bw])
                        t1b = pa.tile([128, 16, 272], F16, tag="t1b",
                                      name="t1b")
                        for i in range(nt):
                            ps = paps.tile([128, 272], F32, tag="aps",
                                           name="aps")
                            nc.tensor.matmul(ps[:],
                                             lhsT=xb[:, i * 128:(i + 1) * 128],
                                             rhs=W1e_sb[:], start=True,
                                             stop=False)
                            nc.tensor.matmul(ps[:], lhsT=oneall[:],
                                             rhs=b1_sb[:], start=False,
                                             stop=True)
                            if i % 2 == 0:
                                nc.vector.tensor_copy(t1b[:, i, :], ps[:])
                            else:
                                nc.scalar.copy(t1b[:, i, :], ps[:])
                        nc.sync.dma_start(
                            out=t1slice[n0:n0 + bw, 0:272].rearrange(
                                "(i p) c -> p i c", p=128),
                            in_=t1b[:, 0:nt, :])
                        if not with_collective and "B" in phases:
                            nc.sync.dma_start(
                                out=T1[n0:n0 + bw, 0:272],
                                in_=t1slice[n0:n0 + bw, 0:272])

            # ---------------- AllGather T1 ----------------
            if "B" in phases:
                if with_collective:
                    nc.gpsimd.collective_compute(
                        "AllGather", A.bypass,
                        replica_groups=[list(range(NCORES))],
                        ins=[t1slice[:]], outs=[T1[:]],
                    )
                # bulk a_dst gather: one call for all T tiles' 128 slots
                _gather_raw(nc.gpsimd, adtall_sb[:],
                            t1slice[0:RPC, 264:272], adidx_sb[:],
                            T * 128, 8, T1_W)

                # ---------------- Phase B: layer-1 aggregation ----------------
                _agg_layer(nc, tc, T, layer=1,
                           tbl_lo=T1[0:T1_LO, 0:264],
                           tbl_hi=T1[T1_LO:NR, 0:264],
                           gidx_sb=g1i_sb, selT_in=selT1,
                           idn=idn, oneall=oneall,
                           adtall_sb=adtall_sb,
                           W2a_sb=W2a_sb, W2b_sb=W2b_sb, b2_sb=b2_sb,
                           adst2_sb=adst2_sb,
                           t2slice=t2slice, outp=None)

            if "D" in phases:
                nc.sync.dma_start(out=g2i_sb[:], in_=g2idx[:])

            if dbg:
                nc.sync.dma_start(out=t1dbg[:], in_=t1slice[:])
                nc.sync.dma_start(
                    out=addbg[:],
                    in_=adtall_sb[:].rearrange("p t c -> p (t c)"))
                nc.sync.dma_start(out=t2dbg[:], in_=t2slice[:])

            # ---------------- AllGather T2 ----------------
            if "C" in phases:
                if with_collective:
                    nc.gpsimd.collective_compute(
                        "AllGather", A.bypass,
                        replica_groups=[list(range(NCORES))],
                        ins=[t2slice[:]], outs=[t2full[:]],
                    )
                else:
                    nc.sync.dma_start(out=t2full[0:t2rows, :], in_=t2slice[:])

            # ---------------- Phase D: layer-2 aggregation ----------------
            if "D" in phases:
                _agg_layer(nc, tc, T, layer=2,
                           tbl_lo=t2full[0:4 * t2rows, 0:68],
                           tbl_hi=t2full[4 * t2rows:8 * t2rows, 0:68],
                           gidx_sb=g2i_sb, selT_in=selT1,
                           idn=idn, oneall=None,
                           adtall_sb=None,
                           W2a_sb=None, W2b_sb=None, b2_sb=None,
                           adst2_sb=adst2_sb,
                           t2slice=None, outp=outp)

    nc.compile()
    return nc


def _agg_layer(nc, tc, T, layer, tbl_lo, tbl_hi, gidx_sb, selT_in,
               idn, oneall, adtall_sb, W2a_sb, W2b_sb,
               b2_sb, adst2_sb, t2slice, outp):
    L1 = layer == 1
    GW = 264 if L1 else 68   # gathered elements per row (payload)
    GS = T1_W if L1 else 256  # table row stride in elements
    NH = 8 if L1 else 1
    VC = 256 if L1 else 64
    ACC_W = 264 if L1 else 65
    name = f"l{layer}"
    PBB = int(os.environ.get("V2_PBB", "4"))
    ACCB = int(os.environ.get("V2_ACCB", "3"))
    with tc.tile_pool(name=f"pb_{name}", bufs=PBB) as pb, \
         tc.tile_pool(name=f"pf_{name}", bufs=2) as pf, \
         tc.tile_pool(name=f"ps_acc_{name}", bufs=ACCB, space="PSUM") as ps_acc, \
         tc.tile_pool(name=f"ps_ad_{name}", bufs=2, space="PSUM") as ps_ad, \
         tc.tile_pool(name=f"ps_fin_{name}", bufs=2, space="PSUM") as ps_fin:
        for pr in range(T // 2):
            accs = [ps_acc.tile([128, ACC_W], F32, tag="acc", name="acc_a"),
                    ps_acc.tile([128, ACC_W], F32, tag="acc", name="acc_b")]
            for hf in (0, 1):
                g = 2 * pr + hf
                # both layers share the same tiles and edge-slot layout, so
                # the selT|sel fp8 stream is shipped from the SAME host
                # tensor (sel2 == sel1, selT2 == selT1). Layer 1 ships both
                # halves; layer 2 ships only selT and rebuilds sel on DVE
                # (is_equal against an iota grid) to take load off the DMA
                # engines, which are the wall.
                scp = pb.tile([128, 2, CH, 128], F8, tag="scp",
                              name="scp", bufs=6)
                nc.sync.dma_start(
                    out=scp[:].rearrange("p s j e -> p (s j e)"),
                    in_=selT_in[:, g * 2 * CALLW:(g + 1) * 2 * CALLW])
                s8 = scp[:, 0]     # selT: [d-part, j, e]
                sel8 = scp[:, 1]   # sel:  [e-part, j, d]
                gt = pb.tile([128, CH, GW], F16 if L1 else F8, tag="gt",
                             name="gt", bufs=5)
                if os.environ.get("V2_GSPLIT", "0") == "1":
                    for gh in (0, 1):
                        _gather_raw(
                            nc.gpsimd, gt[:, gh * CHL:(gh + 1) * CHL],
                            tbl_lo if hf == 0 else tbl_hi,
                            gidx_sb[:, g * CH * 8 + gh * CHL * 8:
                                    g * CH * 8 + (gh + 1) * CHL * 8],
                            CHL * 128, GW, GS)
                else:
                    _gather_raw(
                        nc.gpsimd, gt[:], tbl_lo if hf == 0 else tbl_hi,
                        gidx_sb[:, g * CH * 8:(g + 1) * CH * 8],
                        CALLW, GW, GS)
                # per-edge et = a_dst + a_src entirely in PSUM: the fp8 selT
                # one-hot matmul broadcasts a_dst, then an identity matmul
                # accumulates the gathered a_src columns on top.
                adps = ps_ad.tile([128, CH, NH], F32, tag="adps", name="adps")
                asrc_ap = (gt[:, :, 256:264] if L1
                           else gt[:, :, 64:66].bitcast(F16))
                for j in range(CH):
                    t = 2 * pr + (0 if j < CHL else 1)
                    rhs = adtall_sb[:, t, :] if L1 else adst2_sb[:, t:t + 1]
                    nc.tensor.matmul(adps[:, j, :], lhsT=s8[:, j, :], rhs=rhs,
                                     start=True, stop=False)
                    nc.tensor.matmul(adps[:, j, :], lhsT=idn[:],
                                     rhs=asrc_ap[:, j, :], start=False,
                                     stop=True)
                lk = pb.tile([128, CH, NH], F16, tag="lk", name="lk")
                nc.scalar.activation(lk[:], adps[:], AF.Prelu, alpha=NEG)
                # exp at pair width on ACT (cheap), broadcast to the value
                # width inside the DVE multiply via a stride-0 middle dim —
                # the last dim stays packed so the mult keeps 2x DVE mode.
                exf = pb.tile([128, CH, NH, 2], F16, tag="exf", name="exf")
                nc.scalar.activation(
                    exf[:], lk[:, :, :, None].to_broadcast([128, CH, NH, 2]),
                    AF.Exp)
                w = pb.tile([128, CH, ACC_W], F16, tag="w", name="w")
                nc.vector.tensor_copy(w[:, :, VC:ACC_W], exf[:, :, :, 0])
                cph = VC // NH // 2  # 16 (L1) / 32 (L2) value pairs per head
                nc.vector.tensor_tensor(
                    out=w[:, :, 0:VC].rearrange("p j (h k two) -> p j h k two",
                                                h=NH, two=2),
                    in0=gt[:, :, 0:VC].rearrange("p j (h k two) -> p j h k two",
                                                 h=NH, two=2),
                    in1=exf[:, :, :, None, :].to_broadcast(
                        [128, CH, NH, cph, 2]),
                    op=A.mult)
                for j in range(CH):
                    acc = accs[0 if j < CHL else 1]
                    st = (hf == 0) and (j % CHL == 0)
                    sp = (hf == 1) and (j % CHL == CHL - 1)
                    nc.tensor.matmul(acc[:], lhsT=sel8[:, j, :],
                                     rhs=w[:, j, :], start=st, stop=sp)
            for k in (0, 1):
                t = 2 * pr + k
                if L1:
                    _fin_l1(nc, t, accs[k], pf, ps_fin, idn, oneall, W2a_sb,
                            W2b_sb, b2_sb, adst2_sb, t2slice)
                else:
                    _fin_l2(nc, t, accs[k], pf, outp)


def _fin_l1(nc, t, acc, pf, ps_fin, idn, oneall, W2a_sb, W2b_sb, b2_sb,
            adst2_sb, t2slice):
    # EPS keeps padded dst rows (den=0) finite — their garbage h1 values are
    # never read, but adst2 must stay finite (0*inf = NaN leaks via selT).
    deps = pf.tile([128, 8], F32, tag="deps", name="deps")
    nc.vector.tensor_scalar_add(deps[:], acc[:, 256:264], EPS)
    rec = pf.tile([128, 8], F32, tag="rec", name="rec")
    nc.vector.reciprocal(rec[:], deps[:])
    h1b = pf.tile([128, 256], F16, tag="h1b", name="h1b")
    nc.vector.tensor_tensor(
        out=h1b[:].rearrange("p (h c) -> p h c", h=8),
        in0=acc[:, 0:256].rearrange("p (h c) -> p h c", h=8),
        in1=rec[:, :, None].to_broadcast([128, 8, 32]),
        op=A.mult)
    if t == 0 and getattr(nc, "_dbg", None):
        accs_sb = pf.tile([128, 264], F32, tag="accdbg", name="accdbg")
        nc.vector.tensor_copy(accs_sb[:, 0:256], acc[:, 0:256])
        nc.vector.tensor_copy(accs_sb[:, 256:264], acc[:, 256:264])
        nc.sync.dma_start(out=nc._dbg["h1dbg"][:], in_=accs_sb[:])
    # ho = elu(h1b) + 1 = relu(h1b) + exp(-relu(-h1b)); the -1 is folded
    # into b2eff via W2ext (v1's ACT-based elu decomposition)
    r1 = pf.tile([128, 256], F16, tag="r1", name="r1")
    nc.scalar.activation(r1[:], h1b[:], AF.Relu, scale=-1.0)
    e1 = pf.tile([128, 256], F16, tag="e1", name="e1")
    nc.scalar.activation(e1[:], r1[:], AF.Exp, scale=-1.0)
    rl = pf.tile([128, 256], F16, tag="rl", name="rl")
    nc.scalar.activation(rl[:], h1b[:], AF.Relu)
    ho = pf.tile([128, 256], F16, tag="ho", name="ho")
    nc.gpsimd.tensor_tensor(out=ho[:], in0=rl[:], in1=e1[:], op=A.add)
    h2ps = ps_fin.tile([128, 66], F32, tag="h2ps", name="h2ps")
    for half in (0, 1):
        tp = ps_fin.tile([128, 128], F16, tag="tp", name="tp", bufs=1)
        nc.tensor.transpose(out=tp[:], in_=ho[:, half * 128:(half + 1) * 128],
                            identity=idn[:])
        hoT = pf.tile([128, 128], F16, tag="hoT", name="hoT")
        if half == 0:
            nc.vector.tensor_copy(hoT[:], tp[:])
        else:
            nc.scalar.copy(hoT[:], tp[:])
        nc.tensor.matmul(h2ps[:], lhsT=hoT[:],
                         rhs=(W2a_sb if half == 0 else W2b_sb)[:],
                         start=half == 0, stop=False)
    nc.tensor.matmul(h2ps[:], lhsT=oneall[:], rhs=b2_sb[:], start=False,
                     stop=True)
    t2r = pf.tile([128, 66], F8, tag="t2r", name="t2r")
    nc.scalar.copy(t2r[:, 0:64], h2ps[:, 0:64])
    nc.scalar.copy(t2r[:, 64:66].bitcast(F16), h2ps[:, 64:65])
    nc.vector.tensor_copy(adst2_sb[:, t:t + 1], h2ps[:, 65:66])
    eng = nc.sync if t % 2 == 0 else nc.scalar
    eng.dma_start(out=t2slice[t * 128:(t + 1) * 128, 0:66], in_=t2r[:])


def _fin_l2(nc, t, acc, pf, outp):
    rec = pf.tile([128, 1], F32, tag="rec2", name="rec2")
    nc.vector.reciprocal(rec[:], acc[:, 64:65])
    ot = pf.tile([128, 64], F32, tag="ot", name="ot")
    nc.vector.tensor_scalar_mul(ot[:], acc[:, 0:64], rec[:, 0:1])
    nc.sync.dma_start(out=outp[t * 128:(t + 1) * 128, :], in_=ot[:])


# ---------------------------------------------------------------------------
# entry point
# ---------------------------------------------------------------------------

def make_in_maps(T, common, per_core):
    in_maps = []
    for c in range(NCORES):
        m = {
            "W1ext": common["W1ext"], "W2ext": common["W2ext"],
            "b1ext": common["b1ext"], "b2e66": common["b2e66"],
        }
        pc = per_core[c]
        m.update({k: pc[k] for k in ("xTc", "g1idx", "g2idx", "selT1",
                                     "adidx")})
        in_maps.append(m)
    return in_maps


def kernel(**inputs):
    T, common, per_core = host_prep(inputs)
    nc = build_nc(T)
    in_maps = make_in_maps(T, common, per_core)
    res = run_bass_kernel_spmd(nc, in_maps, core_ids=list(range(NCORES)))
    allrows = np.concatenate([res.results[c]["out"] for c in range(NCORES)],
                             axis=0)
    return allrows[common["slotrow"]].astype(np.float32)



# revision 55
# speedup vs baseline: 1.2307x; 1.0073x over previous
"""Two-layer GAT on 8 Trainium2 NeuronCores (Bass/Tile) — v2.

Changes vs v1 (889us):
  - Phase A sharded: each core computes T1 rows for its own 6250 nodes
    (padded to 6272), AllGather broadcasts the table (timing build: local
    copy of own contribution only).
  - b1 folded into T1 value columns (sum(alpha)=1), b2 and the elu "-1"
    folded into W2ext/b2eff at finalize.
  - selT (dst-onehot, d-on-partitions) shipped from host as fp8 and used
    directly as matmul lhsT (mixed fp8 x f16 matmul) — kills the dlT DMA,
    the DVE selT build, and the adf path stays cheap.
  - exp broadcast stays on ACT; leaky/et on DVE; PSUM->SBUF copies and
    t2r/adst2 finalization moved to Pool (gpsimd).
  - softmax denominator accumulated via a second per-chunk matmul with a
    strided rhs view of exx (no den-copy op).
  - asrc2/adst2 computed via the h2 matmul itself (W2ext has v_s|v_d cols).
"""

import os

import numpy as np

import concourse.bass as bass
import concourse.bacc as bacc
import concourse.tile as tile
import concourse.mybir as mybir
from concourse.bass_utils import run_bass_kernel_spmd
from concourse.masks import make_identity

F32 = mybir.dt.float32
F16 = mybir.dt.float16
F8 = mybir.dt.float8e4
I16 = mybir.dt.int16
I32 = mybir.dt.int32
A = mybir.AluOpType
AF = mybir.ActivationFunctionType
NP_F8 = mybir.dt.np(F8)

# -------- problem constants --------
N, E, IN, HID, OUT, H = 50000, 800000, 128, 32, 64, 8
C1 = H * HID  # 256
NCORES = 8
NPC = N // NCORES        # 6250 dst nodes per core
RPC = 6272               # T1 rows per core (6250 padded to 49*128)
NR = NCORES * RPC        # 50176 T1 rows
T1_LO = 4 * RPC          # 25088: rows of cores 0-3
T1_W = 384               # T1 row stride (768B) — gather granularity
CHL = 9                  # chunks per (tile, half)
CH = 2 * CHL             # chunks per gather call
CALLW = CH * 128         # 2304 edge slots per call
EPS = 1e-16
NEG = 0.2


def _row_of(n):
    """T1 row of node n (cores own contiguous 6250-node ranges, padded)."""
    c = n // NPC
    return c * RPC + (n - c * NPC)


# ---------------------------------------------------------------------------
# host-side preprocessing
# ---------------------------------------------------------------------------

def _prep_weights(W1, as1, ad1, b1, W2, as2, ad2, b2):
    As = np.zeros((C1, H), np.float32)
    Ad = np.zeros((C1, H), np.float32)
    for h in range(H):
        As[h * HID:(h + 1) * HID, h] = as1[h]
        Ad[h * HID:(h + 1) * HID, h] = ad1[h]
    W1ext = np.concatenate([W1, W1 @ As, W1 @ Ad], axis=1)  # [128, 272]
    iotarep = np.zeros((128, 128 * CH), np.float16)
    for d in range(128):
        iotarep[:, d * CH:(d + 1) * CH] = d
    b1ext = np.zeros((272,), np.float32)
    b1ext[:C1] = b1
    vs = W2 @ as2[0]   # [256]
    vd = W2 @ ad2[0]   # [256]
    W2ext = np.concatenate([W2, vs[:, None], vd[:, None]], axis=1)  # [256,66]
    b2eff = np.concatenate([b2 - W2.sum(0), [-vs.sum()], [-vd.sum()]])  # [66]
    return {
        "W1ext": W1ext.astype(np.float16),
        "W2ext": W2ext.astype(np.float16),
        "b1ext": np.tile(b1ext[None, :], (128, 1)).astype(np.float16),
        "b2e66": np.tile(b2eff[None, :], (128, 1)).astype(np.float16),
        "iotarep": iotarep,
    }


def _greedy_tiles(deg_lo1, deg_hi1, deg_lo2, deg_hi2):
    cap = CHL * 128
    tiles = []
    i, n = 0, len(deg_lo1)
    while i < n:
        l1 = h1 = l2 = h2 = 0
        j = i
        while j < n and j - i < 128:
            nl1, nh1 = l1 + deg_lo1[j], h1 + deg_hi1[j]
            nl2, nh2 = l2 + deg_lo2[j], h2 + deg_hi2[j]
            if nl1 > cap or nh1 > cap or nl2 > cap or nh2 > cap:
                break
            l1, h1, l2, h2 = nl1, nh1, nl2, nh2
            j += 1
        assert j > i, "single node exceeds chunk caps"
        tiles.append((i, j))
        i = j
    return tiles


def _pack_calls(rows_half, dloc_half, T, both=False):
    """Build gather idx [128, T*CH*8] i16, dloc [128, T*CH] f16, and the
    fp8 one-hot stream: selT only [128, T*CH*128], or selT|sel interleaved
    per call [128, T*2*CH*128] when both=True."""
    idx16 = np.zeros((128, T * CH * 8), np.int16)
    dloc = np.full((128, T * CH), -1, np.float16)
    dlocT_flat = np.full((T * CALLW,), -1, np.int32)
    for pr in range(T // 2):
        for h in (0, 1):
            g = 2 * pr + h
            rows = np.zeros((CALLW,), np.int64)
            dl = np.full((CALLW,), -1, np.int64)
            for k, t in enumerate((2 * pr, 2 * pr + 1)):
                r = rows_half.get((t, h))
                if r is None:
                    continue
                d = dloc_half[(t, h)]
                off = k * CHL * 128
                rows[off:off + len(r)] = r
                dl[off:off + len(r)] = d
            blk = rows.reshape(CH * 8, 16).T.astype(np.int16)
            idx16[:, g * CH * 8:(g + 1) * CH * 8] = np.tile(blk, (8, 1))
            dloc[:, g * CH:(g + 1) * CH] = dl.reshape(CH, 128).T.astype(np.float16)
            dlocT_flat[g * CALLW:(g + 1) * CALLW] = dl
    # selT[p, g*CH*128 + j*128 + e] = 1.0 iff dloc(edge (j,e) of g) == p
    selT = (dlocT_flat[None, :] == np.arange(128)[:, None])
    if not both:
        return idx16, dloc, selT.astype(NP_F8)
    # sel[p, g, j, d] = 1.0 iff dloc(edge (p,j) of g) == d
    sel = (dloc[:, :, None].astype(np.int32) ==
           np.arange(128)[None, None, :]).reshape(128, T, CH, 128)
    selB = np.stack([selT.reshape(128, T, CH, 128), sel], axis=2).reshape(
        128, T * 2 * CH * 128).astype(NP_F8)
    return idx16, dloc, selB


def _prep_core(c, src, dst):
    base = c * NPC
    own = (dst >= base) & (dst < base + NPC)
    s = src[own].astype(np.int64)
    d = (dst[own] - base).astype(np.int64)
    order = np.argsort(d, kind="stable")
    s, d = s[order], d[order]
    ptr = np.zeros(NPC + 1, np.int64)
    np.cumsum(np.bincount(d, minlength=NPC), out=ptr[1:])

    srow = _row_of(s)
    lo1m = srow < T1_LO
    lo2m = s < (N // 2)  # slot-major half split: cores 0-3 vs 4-7
    deg_lo1 = np.bincount(d, weights=lo1m, minlength=NPC).astype(np.int64)
    deg_hi1 = np.bincount(d, weights=~lo1m, minlength=NPC).astype(np.int64)
    deg_lo2 = np.bincount(d, weights=lo2m, minlength=NPC).astype(np.int64)
    deg_hi2 = np.bincount(d, weights=~lo2m, minlength=NPC).astype(np.int64)
    tiles = _greedy_tiles(deg_lo1, deg_hi1, deg_lo2, deg_hi2)

    rows1, dloc1 = {}, {}
    for t, (n0, n1) in enumerate(tiles):
        e0, e1 = ptr[n0], ptr[n1]
        es, ed = srow[e0:e1], d[e0:e1]
        dl = ed - n0
        m1 = es < T1_LO
        rows1[(t, 0)] = es[m1]
        dloc1[(t, 0)] = dl[m1]
        rows1[(t, 1)] = es[~m1] - T1_LO
        dloc1[(t, 1)] = dl[~m1]
    return {
        "Treal": len(tiles), "tiles": tiles, "s": s, "d": d, "ptr": ptr,
        "rows1": rows1, "dloc1": dloc1,
    }


def _finish_core(pc, c, T, slotrow):
    tiles = list(pc["tiles"]) + [(0, 0)] * (T - pc["Treal"])
    idx1, dloc1, selT1 = _pack_calls(pc["rows1"], pc["dloc1"], T, both=True)

    t2lo = 4 * T * 128
    s, d, ptr = pc["s"], pc["d"], pc["ptr"]
    srow = slotrow[s]
    rows2, dloc2 = {}, {}
    for t, (n0, n1) in enumerate(pc["tiles"]):
        e0, e1 = ptr[n0], ptr[n1]
        dl = d[e0:e1] - n0
        m2 = srow[e0:e1] < t2lo
        rows2[(t, 0)] = srow[e0:e1][m2]
        dloc2[(t, 0)] = dl[m2]
        rows2[(t, 1)] = srow[e0:e1][~m2] - t2lo
        dloc2[(t, 1)] = dl[~m2]
    # idx2 only: the layer-2 slot layout (dloc/sel/selT) is identical to
    # layer-1's by construction — both split halves by core(src) < 4 and
    # keep the same dst-sorted order — so selT1 is reused in phase D.
    idx2, dloc2a, _ = _pack_calls(rows2, dloc2, T)
    assert all(np.array_equal(dloc2[k], pc["dloc1"][k]) for k in dloc2)

    # adidx: local t1slice rows of each tile's nodes (slot-major, clamped),
    # in dma_gather int16 index layout, one call of T*128 rows
    p = np.arange(128)
    rows = np.zeros((T * 128,), np.int64)
    for t, (n0, n1) in enumerate(tiles):
        w = n1 - n0
        rows[t * 128:(t + 1) * 128] = n0 + np.minimum(p, max(w - 1, 0))
    blk = rows.reshape(T * 8, 16).T.astype(np.int16)
    adidx = np.tile(blk, (8, 1))  # [128, T*8]
    return {
        "g1idx": idx1, "dloc1": dloc1, "selT1": selT1,
        "g2idx": idx2,
        "adidx": adidx,
    }


def host_prep(inputs):
    ei = np.asarray(inputs["edge_index"]).astype(np.int64)
    wd = _prep_weights(
        np.asarray(inputs["W1"], np.float32),
        np.asarray(inputs["att_src1"], np.float32),
        np.asarray(inputs["att_dst1"], np.float32),
        np.asarray(inputs["b1"], np.float32),
        np.asarray(inputs["W2"], np.float32),
        np.asarray(inputs["att_src2"], np.float32),
        np.asarray(inputs["att_dst2"], np.float32),
        np.asarray(inputs["b2"], np.float32),
    )
    loops = np.arange(N, dtype=np.int64)
    src = np.concatenate([ei[0], loops])
    dst = np.concatenate([ei[1], loops])

    x = np.asarray(inputs["x"], np.float32).astype(np.float16)

    cores = [_prep_core(c, src, dst) for c in range(NCORES)]
    T = max(pc["Treal"] for pc in cores)
    if T % 2:
        T += 1
    slotrow = np.zeros(N, np.int64)
    for c, pc in enumerate(cores):
        base_row = c * T * 128
        for t, (n0, n1) in enumerate(pc["tiles"]):
            nodes = c * NPC + np.arange(n0, n1)
            slotrow[nodes] = base_row + t * 128 + np.arange(n1 - n0)
    per_core = [_finish_core(pc, c, T, slotrow) for c, pc in enumerate(cores)]
    for c in range(NCORES):
        xc = np.zeros((IN, RPC), np.float16)
        xc[:, :NPC] = x[c * NPC:(c + 1) * NPC].T
        per_core[c]["xTc"] = xc

    common = dict(wd)
    common["slotrow"] = slotrow
    return T, common, per_core


# ---------------------------------------------------------------------------
# device program
# ---------------------------------------------------------------------------

def _gather_raw(eng, out_ap, in_ap, idxs_ap, num_idxs, elem_size, elem_step):
    """dma_gather with elem_size_bytes not a multiple of 256B (non-transpose
    path only; the 256B rule is a transpose-mode restriction — the Q7 kernel
    packets arbitrary elem sizes, only the row stride is encoded in 256B
    units).  Mirrors BassGpSimd.dma_gather's construction."""
    from concourse.ap_utils import ap_is_contiguous
    import concourse.mybir as mb
    assert idxs_ap.dtype == mybir.dt.int16
    assert in_ap.dtype == out_ap.dtype
    elem_size_bytes = elem_size * mybir.dt.size(in_ap.dtype)
    assert in_ap.ap[-1][1] == out_ap.ap[-1][1] == elem_size
    assert ap_is_contiguous(out_ap.ap[1:])
    assert ap_is_contiguous(idxs_ap.ap[1:])
    assert in_ap.ap[0][0] == elem_step
    stride_bytes = elem_step * mybir.dt.size(in_ap.dtype)
    assert stride_bytes % 256 == 0 and stride_bytes // 256 < 256
    _in_ap = eng.lower_ap_dma(in_ap, for_custom_bir_dma=True)
    _idxs_ap = eng.lower_ap(idxs_ap)
    _out_ap = eng.lower_ap(out_ap)
    return eng.add_instruction(
        mb.InstDMAGatherAnt(
            name=eng.bass.get_next_instruction_name(),
            ins=[*_in_ap, _idxs_ap,
                 eng.lower_val_access(eng.to_reg(num_idxs))],
            outs=[_out_ap],
            transpose=False,
            num_idxs=num_idxs,
            elem_size=elem_size,
            stride_bytes_256=stride_bytes // 256,
            gen_mode=0,
            single_packet=False,
            queue_num=0,
            sbuf_tokens_per_rank=0,
            sbuf_free_dim_per_rank=0,
            sbuf_free_dim_pad_per_rank=0,
            sbuf_byte_offset=0,
        )
    )


def build_nc(T, num_devices=NCORES, with_collective=True, phases="ABCD",
             dbg=False):
    nc = bacc.Bacc("TRN2", target_bir_lowering=False, debug=False,
                   num_devices=num_devices)
    dt = nc.dram_tensor
    xTc = dt("xTc", [IN, RPC], F16, kind="ExternalInput").ap()
    W1ext = dt("W1ext", [128, 272], F16, kind="ExternalInput").ap()
    W2ext = dt("W2ext", [256, 66], F16, kind="ExternalInput").ap()
    b1ext = dt("b1ext", [128, 272], F16, kind="ExternalInput").ap()
    b2e66 = dt("b2e66", [128, 66], F16, kind="ExternalInput").ap()
    g1idx = dt("g1idx", [128, T * CH * 8], I16, kind="ExternalInput").ap()
    g2idx = dt("g2idx", [128, T * CH * 8], I16, kind="ExternalInput").ap()
    selT1 = dt("selT1", [128, T * 2 * CH * 128], F8, kind="ExternalInput").ap()
    adidx = dt("adidx", [128, T * 8], I16, kind="ExternalInput").ap()
    t1slice = dt("t1slice", [RPC, T1_W], F16, kind="Internal").ap()
    T1 = dt("T1", [NR, T1_W], F16, kind="Internal",
            addr_space="Shared" if with_collective else "Local").ap()
    t2rows = T * 128
    # t2 rows are fp8: 64 fp8 h2 values + asrc2 as raw f16 in bytes 64:66,
    # padded to a 256B stride (gather stride must be a 256B multiple). The
    # 66B gather elem rides the 7ns/desc floor instead of f16's 11.6ns.
    t2slice = dt("t2slice", [t2rows, 256], F8, kind="Internal").ap()
    t2full = dt("t2full", [NCORES * t2rows, 256], F8, kind="Internal",
                addr_space="Shared" if with_collective else "Local").ap()
    outp = dt("out", [t2rows, 64], F16, kind="ExternalOutput").ap()
    if dbg:
        t1dbg = dt("t1dbg", [RPC, T1_W], F16, kind="ExternalOutput").ap()
        addbg = dt("addbg", [128, T * 8], F16, kind="ExternalOutput").ap()
        t2dbg = dt("t2dbg", [t2rows, 128], F16, kind="ExternalOutput").ap()
        gtdbg = dt("gtdbg", [128, CH * 264], F16, kind="ExternalOutput").ap()
        etdbg = dt("etdbg", [128, CH * 8], F16, kind="ExternalOutput").ap()
        wdbg = dt("wdbg", [128, CH * 256], F16, kind="ExternalOutput").ap()
        lkdbg = dt("lkdbg", [128, CH * 8], F16, kind="ExternalOutput").ap()
        exxdbg = dt("exxdbg", [128, CH * 256], F16,
                    kind="ExternalOutput").ap()
        h1dbg = dt("h1dbg", [128, 256 + 8], F32, kind="ExternalOutput").ap()
        nc._dbg = dict(gtdbg=gtdbg, etdbg=etdbg, wdbg=wdbg, h1dbg=h1dbg,
                       lkdbg=lkdbg, exxdbg=exxdbg)
    else:
        nc._dbg = None

    with tile.TileContext(nc) as tc:
        with tc.tile_pool(name="consts", bufs=1) as cp:
            W1e_sb = cp.tile([128, 272], F16)
            nc.sync.dma_start(out=W1e_sb[:], in_=W1ext[:])
            W2a_sb = cp.tile([128, 66], F16)
            nc.sync.dma_start(out=W2a_sb[:], in_=W2ext[0:128, :])
            W2b_sb = cp.tile([128, 66], F16)
            nc.sync.dma_start(out=W2b_sb[:], in_=W2ext[128:256, :])
            b1_sb = cp.tile([128, 272], F16)
            nc.sync.dma_start(out=b1_sb[:], in_=b1ext[:])
            b2_sb = cp.tile([128, 66], F16)
            nc.sync.dma_start(out=b2_sb[:], in_=b2e66[:])
            oneall = cp.tile([128, 128], F16)
            nc.vector.memset(oneall[:], 1.0 / 128.0)
            idn = cp.tile([128, 128], F16)
            make_identity(nc, idn[:])
            g1i_sb = cp.tile([128, T * CH * 8], I16)
            nc.sync.dma_start(out=g1i_sb[:], in_=g1idx[:])
            g2i_sb = cp.tile([128, T * CH * 8], I16)
            nc.sync.dma_start(out=g2i_sb[:], in_=g2idx[:])
            adidx_sb = cp.tile([128, T * 8], I16)
            nc.sync.dma_start(out=adidx_sb[:], in_=adidx[:])
            adtall_sb = cp.tile([128, T, 8], F16)  # bulk a_dst gather target
            adst2_sb = cp.tile([128, T], F16)  # written in B-fin, read in D

            # ---------------- Phase A: own T1 slice ----------------
            if "A" in phases:
                with tc.tile_pool(name="pa", bufs=2) as pa, \
                     tc.tile_pool(name="paps", bufs=4, space="PSUM") as paps:
                    XB = 2048
                    nblk = (RPC + XB - 1) // XB
                    for blk in range(nblk):
                        n0 = blk * XB
                        bw = min(XB, RPC - n0)
                        nt = bw // 128
                        xb = pa.tile([128, XB], F16, tag="xb", name="xb")
                        nc.sync.dma_start(out=xb[:, 0:bw],
                                          in_=xTc[:, n0:n0 + bw])
                        t1b = pa.tile([128, 16, 272], F16, tag="t1b",
                                      name="t1b")
                        for i in range(nt):
                            ps = paps.tile([128, 272], F32, tag="aps",
                                           name="aps")
                            nc.tensor.matmul(ps[:],
                                             lhsT=xb[:, i * 128:(i + 1) * 128],
                                             rhs=W1e_sb[:], start=True,
                                             stop=False)
                            nc.tensor.matmul(ps[:], lhsT=oneall[:],
                                             rhs=b1_sb[:], start=False,
                                             stop=True)
                            if i % 2 == 0:
                                nc.vector.tensor_copy(t1b[:, i, :], ps[:])
                            else:
                                nc.scalar.copy(t1b[:, i, :], ps[:])
                        nc.sync.dma_start(
                            out=t1slice[n0:n0 + bw, 0:272].rearrange(
                                "(i p) c -> p i c", p=128),
                            in_=t1b[:, 0:nt, :])
                        if not with_collective and "B" in phases:
                            nc.sync.dma_start(
                                out=T1[n0:n0 + bw, 0:272],
                                in_=t1slice[n0:n0 + bw, 0:272])

            # ---------------- AllGather T1 ----------------
            if "B" in phases:
                if with_collective:
                    nc.gpsimd.collective_compute(
                        "AllGather", A.bypass,
                        replica_groups=[list(range(NCORES))],
                        ins=[t1slice[:]], outs=[T1[:]],
                    )
                # bulk a_dst gather: one call for all T tiles' 128 slots
                _gather_raw(nc.gpsimd, adtall_sb[:],
                            t1slice[0:RPC, 264:272], adidx_sb[:],
                            T * 128, 8, T1_W)

                # ---------------- Phase B: layer-1 aggregation ----------------
                _agg_layer(nc, tc, T, layer=1,
                           tbl_lo=T1[0:T1_LO, 0:264],
                           tbl_hi=T1[T1_LO:NR, 0:264],
                           gidx_sb=g1i_sb, selT_in=selT1,
                           idn=idn, oneall=oneall,
                           adtall_sb=adtall_sb,
                           W2a_sb=W2a_sb, W2b_sb=W2b_sb, b2_sb=b2_sb,
                           adst2_sb=adst2_sb,
                           t2slice=t2slice, outp=None)

            if "D" in phases:
                nc.sync.dma_start(out=g2i_sb[:], in_=g2idx[:])

            if dbg:
                nc.sync.dma_start(out=t1dbg[:], in_=t1slice[:])
                nc.sync.dma_start(
                    out=addbg[:],
                    in_=adtall_sb[:].rearrange("p t c -> p (t c)"))
                nc.sync.dma_start(out=t2dbg[:], in_=t2slice[:])

            # ---------------- AllGather T2 ----------------
            if "C" in phases:
                if with_collective:
                    nc.gpsimd.collective_compute(
                        "AllGather", A.bypass,
                        replica_groups=[list(range(NCORES))],
                        ins=[t2slice[:]], outs=[t2full[:]],
                    )
                else:
                    nc.sync.dma_start(out=t2full[0:t2rows, :], in_=t2slice[:])

            # ---------------- Phase D: layer-2 aggregation ----------------
            if "D" in phases:
                _agg_layer(nc, tc, T, layer=2,
                           tbl_lo=t2full[0:4 * t2rows, 0:68],
                           tbl_hi=t2full[4 * t2rows:8 * t2rows, 0:68],
                           gidx_sb=g2i_sb, selT_in=selT1,
                           idn=idn, oneall=None,
                           adtall_sb=None,
                           W2a_sb=None, W2b_sb=None, b2_sb=None,
                           adst2_sb=adst2_sb,
                           t2slice=None, outp=outp)

    nc.compile()
    return nc


def _agg_layer(nc, tc, T, layer, tbl_lo, tbl_hi, gidx_sb, selT_in,
               idn, oneall, adtall_sb, W2a_sb, W2b_sb,
               b2_sb, adst2_sb, t2slice, outp):
    L1 = layer == 1
    GW = 264 if L1 else 68   # gathered elements per row (payload)
    GS = T1_W if L1 else 256  # table row stride in elements
    NH = 8 if L1 else 1
    VC = 256 if L1 else 64
    ACC_W = 264 if L1 else 65
    name = f"l{layer}"
    PBB = int(os.environ.get("V2_PBB", "4"))
    ACCB = int(os.environ.get("V2_ACCB", "3"))
    with tc.tile_pool(name=f"pb_{name}", bufs=PBB) as pb, \
         tc.tile_pool(name=f"pf_{name}", bufs=2) as pf, \
         tc.tile_pool(name=f"ps_acc_{name}", bufs=ACCB, space="PSUM") as ps_acc, \
         tc.tile_pool(name=f"ps_ad_{name}", bufs=2, space="PSUM") as ps_ad, \
         tc.tile_pool(name=f"ps_fin_{name}", bufs=2, space="PSUM") as ps_fin:
        for pr in range(T // 2):
            accs = [ps_acc.tile([128, ACC_W], F32, tag="acc", name="acc_a"),
                    ps_acc.tile([128, ACC_W], F32, tag="acc", name="acc_b")]
            for hf in (0, 1):
                g = 2 * pr + hf
                # both layers share the same tiles and edge-slot layout, so
                # the selT|sel fp8 stream is shipped from the SAME host
                # tensor (sel2 == sel1, selT2 == selT1). Layer 1 ships both
                # halves; layer 2 ships only selT and rebuilds sel on DVE
                # (is_equal against an iota grid) to take load off the DMA
                # engines, which are the wall.
                scp = pb.tile([128, 2, CH, 128], F8, tag="scp",
                              name="scp", bufs=6)
                nc.sync.dma_start(
                    out=scp[:].rearrange("p s j e -> p (s j e)"),
                    in_=selT_in[:, g * 2 * CALLW:(g + 1) * 2 * CALLW])
                s8 = scp[:, 0]     # selT: [d-part, j, e]
                sel8 = scp[:, 1]   # sel:  [e-part, j, d]
                gt = pb.tile([128, CH, GW], F16 if L1 else F8, tag="gt",
                             name="gt", bufs=5)
                if os.environ.get("V2_GSPLIT", "0") == "1":
                    for gh in (0, 1):
                        _gather_raw(
                            nc.gpsimd, gt[:, gh * CHL:(gh + 1) * CHL],
                            tbl_lo if hf == 0 else tbl_hi,
                            gidx_sb[:, g * CH * 8 + gh * CHL * 8:
                                    g * CH * 8 + (gh + 1) * CHL * 8],
                            CHL * 128, GW, GS)
                else:
                    _gather_raw(
                        nc.gpsimd, gt[:], tbl_lo if hf == 0 else tbl_hi,
                        gidx_sb[:, g * CH * 8:(g + 1) * CH * 8],
                        CALLW, GW, GS)
                # per-edge et = a_dst + a_src entirely in PSUM: the fp8 selT
                # one-hot matmul broadcasts a_dst, then an identity matmul
                # accumulates the gathered a_src columns on top.
                adps = ps_ad.tile([128, CH, NH], F32, tag="adps", name="adps")
                asrc_ap = (gt[:, :, 256:264] if L1
                           else gt[:, :, 64:66].bitcast(F16))
                for j in range(CH):
                    t = 2 * pr + (0 if j < CHL else 1)
                    rhs = adtall_sb[:, t, :] if L1 else adst2_sb[:, t:t + 1]
                    nc.tensor.matmul(adps[:, j, :], lhsT=s8[:, j, :], rhs=rhs,
                                     start=True, stop=False)
                    nc.tensor.matmul(adps[:, j, :], lhsT=idn[:],
                                     rhs=asrc_ap[:, j, :], start=False,
                                     stop=True)
                lk = pb.tile([128, CH, NH], F16, tag="lk", name="lk")
                nc.scalar.activation(lk[:], adps[:], AF.Prelu, alpha=NEG)
                # exp at pair width on ACT (cheap), broadcast to the value
                # width inside the DVE multiply via a stride-0 middle dim —
                # the last dim stays packed so the mult keeps 2x DVE mode.
                exf = pb.tile([128, CH, NH, 2], F16, tag="exf", name="exf")
                nc.scalar.activation(
                    exf[:], lk[:, :, :, None].to_broadcast([128, CH, NH, 2]),
                    AF.Exp)
                w = pb.tile([128, CH, ACC_W], F16, tag="w", name="w")
                nc.vector.tensor_copy(w[:, :, VC:ACC_W], exf[:, :, :, 0])
                cph = VC // NH // 2  # 16 (L1) / 32 (L2) value pairs per head
                nc.vector.tensor_tensor(
                    out=w[:, :, 0:VC].rearrange("p j (h k two) -> p j h k two",
                                                h=NH, two=2),
                    in0=gt[:, :, 0:VC].rearrange("p j (h k two) -> p j h k two",
                                                 h=NH, two=2),
                    in1=exf[:, :, :, None, :].to_broadcast(
                        [128, CH, NH, cph, 2]),
                    op=A.mult)
                for j in range(CH):
                    acc = accs[0 if j < CHL else 1]
                    st = (hf == 0) and (j % CHL == 0)
                    sp = (hf == 1) and (j % CHL == CHL - 1)
                    nc.tensor.matmul(acc[:], lhsT=sel8[:, j, :],
                                     rhs=w[:, j, :], start=st, stop=sp)
            for k in (0, 1):
                t = 2 * pr + k
                if L1:
                    _fin_l1(nc, t, accs[k], pf, ps_fin, idn, oneall, W2a_sb,
                            W2b_sb, b2_sb, adst2_sb, t2slice)
                else:
                    _fin_l2(nc, t, accs[k], pf, outp)


def _fin_l1(nc, t, acc, pf, ps_fin, idn, oneall, W2a_sb, W2b_sb, b2_sb,
            adst2_sb, t2slice):
    # EPS keeps padded dst rows (den=0) finite — their garbage h1 values are
    # never read, but adst2 must stay finite (0*inf = NaN leaks via selT).
    deps = pf.tile([128, 8], F32, tag="deps", name="deps")
    nc.vector.tensor_scalar_add(deps[:], acc[:, 256:264], EPS)
    rec = pf.tile([128, 8], F32, tag="rec", name="rec")
    nc.vector.reciprocal(rec[:], deps[:])
    h1b = pf.tile([128, 256], F16, tag="h1b", name="h1b")
    nc.vector.tensor_tensor(
        out=h1b[:].rearrange("p (h c) -> p h c", h=8),
        in0=acc[:, 0:256].rearrange("p (h c) -> p h c", h=8),
        in1=rec[:, :, None].to_broadcast([128, 8, 32]),
        op=A.mult)
    if t == 0 and getattr(nc, "_dbg", None):
        accs_sb = pf.tile([128, 264], F32, tag="accdbg", name="accdbg")
        nc.vector.tensor_copy(accs_sb[:, 0:256], acc[:, 0:256])
        nc.vector.tensor_copy(accs_sb[:, 256:264], acc[:, 256:264])
        nc.sync.dma_start(out=nc._dbg["h1dbg"][:], in_=accs_sb[:])
    # ho = elu(h1b) + 1 = relu(h1b) + exp(-relu(-h1b)); the -1 is folded
    # into b2eff via W2ext (v1's ACT-based elu decomposition)
    r1 = pf.tile([128, 256], F16, tag="r1", name="r1")
    nc.scalar.activation(r1[:], h1b[:], AF.Relu, scale=-1.0)
    e1 = pf.tile([128, 256], F16, tag="e1", name="e1")
    nc.scalar.activation(e1[:], r1[:], AF.Exp, scale=-1.0)
    rl = pf.tile([128, 256], F16, tag="rl", name="rl")
    nc.scalar.activation(rl[:], h1b[:], AF.Relu)
    ho = pf.tile([128, 256], F16, tag="ho", name="ho")
    nc.gpsimd.tensor_tensor(out=ho[:], in0=rl[:], in1=e1[:], op=A.add)
    h2ps = ps_fin.tile([128, 66], F32, tag="h2ps", name="h2ps")
    for half in (0, 1):
        tp = ps_fin.tile([128, 128], F16, tag="tp", name="tp", bufs=1)
        nc.tensor.transpose(out=tp[:], in_=ho[:, half * 128:(half + 1) * 128],
                            identity=idn[:])
        hoT = pf.tile([128, 128], F16, tag="hoT", name="hoT")
        if half == 0:
            nc.vector.tensor_copy(hoT[:], tp[:])
        else:
            nc.scalar.copy(hoT[:], tp[:])
        nc.tensor.matmul(h2ps[:], lhsT=hoT[:],
                         rhs=(W2a_sb if half == 0 else W2b_sb)[:],
                         start=half == 0, stop=False)
    nc.tensor.matmul(h2ps[:], lhsT=oneall[:], rhs=b2_sb[:], start=False,
                     stop=True)
    t2r = pf.tile([128, 66], F8, tag="t2r", name="t2r")
    nc.scalar.copy(t2r[:, 0:64], h2ps[:, 0:64])
    nc.scalar.copy(t2r[:, 64:66].bitcast(F16), h2ps[:, 64:65])
    nc.vector.tensor_copy(adst2_sb[:, t:t + 1], h2ps[:, 65:66])
    eng = nc.sync if t % 2 == 0 else nc.scalar
    eng.dma_start(out=t2slice[t * 128:(t + 1) * 128, 0:66], in_=t2r[:])


def _fin_l2(nc, t, acc, pf, outp):
    rec = pf.tile([128, 1], F32, tag="rec2", name="rec2")
    nc.vector.reciprocal(rec[:], acc[:, 64:65])
    ot = pf.tile([128, 64], F16, tag="ot", name="ot")
    nc.vector.tensor_scalar_mul(ot[:], acc[:, 0:64], rec[:, 0:1])
    nc.sync.dma_start(out=outp[t * 128:(t + 1) * 128, :], in_=ot[:])


# ---------------------------------------------------------------------------
# entry point
# ---------------------------------------------------------------------------

def make_in_maps(T, common, per_core):
    in_maps = []
    for c in range(NCORES):
        m = {
            "W1ext": common["W1ext"], "W2ext": common["W2ext"],
            "b1ext": common["b1ext"], "b2e66": common["b2e66"],
        }
        pc = per_core[c]
        m.update({k: pc[k] for k in ("xTc", "g1idx", "g2idx", "selT1",
                                     "adidx")})
        in_maps.append(m)
    return in_maps


def kernel(**inputs):
    T, common, per_core = host_prep(inputs)
    nc = build_nc(T)
    in_maps = make_in_maps(T, common, per_core)
    res = run_bass_kernel_spmd(nc, in_maps, core_ids=list(range(NCORES)))
    allrows = np.concatenate([res.results[c]["out"] for c in range(NCORES)],
                             axis=0)
    return allrows[common["slotrow"]].astype(np.float32)

